# revision 1
# baseline (speedup 1.0000x reference)
"""AdapterGNN on 8 TRN2 NeuronCores.

Strategy (dst-node sharding):
  - Nodes sharded: core c owns nodes [c*6250, (c+1)*6250). All edges whose dst
    belongs to core c are processed by core c (~100k edges/core).
  - Per layer: AllGather replicates h (node-major, padded to 6272 rows/core) to
    every core; each core dma_gather's its edges' source rows (token stream,
    grouped by 128-node dst block), then reduces tokens -> nodes with a
    one-hot matmul on TensorE (segment-sum, f32 PSUM accumulation, race-free).
  - All per-node compute (conv matmul, adapters, BN apply) runs feature-major
    ([64, nodes] in SBUF).  BatchNorm statistics are computed as per-core
    partial sums + a tiny AllReduce ([64,6] per layer).
  - Linear biases feeding straight into BatchNorm (b, pb2) cancel exactly and
    are skipped; gating is folded into the adapter BN affine on the host.
"""

import math
import os
import sys

import numpy as np

sys.path.insert(0, "/opt/trn_rl_repo")

from concourse import bass, mybir  # noqa: E402
from concourse.bacc import Bacc  # noqa: E402
from concourse.bass_utils import run_bass_kernel_spmd  # noqa: E402

F32 = mybir.dt.float32
I16 = mybir.dt.int16
AX = mybir.AxisListType.X
ALU = mybir.AluOpType
ACTF = mybir.ActivationFunctionType

EPS = 1e-5
TRUNC = None  # debug: 'B' | 'C' | 'E' | None


def _r128(v):
    return ((int(v) + 127) // 128) * 128


class Cfg:
    def __init__(self, N, E, D, L, BOT, NC, SPLIT, CH_BLOCKS, CAPA, CAPB):
        self.N, self.E, self.D, self.L, self.BOT = N, E, D, L, BOT
        self.NC, self.SPLIT = NC, SPLIT
        self.NPC = N // NC                      # real nodes per core
        self.NBLK = (self.NPC + 127) // 128     # dst blocks per core
        self.NPAD = self.NBLK * 128             # padded nodes per core
        self.GN = NC * self.NPAD                # padded global rows
        self.CAPA, self.CAPB = CAPA, CAPB       # tokens per block (A/B region)
        self.TPA, self.TPB = CAPA // 128, CAPB // 128
        self.ASZ = self.NBLK * CAPA             # B region token offset
        self.ASZT = self.NBLK * self.TPA        # B region tile offset
        self.NTOK = self.NBLK * (CAPA + CAPB)
        self.NTILES = self.NTOK // 128
        # gather chunks: groups of dst blocks
        self.chunks = []
        b = 0
        while b < self.NBLK:
            nb = min(CH_BLOCKS, self.NBLK - b)
            self.chunks.append((b, nb))
            b += nb
        self.NCHL = len(self.chunks)
        self.MAXCB = max(nb for _, nb in self.chunks)
        # phase-C column chunks over [0, NPAD)
        self.kchunks = []
        off = 0
        while off < self.NPAD:
            w = min(512, self.NPAD - off)
            self.kchunks.append((off, w))
            off += w
        self.NKC = len(self.kchunks)


def _prep_tokens(cfg, edge_index):
    """Per-core token streams: gather idx (wrapped int16) + dst_rel (f32)."""
    c_ = cfg
    src = edge_index[0].astype(np.int64)
    dst = edge_index[1].astype(np.int64)
    owner = dst // c_.NPC
    dloc = dst - owner * c_.NPC
    blk = dloc >> 7
    rel = (dloc & 127).astype(np.float32)
    gsrc = (src // c_.NPC) * c_.NPAD + (src % c_.NPC)
    isB = (gsrc >= c_.SPLIT).astype(np.int64)

    key = (owner * c_.NBLK + blk) * 2 + isB
    order = np.argsort(key, kind="stable")
    skey = key[order]
    counts = np.bincount(key, minlength=c_.NC * c_.NBLK * 2)
    starts = np.concatenate([[0], np.cumsum(counts)[:-1]])
    rank = np.arange(c_.E) - starts[skey]

    core = skey // (2 * c_.NBLK)
    remk = skey % (2 * c_.NBLK)
    b2 = remk // 2
    piece = remk % 2
    pos = np.where(piece == 0, b2 * c_.CAPA + rank,
                   c_.ASZ + b2 * c_.CAPB + rank)

    gs = gsrc[order]
    gs = np.where(piece == 1, gs - c_.SPLIT, gs)
    assert gs.max() < 32768 and gs.min() >= 0

    gidx_val = np.zeros((c_.NC, c_.NTOK), np.int16)
    rel_val = np.full((c_.NC, c_.NTOK), -1.0, np.float32)
    gidx_val[core, pos] = gs.astype(np.int16)
    rel_val[core, pos] = rel[order]

    # wrapped layout [16, NTOK/16], replicated to all 8 groups of 16
    # partitions (each Q7 descriptor-gen core reads its own group)
    wrap = gidx_val.reshape(c_.NC, c_.NTOK // 16, 16).transpose(0, 2, 1)
    gidx_w = np.tile(wrap, (1, 8, 1)).astype(np.int16)
    drel = rel_val.reshape(c_.NC, c_.NTILES, 128).transpose(0, 2, 1).copy()
    return gidx_w, drel


def _caps_from_edges(cfg_dims, edge_index):
    """Max per-(core, block) token counts for the A/B regions."""
    N, NC = cfg_dims["N"], cfg_dims["NC"]
    NPC = N // NC
    NBLK = (NPC + 127) // 128
    NPAD = NBLK * 128
    SPLIT = cfg_dims["SPLIT"]
    src = edge_index[0].astype(np.int64)
    dst = edge_index[1].astype(np.int64)
    owner = dst // NPC
    blk = (dst - owner * NPC) >> 7
    gsrc = (src // NPC) * NPAD + (src % NPC)
    isB = (gsrc >= SPLIT).astype(np.int64)
    key = (owner * NBLK + blk) * 2 + isB
    counts = np.bincount(key, minlength=NC * NBLK * 2).reshape(-1, 2)
    capa = _r128(max(counts[:, 0].max(), 1))
    capb = _r128(max(counts[:, 1].max(), 1))
    return capa, capb


def build_graph(cfg):
    c_ = cfg
    D, BOT, L = c_.D, c_.BOT, c_.L
    NPC, NPAD, NBLK, GN = c_.NPC, c_.NPAD, c_.NBLK, c_.GN
    TBLK = c_.TPA + c_.TPB      # tiles per block
    MSG_T = c_.MAXCB * TBLK     # tiles per msgs buffer

    nc = Bacc(target_bir_lowering=False, debug=False, num_swdge_queues=2)

    # ---------- dram parameters ----------
    x_full = nc.declare_dram_parameter("x_full", [GN, D], F32, isOutput=False)
    x_own = nc.declare_dram_parameter("x_own", [NPC, D], F32, isOutput=False)
    gidx_p = nc.declare_dram_parameter("gidx", [128, c_.NTOK // 16], I16,
                                       isOutput=False)
    drel_p = nc.declare_dram_parameter("drel", [128, c_.NTILES], F32,
                                       isOutput=False)
    iota_p = nc.declare_dram_parameter("iota", [128, 128], F32, isOutput=False)
    idn_p = nc.declare_dram_parameter("idn", [128, 128], F32, isOutput=False)
    wt_p = nc.declare_dram_parameter("wt", [D, L * D], F32, isOutput=False)
    pw1_p = nc.declare_dram_parameter("pw1t", [D, 2 * L * BOT], F32,
                                      isOutput=False)
    pw2_p = nc.declare_dram_parameter("pw2t", [16, 2 * L * D], F32,
                                      isOutput=False)
    pb1_p = nc.declare_dram_parameter("pb1t", [16, 2 * L], F32, isOutput=False)
    gb_p = nc.declare_dram_parameter("gbvec", [D, 6 * L], F32, isOutput=False)
    out_p = nc.declare_dram_parameter("out", [NPC, D], F32, isOutput=True)

    # ---------- internal dram ----------
    h_shard = nc.dram_tensor("h_shard", [NPAD, D], F32)
    h_full = nc.dram_tensor("h_full", [GN, D], F32, addr_space="Shared")
    stat_in = nc.dram_tensor("stat_in", [D, 6], F32)
    stat_out = nc.dram_tensor("stat_out", [D, 6], F32, addr_space="Shared")

    rg = [list(range(c_.NC))]

    import contextlib
    ctx = contextlib.ExitStack()

    def sb(name, shape, dt=F32):
        return ctx.enter_context(nc.sbuf_tensor(name, shape, dt))

    def ps(name, shape):
        return ctx.enter_context(nc.psum_tensor(name, shape, F32))

    def sem(name):
        return ctx.enter_context(nc.semaphore(name))

    # ---------- sbuf ----------
    h_bufs = [sb("h0", [D, NPAD]), sb("h1", [D, NPAD])]
    x_aggrT = sb("x_aggrT", [D, NPAD])          # also reused as phase-E accumulator
    h_mlpT = sb("h_mlpT", [D, NPAD])
    ad0T = sb("ad0T", [D, NPAD])
    ad1T = sb("ad1T", [D, NPAD])
    msgs = [sb("msgs0", [128, MSG_T, D]), sb("msgs1", [128, MSG_T, D])]
    TMX = max(c_.TPA, c_.TPB)
    oh_buf = sb("oh_buf", [128, 2, TMX * 128])      # per-part one-hot, 2 slots
    drel_sb = sb("drel_sb", [128, c_.NTILES])
    gidx_sb = sb("gidx_sb", [128, c_.NTOK // 16], I16)
    iota_sb = sb("iota_sb", [128, 128])
    idn_sb = sb("idn_sb", [128, 128])
    w_sb = sb("w_sb", [D, L * D])
    pw1_sb = sb("pw1_sb", [D, 2 * L * BOT])
    pw2_sb = sb("pw2_sb", [16, 2 * L * D])
    pb1_sb = sb("pb1_sb", [16, 2 * L])
    gb_sb = sb("gb_sb", [D, 6 * L])
    hid0 = sb("hid0", [16, 512])
    hid1 = sb("hid1", [16, 512])
    # squares scratch overlays oh_buf (disjoint lifetime: phase C vs phase B)
    sum_cols = [sb(f"sum_cols{i}", [D, c_.NKC]) for i in range(3)]
    sq_cols = [sb(f"sq_cols{i}", [D, c_.NKC]) for i in range(3)]
    stats_sb = sb("stats_sb", [D, 8])
    stats_g = sb("stats_g", [D, 8])
    means = sb("means", [D, 4])
    msq = sb("msq", [D, 4])
    var3 = sb("var3", [D, 4])
    sd3 = sb("sd3", [D, 4])
    rs3 = sb("rs3", [D, 4])
    svec = sb("svec", [D, 4])
    mS = sb("mS", [D, 4])
    t3 = sb("t3", [D, 4])
    tv = sb("tv", [D, 1])
    eps_sb = sb("eps_sb", [D, 1])
    stage = sb("stage", [128, 2, D])          # transpose drain, 2 slots

    # ---------- psum ----------
    p_agg2 = [ps("p_agg0", [128, 128]), ps("p_agg1", [128, 128])]
    p_c = [ps("p_c0", [128, 512]), ps("p_c1", [128, 512])]
    p_h = ps("p_h", [128, 512])
    p_a = ps("p_a", [128, 512])
    p_t2 = [ps("p_t0", [128, 128]), ps("p_t1", [128, 128])]

    # ---------- semaphores ----------
    s_g2 = [sem("gatherA"), sem("gatherB")]   # +16/call, parity by chunk
    s_oh = sem("oh")          # +1 per block (vector)
    s_peb = sem("peb")        # +1 per agg block (tensor)
    s_cp = sem("cp")          # +1 per agg copy (scalar)
    s_pe2 = sem("pe2")        # +5 per phase-C chunk (tensor)
    s_s2 = sem("s2")          # +2 per phase-C chunk (scalar relu)
    s_cp2 = sem("cp2")        # +1 per phase-C chunk copied (scalar)
    s_var = sem("var")        # +1 per layer (vector: vars ready)
    s_sqr = sem("sqr")        # +1 per layer (scalar: sqrt done)
    s_v2 = sem("v2")          # +1 per layer (vector: affines ready)
    s_acc = sem("acc")        # +1 per layer (vector: acc ready)
    s_hn = sem("hn")          # +1 per layer (h_new ready)
    s_pe3 = sem("pe3")        # +1 per transpose (tensor)
    s_s3 = sem("s3")          # +1 per stage copy (scalar)
    s_dma = sem("dma")        # +16 per sync DMA (init + stats)
    s_dt = [sem("dt0"), sem("dt1")]   # +16 per tile DMA, parity by tile
    s_cc = sem("cc")          # +1 per collective
    s_sq = sem("sq")          # +1 per layer (vector stats ready)
    s_vz = sem("vz")          # +1 init memset

    # ---------- schedule bookkeeping ----------
    # x staging layout: tile t lives in msgs[0] col t (t < M0) else
    # msgs[1] col t - M0
    NFULL = NPC // 128
    REM = NPC - NFULL * 128
    M0 = min(MSG_T, NBLK)

    def stg(t):
        return (0, t) if t < M0 else (1, t - M0)

    # sync-engine DMA milestone values (must mirror the sync stream exactly)
    N_INIT_DMA = 9 + (1 if min(NFULL, M0) > 0 else 0) \
        + (1 if NFULL > M0 else 0) + (1 if REM else 0)
    dma_init = 16 * N_INIT_DMA

    def dma_after_statin(layer):
        # s_dma counts: init DMAs + 2 stats DMAs per layer
        return 16 * (N_INIT_DMA + 2 * layer + 1)

    def dma_after_statout(layer):
        return dma_after_statin(layer) + 16

    def tile_sem(layer, t):
        # tile DMA (layer, t) increments s_dt[gt % 2]; returns (sem index,
        # cumulative count after it)
        gt = layer * NBLK + t
        return gt % 2, 16 * (gt // 2 + 1)

    def bg(layer, b):
        return layer * NBLK + b

    def cg(layer, g):
        return layer * c_.NCHL + g

    def parts_done_through_chunk(cgi):
        # s_peb counts aggregation *parts* (2 per block: A then B)
        layer, g = divmod(cgi, c_.NCHL)
        b0, nb = c_.chunks[g]
        return 2 * (layer * NBLK + b0 + nb)

    def _split1024(n):
        subs, off = [], 0
        while off < n:
            c2 = min(1024, n - off)
            subs.append((off, c2))
            off += c2
        return subs

    SUBS = [( _split1024(nb * c_.CAPA), _split1024(nb * c_.CAPB))
            for b0, nb in c_.chunks]
    SC = [len(a) + len(b) for a, b in SUBS]

    def gather_target(cgi):
        tot = 0
        for q in range(cgi % 2, cgi + 1, 2):
            tot += SC[q % c_.NCHL]
        return 16 * tot

    def cc_ar(layer):
        return 2 * layer + 1

    def cc_ag(layer):
        return 2 * layer + 2

    def pe3_val(layer, t):
        # init transposes occupy [1, NBLK]; layer l tile t -> NBLK + l*NBLK+t+1
        return NBLK + layer * NBLK + t + 1

    def pe3_init(t):
        return t + 1

    # tiles of block b: (msgs columns, drel global tile index)
    def block_tiles(g, lb):
        b0, nb = c_.chunks[g]
        b = b0 + lb
        tiles = []
        for i in range(c_.TPA):
            tiles.append((lb * c_.TPA + i, b * c_.TPA + i))
        for i in range(c_.TPB):
            tiles.append((nb * c_.TPA + lb * c_.TPB + i,
                          c_.ASZT + b * c_.TPB + i))
        return tiles

    with nc.Block() as block:

        # ================= SYNC: plain DMAs =================
        @block.sync
        def _(eng):
            dmac = [0]

            def dma(dst, src_ap):
                eng.dma_start(out=dst, in_=src_ap).then_inc(s_dma, 16)
                dmac[0] += 16

            dma(gidx_sb[:, :], gidx_p[:, :])
            dma(drel_sb[:, :], drel_p[:, :])
            dma(iota_sb[:, :], iota_p[:, :])
            dma(idn_sb[:, :], idn_p[:, :])
            dma(w_sb[:, :], wt_p[:, :])
            dma(pw1_sb[:, :], pw1_p[:, :])
            dma(pw2_sb[:, :], pw2_p[:, :])
            dma(pb1_sb[:, :], pb1_p[:, :])
            dma(gb_sb[:, :], gb_p[:, :])
            # x_own -> staging (node-major tiles, spans both msgs buffers)
            eng.wait_ge(s_vz, 1)   # staging pad rows zeroed
            n1 = min(NFULL, M0)
            if n1 > 0:
                dma(msgs[0][:, 0:n1, :],
                    x_own[0:n1 * 128, :].rearrange("(t p) d -> p t d", p=128))
            if NFULL > M0:
                dma(msgs[1][:, 0:NFULL - M0, :],
                    x_own[M0 * 128:NFULL * 128, :]
                    .rearrange("(t p) d -> p t d", p=128))
            if REM:
                bi, bc = stg(NFULL)
                dma(msgs[bi][0:REM, bc, :], x_own[NFULL * 128:NPC, :])
            assert dmac[0] == dma_init

            if TRUNC == 'B':
                eng.wait_ge(s_cp, NBLK)
                eng.dma_start(out=out_p[:, :],
                              in_=x_aggrT[:, 0:NPC]).then_inc(s_dt[0], 16)
                return
            if TRUNC == 'C':
                eng.wait_ge(s_cp2, c_.NKC)
                eng.dma_start(out=out_p[:, :],
                              in_=h_mlpT[:, 0:NPC]).then_inc(s_dt[0], 16)
                return

            for l in range(L):
                # stats out
                eng.wait_ge(s_sq, l + 1)
                dma(stat_in[:, 0:6], stats_sb[:, 0:6])
                assert dmac[0] == dma_after_statin(l)
                # stats back
                eng.wait_ge(s_cc, cc_ar(l))
                dma(stats_g[:, 0:6], stat_out[:, 0:6])
                if TRUNC == 'E':
                    eng.wait_ge(s_hn, 1)
                    eng.dma_start(out=out_p[:, :],
                                  in_=h_bufs[(l + 1) % 2][:, 0:NPC])\
                        .then_inc(s_dt[0], 16)
                    return
                # h_new tiles out
                if l > 0:
                    eng.wait_ge(s_cc, cc_ag(l - 1))  # h_shard free
                for t in range(NBLK):
                    eng.wait_ge(s_s3, pe3_val(l, t))
                    slot = stage[:, t % 2, :]
                    sidx, _ = tile_sem(l, t)
                    if l < L - 1:
                        tgt, src_ap = h_shard[t * 128:(t + 1) * 128, :], slot
                    elif t < NPC // 128:
                        tgt, src_ap = out_p[t * 128:(t + 1) * 128, :], slot
                    else:
                        rem = NPC - (NPC // 128) * 128
                        tgt = out_p[t * 128:t * 128 + rem, :]
                        src_ap = stage[0:rem, t % 2, :]
                    eng.dma_start(out=tgt, in_=src_ap).then_inc(s_dt[sidx], 16)
                if l == L - 1:
                    # flush: nothing further
                    pass

        # ================= GPSIMD: gathers + collectives =================
        @block.gpsimd
        def _(eng):
            for l in range(L):
                hsrc = x_full if l == 0 else h_full
                if l == 0:
                    eng.wait_ge(s_dma, dma_init)
                else:
                    eng.wait_ge(s_cc, cc_ag(l - 1))
                for g, (b0, nb) in enumerate(c_.chunks):
                    cgi = cg(l, g)
                    if cgi <= 1:
                        # msgs buffers double as the x_own staging buffer
                        eng.wait_ge(s_pe3, NBLK)
                    if cgi >= 2:
                        eng.wait_ge(s_peb, parts_done_through_chunk(cgi - 2))
                    buf = msgs[cgi % 2]
                    subsA, subsB = SUBS[g]
                    a0 = b0 * c_.CAPA
                    for off, cnt in subsA:
                        t0 = a0 + off
                        eng.dma_gather(
                            buf[:, off // 128:(off + cnt) // 128, :],
                            hsrc[0:c_.SPLIT, :],
                            gidx_sb[:, t0 // 16:(t0 + cnt) // 16],
                            cnt, cnt, D, queue_num=cgi % 2,
                        ).then_inc(s_g2[cgi % 2], 16)
                    b0tok = c_.ASZ + b0 * c_.CAPB
                    for off, cnt in subsB:
                        t0 = b0tok + off
                        bt = nb * c_.TPA + off // 128
                        eng.dma_gather(
                            buf[:, bt:bt + cnt // 128, :],
                            hsrc[c_.SPLIT:GN, :],
                            gidx_sb[:, t0 // 16:(t0 + cnt) // 16],
                            cnt, cnt, D, queue_num=cgi % 2,
                        ).then_inc(s_g2[cgi % 2], 16)
                if TRUNC in ('B', 'C'):
                    return
                # stats AllReduce
                eng.wait_ge(s_dma, dma_after_statin(l))
                eng.collective_compute(
                    "AllReduce", ALU.add, replica_groups=rg,
                    ins=[stat_in[:, 0:6].opt()], outs=[stat_out[:, 0:6].opt()],
                ).then_inc(s_cc, 1)
                if TRUNC == 'E':
                    return
                # h AllGather
                if l < L - 1:
                    for tq in (NBLK - 1, NBLK - 2):
                        if tq >= 0:
                            si, cnt = tile_sem(l, tq)
                            eng.wait_ge(s_dt[si], cnt)
                    eng.collective_compute(
                        "AllGather", ALU.bypass, replica_groups=rg,
                        ins=[h_shard[:, :].opt()], outs=[h_full[:, :].opt()],
                    ).then_inc(s_cc, 1)

        # ================= VECTOR =================
        @block.vector
        def _(eng):
            # init: zero staging pad region for partial x tile
            eng.memset(eps_sb[:, :], EPS)
            if REM:
                bi, bc = stg(NFULL)
                eng.memset(msgs[bi][:, bc, :], 0.0)
            eng.drain().then_inc(s_vz, 1)
            eng.wait_ge(s_dma, dma_init)

            for l in range(L):
                # --- phase B: one-hot generation per block ---
                for g, (b0, nb) in enumerate(c_.chunks):
                    for lb in range(nb):
                        b = b0 + lb
                        bgi = bg(l, b)
                        # parts: (A tiles, drel base) then (B tiles, base)
                        parts = [(c_.TPA, b * c_.TPA),
                                 (c_.TPB, c_.ASZT + b * c_.TPB)]
                        for pi, (tcnt, d0) in enumerate(parts):
                            pgi = 2 * bgi + pi
                            if pgi >= 2:
                                eng.wait_ge(s_peb, pgi - 1)
                            o = oh_buf[:, pgi % 2, 0:tcnt * 128]
                            o = o.rearrange("p (t j) -> p t j", j=128)
                            d_in = drel_sb[:, d0:d0 + tcnt].unsqueeze(-1)\
                                .broadcast_to([128, tcnt, 128])
                            i_in = iota_sb[:, :].unsqueeze(1)\
                                .broadcast_to([128, tcnt, 128])
                            eng.tensor_tensor(
                                out=o, in0=d_in, in1=i_in,
                                op=ALU.is_equal).then_inc(s_oh, 1)

                if TRUNC in ('B', 'C'):
                    return
                # --- phase D (squares are computed by the scalar engine) ---
                base2 = l * c_.NKC
                eng.wait_ge(s_cp2, base2 + c_.NKC)
                for j in range(3):
                    eng.reduce_sum(out=stats_sb[:, j:j + 1],
                                   in_=sum_cols[j][:, :], axis=AX)
                    eng.reduce_sum(out=stats_sb[:, 3 + j:4 + j],
                                   in_=sq_cols[j][:, :], axis=AX)
                eng.drain().then_inc(s_sq, 1)
                # affine math
                eng.wait_ge(s_dma, dma_after_statout(l))
                invn = 1.0 / c_.N
                eng.tensor_scalar_mul(means[:, 0:3], stats_g[:, 0:3], invn)
                eng.tensor_scalar_mul(msq[:, 0:3], stats_g[:, 3:6], invn)
                eng.drain()
                eng.tensor_tensor(out=var3[:, 0:3], in0=means[:, 0:3],
                                  in1=means[:, 0:3], op=ALU.mult)
                eng.drain()
                eng.tensor_sub(var3[:, 0:3], msq[:, 0:3], var3[:, 0:3])
                eng.drain().then_inc(s_var, 1)
                eng.wait_ge(s_sqr, l + 1)
                eng.reciprocal(rs3[:, 0:3], sd3[:, 0:3])
                eng.drain()
                eng.tensor_tensor(out=svec[:, 0:3], in0=rs3[:, 0:3],
                                  in1=gb_sb[:, 6 * l:6 * l + 3], op=ALU.mult)
                eng.drain()
                eng.tensor_tensor(out=mS[:, 0:3], in0=means[:, 0:3],
                                  in1=svec[:, 0:3], op=ALU.mult)
                eng.drain()
                eng.tensor_sub(t3[:, 0:3], gb_sb[:, 6 * l + 3:6 * l + 6],
                               mS[:, 0:3])
                eng.drain()
                eng.reduce_sum(out=tv[:, :], in_=t3[:, 0:3], axis=AX)
                eng.drain().then_inc(s_v2, 1)
                # --- phase E ---
                eng.tensor_scalar_mul(x_aggrT[:, 0:NPC], h_mlpT[:, 0:NPC],
                                      svec[:, 0:1])
                eng.drain()
                eng.scalar_tensor_tensor(
                    out=x_aggrT[:, 0:NPC], in0=ad0T[:, 0:NPC],
                    scalar=svec[:, 1:2], in1=x_aggrT[:, 0:NPC],
                    op0=ALU.mult, op1=ALU.add)
                eng.drain()
                eng.scalar_tensor_tensor(
                    out=x_aggrT[:, 0:NPC], in0=ad1T[:, 0:NPC],
                    scalar=svec[:, 2:3], in1=x_aggrT[:, 0:NPC],
                    op0=ALU.mult, op1=ALU.add)
                eng.drain().then_inc(s_acc, 1)
                if l == L - 1:
                    h_new = h_bufs[(l + 1) % 2]
                    inst = eng.tensor_scalar_add(h_new[:, 0:NPAD],
                                                 x_aggrT[:, 0:NPAD],
                                                 tv[:, 0:1])
                    inst.then_inc(s_hn, 1)

        # ================= TENSOR =================
        @block.tensor
        def _(eng):
            # init: build h_ownT from x staging
            eng.wait_ge(s_dma, dma_init)
            for t in range(NBLK):
                if t >= 2:
                    eng.wait_ge(s_s3, pe3_init(t) - 2)
                bi, bc = stg(t)
                inst = eng.transpose(
                    p_t2[t % 2][0:D, 0:128],
                    msgs[bi][:, bc, :], idn_sb[0:128, 0:128])
                inst.then_inc(s_pe3, 1)

            for l in range(L):
                h_own = h_bufs[l % 2]
                # --- phase B: aggregation matmuls ---
                for g, (b0, nb) in enumerate(c_.chunks):
                    cgi = cg(l, g)
                    eng.wait_ge(s_g2[cgi % 2], gather_target(cgi))
                    buf = msgs[cgi % 2]
                    for lb in range(nb):
                        b = b0 + lb
                        bgi = bg(l, b)
                        if bgi >= 2:
                            eng.wait_ge(s_cp, bgi - 1)
                        pslot = p_agg2[bgi % 2][0:D, 0:128]
                        # msgs columns: A tiles, then B tiles (per chunk)
                        parts = [
                            (c_.TPA, lambda i, lb=lb: lb * c_.TPA + i),
                            (c_.TPB, lambda i, lb=lb, nb=nb:
                                nb * c_.TPA + lb * c_.TPB + i),
                        ]
                        for pi, (tcnt, mcol_of) in enumerate(parts):
                            pgi = 2 * bgi + pi
                            eng.wait_ge(s_oh, pgi + 1)
                            slot = oh_buf[:, pgi % 2, :]
                            for i in range(tcnt):
                                inst = eng.matmul(
                                    pslot, buf[:, mcol_of(i), :],
                                    slot[:, i * 128:(i + 1) * 128],
                                    start=(pi == 0 and i == 0),
                                    stop=(pi == 1 and i == tcnt - 1))
                            inst.then_inc(s_peb, 1)
                if TRUNC == 'B':
                    return
                # --- phase C ---
                eng.wait_ge(s_cp, (l + 1) * NBLK)
                if l == 0:
                    eng.wait_ge(s_s3, NBLK)        # init copies done
                else:
                    eng.wait_ge(s_hn, l)           # h_own ready
                base2, base_s2 = l * c_.NKC * 5, l * c_.NKC * 2
                wl = w_sb[:, l * D:(l + 1) * D]
                for k, (o0, w) in enumerate(c_.kchunks):
                    if k >= 2:
                        eng.wait_ge(s_cp2, base2 // 5 + k - 1)
                    pc = p_c[k % 2][0:D, 0:w]
                    eng.matmul(pc, wl, x_aggrT[:, o0:o0 + w],
                               start=True, stop=True).then_inc(s_pe2, 1)
                    # adapter 0 hidden (input h_own)
                    ph = p_h[0:BOT, 0:w]
                    pw1_0 = pw1_sb[:, l * BOT:(l + 1) * BOT]
                    eng.matmul(ph, pw1_0, h_own[:, o0:o0 + w],
                               start=True, stop=True).then_inc(s_pe2, 1)
                    eng.wait_ge(s_s2, base_s2 + 2 * k + 1)
                    pa = p_a[0:D, 0:w]
                    pw2_0 = pw2_sb[0:BOT, l * D:(l + 1) * D]
                    eng.matmul(pa, pw2_0, hid0[0:BOT, 0:w],
                               start=True, stop=True).then_inc(s_pe2, 1)
                    # adapter 1 hidden (input x_aggr)
                    pw1_1 = pw1_sb[:, (L + l) * BOT:(L + l + 1) * BOT]
                    eng.matmul(ph, pw1_1, x_aggrT[:, o0:o0 + w],
                               start=True, stop=True).then_inc(s_pe2, 1)
                    eng.wait_ge(s_s2, base_s2 + 2 * k + 2)
                    pw2_1 = pw2_sb[0:BOT, (L + l) * D:(L + l + 1) * D]
                    eng.matmul(pa, pw2_1, hid1[0:BOT, 0:w],
                               start=True, stop=True).then_inc(s_pe2, 1)
                if TRUNC in ('C', 'E'):
                    return
                # --- phase F: transposes ---
                h_new = h_bufs[(l + 1) % 2]
                eng.wait_ge(s_hn, l + 1)
                for t in range(NBLK):
                    eng.wait_ge(s_s3, pe3_val(l, t) - 2)
                    inst = eng.transpose(
                        p_t2[t % 2][0:128, 0:64],
                        h_new[:, t * 128:(t + 1) * 128],
                        idn_sb[0:64, 0:64])
                    inst.then_inc(s_pe3, 1)

        # ================= SCALAR =================
        @block.scalar
        def _(eng):
            # init: drain h_ownT transposes ([64 feat, 128 nodes] psum slots)
            for t in range(NBLK):
                eng.wait_ge(s_pe3, pe3_init(t))
                inst = eng.activation(
                    h_bufs[0][:, t * 128:(t + 1) * 128],
                    p_t2[t % 2][0:D, 0:128],
                    ACTF.Copy)
                inst.then_inc(s_s3, 1)

            for l in range(L):
                # --- phase B: psum -> x_aggrT ---
                for b in range(NBLK):
                    bgi = bg(l, b)
                    eng.wait_ge(s_peb, 2 * bgi + 2)
                    pslot = p_agg2[bgi % 2][0:D, 0:128]
                    inst = eng.activation(
                        x_aggrT[:, b * 128:(b + 1) * 128], pslot, ACTF.Copy)
                    inst.then_inc(s_cp, 1)
                if TRUNC == 'B':
                    return
                # --- phase C ---
                base2, base_s2 = l * c_.NKC * 5, l * c_.NKC * 2
                oh_flat = oh_buf[:, :, :].rearrange("p q f -> p (q f)")
                for k, (o0, w) in enumerate(c_.kchunks):
                    we = w if o0 + w <= NPC else max(NPC - o0, 0)
                    assert 3 * we <= 2 * TMX * 128
                    scr = [oh_flat[0:D, j * we:(j + 1) * we] for j in range(3)]
                    eng.wait_ge(s_pe2, base2 + 5 * k + 1)
                    eng.activation(h_mlpT[:, o0:o0 + we],
                                   p_c[k % 2][0:D, 0:we], ACTF.Copy,
                                   accum_out=sum_cols[0][:, k:k + 1])
                    eng.activation(scr[0], p_c[k % 2][0:D, 0:we], ACTF.Square,
                                   accum_out=sq_cols[0][:, k:k + 1])
                    eng.wait_ge(s_pe2, base2 + 5 * k + 2)
                    pb1_0 = pb1_sb[0:BOT, l:l + 1]
                    inst = eng.activation(hid0[0:BOT, 0:w], p_h[0:BOT, 0:w],
                                          ACTF.Relu, bias=pb1_0)
                    inst.then_inc(s_s2, 1)
                    eng.wait_ge(s_pe2, base2 + 5 * k + 3)
                    eng.activation(ad0T[:, o0:o0 + we], p_a[0:D, 0:we],
                                   ACTF.Copy,
                                   accum_out=sum_cols[1][:, k:k + 1])
                    eng.activation(scr[1], p_a[0:D, 0:we], ACTF.Square,
                                   accum_out=sq_cols[1][:, k:k + 1])
                    eng.drain()   # a0 square must finish before PE reuses p_a
                    eng.wait_ge(s_pe2, base2 + 5 * k + 4)
                    pb1_1 = pb1_sb[0:BOT, L + l:L + l + 1]
                    inst = eng.activation(hid1[0:BOT, 0:w], p_h[0:BOT, 0:w],
                                          ACTF.Relu, bias=pb1_1)
                    inst.then_inc(s_s2, 1)
                    eng.wait_ge(s_pe2, base2 + 5 * k + 5)
                    eng.activation(ad1T[:, o0:o0 + we], p_a[0:D, 0:we],
                                   ACTF.Copy,
                                   accum_out=sum_cols[2][:, k:k + 1])
                    eng.activation(scr[2], p_a[0:D, 0:we], ACTF.Square,
                                   accum_out=sq_cols[2][:, k:k + 1])
                    eng.drain().then_inc(s_cp2, 1)
                if TRUNC == 'C':
                    return
                # --- phase D: sqrt(var + eps) ---
                eng.wait_ge(s_var, l + 1)
                eng.activation(sd3[:, 0:3], var3[:, 0:3], ACTF.Sqrt,
                               bias=eps_sb[:, 0:1]).then_inc(s_sqr, 1)
                # --- phase E: relu ---
                if l < L - 1:
                    h_new = h_bufs[(l + 1) % 2]
                    eng.wait_ge(s_acc, l + 1)
                    eng.activation(h_new[:, 0:NPC], x_aggrT[:, 0:NPC],
                                   ACTF.Relu, bias=tv[:, 0:1])
                    if NPAD > NPC:
                        eng.activation(h_new[:, NPC:NPAD],
                                       x_aggrT[:, NPC:NPAD],
                                       ACTF.Copy, scale=0.0)
                    eng.drain().then_inc(s_hn, 1)
                if TRUNC == 'E':
                    return
                # --- phase F: psum -> stage ---
                for t in range(NBLK):
                    gt = l * NBLK + t
                    if gt >= 2:
                        lp, tp = divmod(gt - 2, NBLK)
                        si, cnt = tile_sem(lp, tp)
                        eng.wait_ge(s_dt[si], cnt)
                    eng.wait_ge(s_pe3, pe3_val(l, t))
                    inst = eng.activation(
                        stage[:, t % 2, :],
                        p_t2[t % 2][0:128, 0:64],
                        ACTF.Copy)
                    inst.then_inc(s_s3, 1)

    ctx.close()
    nc.finalize()
    return nc


def _host_pack(cfg, W, pw1, pw2, pb1, bn_g, bn_b, pbn_g, pbn_b, gating):
    L, D, BOT = cfg.L, cfg.D, cfg.BOT
    wt = np.ascontiguousarray(W.transpose(1, 0, 2).reshape(D, L * D))
    pw1t = np.ascontiguousarray(
        pw1.transpose(2, 0, 1, 3).reshape(D, 2 * L * BOT))
    pw2t = np.zeros((16, 2 * L * D), np.float32)
    pw2t[0:BOT] = pw2.transpose(2, 0, 1, 3).reshape(BOT, 2 * L * D)
    pb1t = np.zeros((16, 2 * L), np.float32)
    pb1t[0:BOT] = pb1.transpose(2, 0, 1).reshape(BOT, 2 * L)
    gb = np.zeros((D, 6 * L), np.float32)
    for l in range(L):
        g0 = gating[0, l, 0]
        g1 = gating[1, l, 0]
        gb[:, 6 * l + 0] = bn_g[l]
        gb[:, 6 * l + 1] = pbn_g[0, l] * g0
        gb[:, 6 * l + 2] = pbn_g[1, l] * g1
        gb[:, 6 * l + 3] = bn_b[l]
        gb[:, 6 * l + 4] = pbn_b[0, l] * g0
        gb[:, 6 * l + 5] = pbn_b[1, l] * g1
    iota = np.tile(np.arange(128, dtype=np.float32), (128, 1))
    idn = np.eye(128, dtype=np.float32)
    return dict(wt=wt, pw1t=pw1t, pw2t=pw2t, pb1t=pb1t, gbvec=gb,
                iota=np.ascontiguousarray(iota), idn=idn)


def make_in_maps(cfg, inputs):
    c_ = cfg
    x = np.asarray(inputs["x"], np.float32)
    edge_index = np.asarray(inputs["edge_index"])
    gidx_w, drel = _prep_tokens(c_, edge_index)
    packs = _host_pack(c_, *[np.asarray(inputs[k], np.float32) for k in
                             ("W", "pw1", "pw2", "pb1", "bn_g", "bn_b",
                              "pbn_g", "pbn_b", "gating")])
    xpad = np.zeros((c_.GN, c_.D), np.float32)
    xpad.reshape(c_.NC, c_.NPAD, c_.D)[:, 0:c_.NPC] = \
        x.reshape(c_.NC, c_.NPC, c_.D)
    in_maps = []
    for i in range(c_.NC):
        m = dict(packs)
        m["x_full"] = xpad
        m["x_own"] = np.ascontiguousarray(
            x[i * c_.NPC:(i + 1) * c_.NPC])
        m["gidx"] = np.ascontiguousarray(gidx_w[i])
        m["drel"] = np.ascontiguousarray(drel[i])
        in_maps.append(m)
    return in_maps


def _make_cfg(inputs, N=50000, E=800000, D=64, L=5, BOT=15, NC=8,
              SPLIT=32768, CH_BLOCKS=2):
    edge_index = np.asarray(inputs["edge_index"])
    capa, capb = _caps_from_edges(dict(N=N, NC=NC, SPLIT=SPLIT), edge_index)
    return Cfg(N, E, D, L, BOT, NC, SPLIT, CH_BLOCKS, capa, capb)


_GRAPH_CACHE = {}


def kernel(**inputs) -> np.ndarray:
    cfg = _make_cfg(inputs)
    key = (cfg.CAPA, cfg.CAPB)
    if key not in _GRAPH_CACHE:
        _GRAPH_CACHE[key] = build_graph(cfg)
    nc = _GRAPH_CACHE[key]
    in_maps = make_in_maps(cfg, inputs)
    res = run_bass_kernel_spmd(nc, in_maps, core_ids=list(range(cfg.NC)))
    outs = [res.results[i]["out"] for i in range(cfg.NC)]
    return np.concatenate(outs, axis=0)



# revision 11
# speedup vs baseline: 1.2435x; 1.2435x over previous
"""AdapterGNN on 8 TRN2 NeuronCores.

Strategy (dst-node sharding):
  - Nodes sharded: core c owns nodes [c*6250, (c+1)*6250). All edges whose dst
    belongs to core c are processed by core c (~100k edges/core).
  - Per layer: AllGather replicates h (node-major, padded to 6272 rows/core) to
    every core; each core dma_gather's its edges' source rows (token stream,
    grouped by 128-node dst block), then reduces tokens -> nodes with a
    one-hot matmul on TensorE (segment-sum, f32 PSUM accumulation, race-free).
  - All per-node compute (conv matmul, adapters, BN apply) runs feature-major
    ([64, nodes] in SBUF).  BatchNorm statistics are computed as per-core
    partial sums + a tiny AllReduce ([64,6] per layer).
  - Linear biases feeding straight into BatchNorm (b, pb2) cancel exactly and
    are skipped; gating is folded into the adapter BN affine on the host.
"""

import math
import os
import sys

import numpy as np

sys.path.insert(0, "/opt/trn_rl_repo")

from concourse import bass, mybir  # noqa: E402
from concourse.bacc import Bacc  # noqa: E402
from concourse.bass_utils import run_bass_kernel_spmd  # noqa: E402

F32 = mybir.dt.float32
I16 = mybir.dt.int16
AX = mybir.AxisListType.X
ALU = mybir.AluOpType
ACTF = mybir.ActivationFunctionType

EPS = 1e-5
TRUNC = None  # debug: 'B' | 'C' | 'E' | None


def _r128(v):
    return ((int(v) + 127) // 128) * 128


class Cfg:
    def __init__(self, N, E, D, L, BOT, NC, SPLIT, CH_BLOCKS, CAPA, CAPB,
                 NQ=4):
        self.N, self.E, self.D, self.L, self.BOT = N, E, D, L, BOT
        self.NC, self.SPLIT = NC, SPLIT
        self.NQ = NQ
        self.NPC = N // NC                      # real nodes per core
        self.NBLK = (self.NPC + 127) // 128     # dst blocks per core
        self.NPAD = self.NBLK * 128             # padded nodes per core
        self.GN = NC * self.NPAD                # padded global rows
        self.CAPA, self.CAPB = CAPA, CAPB       # tokens per block (A/B region)
        self.TPA, self.TPB = CAPA // 128, CAPB // 128
        self.ASZ = self.NBLK * CAPA             # B region token offset
        self.ASZT = self.NBLK * self.TPA        # B region tile offset
        self.NTOK = self.NBLK * (CAPA + CAPB)
        self.NTILES = self.NTOK // 128
        # gather chunks: groups of dst blocks
        self.chunks = []
        b = 0
        while b < self.NBLK:
            nb = min(CH_BLOCKS, self.NBLK - b)
            self.chunks.append((b, nb))
            b += nb
        self.NCHL = len(self.chunks)
        self.MAXCB = max(nb for _, nb in self.chunks)
        # phase-C column chunks over [0, NPAD)
        self.kchunks = []
        off = 0
        while off < self.NPAD:
            w = min(512, self.NPAD - off)
            self.kchunks.append((off, w))
            off += w
        self.NKC = len(self.kchunks)


def _prep_tokens(cfg, edge_index):
    """Per-core token streams: gather idx (wrapped int16) + dst_rel (f32)."""
    c_ = cfg
    src = edge_index[0].astype(np.int64)
    dst = edge_index[1].astype(np.int64)
    owner = dst // c_.NPC
    dloc = dst - owner * c_.NPC
    blk = dloc >> 7
    rel = (dloc & 127).astype(np.float32)
    gsrc = (src // c_.NPC) * c_.NPAD + (src % c_.NPC)
    isB = (gsrc >= c_.SPLIT).astype(np.int64)

    key = (owner * c_.NBLK + blk) * 2 + isB
    order = np.argsort(key, kind="stable")
    skey = key[order]
    counts = np.bincount(key, minlength=c_.NC * c_.NBLK * 2)
    starts = np.concatenate([[0], np.cumsum(counts)[:-1]])
    rank = np.arange(c_.E) - starts[skey]

    core = skey // (2 * c_.NBLK)
    remk = skey % (2 * c_.NBLK)
    b2 = remk // 2
    piece = remk % 2
    pos = np.where(piece == 0, b2 * c_.CAPA + rank,
                   c_.ASZ + b2 * c_.CAPB + rank)

    gs = gsrc[order]
    gs = np.where(piece == 1, gs - c_.SPLIT, gs)
    assert gs.max() < 32768 and gs.min() >= 0

    gidx_val = np.zeros((c_.NC, c_.NTOK), np.int16)
    rel_val = np.full((c_.NC, c_.NTOK), -1.0, np.float32)
    gidx_val[core, pos] = gs.astype(np.int16)
    rel_val[core, pos] = rel[order]

    # wrapped layout [16, NTOK/16], replicated to all 8 groups of 16
    # partitions (each Q7 descriptor-gen core reads its own group)
    wrap = gidx_val.reshape(c_.NC, c_.NTOK // 16, 16).transpose(0, 2, 1)
    gidx_w = np.tile(wrap, (1, 8, 1)).astype(np.int16)
    drel = rel_val.reshape(c_.NC, c_.NTILES, 128).transpose(0, 2, 1).copy()
    return gidx_w, drel


def _caps_from_edges(cfg_dims, edge_index):
    """Max per-(core, block) token counts for the A/B regions."""
    N, NC = cfg_dims["N"], cfg_dims["NC"]
    NPC = N // NC
    NBLK = (NPC + 127) // 128
    NPAD = NBLK * 128
    SPLIT = cfg_dims["SPLIT"]
    src = edge_index[0].astype(np.int64)
    dst = edge_index[1].astype(np.int64)
    owner = dst // NPC
    blk = (dst - owner * NPC) >> 7
    gsrc = (src // NPC) * NPAD + (src % NPC)
    isB = (gsrc >= SPLIT).astype(np.int64)
    key = (owner * NBLK + blk) * 2 + isB
    counts = np.bincount(key, minlength=NC * NBLK * 2).reshape(-1, 2)
    capa = _r128(max(counts[:, 0].max(), 1))
    capb = _r128(max(counts[:, 1].max(), 1))
    return capa, capb


def build_graph(cfg):
    c_ = cfg
    D, BOT, L = c_.D, c_.BOT, c_.L
    NPC, NPAD, NBLK, GN = c_.NPC, c_.NPAD, c_.NBLK, c_.GN
    TBLK = c_.TPA + c_.TPB      # tiles per block
    MSG_T = c_.MAXCB * TBLK     # tiles per msgs buffer

    nc = Bacc(target_bir_lowering=False, debug=False,
              num_swdge_queues=cfg.NQ)

    # ---------- dram parameters ----------
    x_full = nc.declare_dram_parameter("x_full", [GN, D], F32, isOutput=False)
    x_own = nc.declare_dram_parameter("x_own", [NPC, D], F32, isOutput=False)
    gidx_p = nc.declare_dram_parameter("gidx", [128, c_.NTOK // 16], I16,
                                       isOutput=False)
    drel_p = nc.declare_dram_parameter("drel", [128, c_.NTILES], F32,
                                       isOutput=False)
    iota_p = nc.declare_dram_parameter("iota", [128, 128], F32, isOutput=False)
    idn_p = nc.declare_dram_parameter("idn", [128, 128], F32, isOutput=False)
    wt_p = nc.declare_dram_parameter("wt", [D, L * D], F32, isOutput=False)
    pw1_p = nc.declare_dram_parameter("pw1t", [D, 2 * L * BOT], F32,
                                      isOutput=False)
    pw2_p = nc.declare_dram_parameter("pw2t", [16, 2 * L * D], F32,
                                      isOutput=False)
    pb1_p = nc.declare_dram_parameter("pb1t", [16, 2 * L], F32, isOutput=False)
    gb_p = nc.declare_dram_parameter("gbvec", [D, 6 * L], F32, isOutput=False)
    out_p = nc.declare_dram_parameter("out", [NPC, D], F32, isOutput=True)

    # ---------- internal dram ----------
    h_shard = nc.dram_tensor("h_shard", [NPAD, D], F32)
    h_full = nc.dram_tensor("h_full", [GN, D], F32, addr_space="Shared")
    stat_in = nc.dram_tensor("stat_in", [D, 6], F32)
    stat_out = nc.dram_tensor("stat_out", [D, 6], F32, addr_space="Shared")

    rg = [list(range(c_.NC))]

    import contextlib
    ctx = contextlib.ExitStack()

    def sb(name, shape, dt=F32):
        return ctx.enter_context(nc.sbuf_tensor(name, shape, dt))

    def ps(name, shape):
        return ctx.enter_context(nc.psum_tensor(name, shape, F32))

    def sem(name):
        return ctx.enter_context(nc.semaphore(name))

    # ---------- sbuf ----------
    h_bufs = [sb("h0", [D, NPAD]), sb("h1", [D, NPAD])]
    x_aggrT = sb("x_aggrT", [D, NPAD])          # also reused as phase-E accumulator
    h_mlpT = sb("h_mlpT", [D, NPAD])
    ad0T = sb("ad0T", [D, NPAD])
    ad1T = sb("ad1T", [D, NPAD])
    msgs = [sb(f"msgs{i}", [128, MSG_T, D]) for i in range(c_.NQ)]
    TMX = max(c_.TPA, c_.TPB)
    oh_buf = sb("oh_buf", [128, 2, TMX * 128])      # per-part one-hot, 2 slots
    drel_sb = sb("drel_sb", [128, c_.NTILES])
    gidx_sb = sb("gidx_sb", [128, c_.NTOK // 16], I16)
    iota_sb = sb("iota_sb", [128, 128])
    idn_sb = sb("idn_sb", [128, 128])
    w_sb = sb("w_sb", [D, L * D])
    pw1_sb = sb("pw1_sb", [D, 2 * L * BOT])
    pw2_sb = sb("pw2_sb", [16, 2 * L * D])
    pb1_sb = sb("pb1_sb", [16, 2 * L])
    gb_sb = sb("gb_sb", [D, 6 * L])
    hid0 = sb("hid0", [16, 512])
    hid1 = sb("hid1", [16, 512])
    # squares scratch overlays oh_buf (disjoint lifetime: phase C vs phase B)
    sum_cols = [sb(f"sum_cols{i}", [D, c_.NKC]) for i in range(3)]
    sq_cols = [sb(f"sq_cols{i}", [D, c_.NKC]) for i in range(3)]
    stats_sb = sb("stats_sb", [D, 8])
    stats_g = sb("stats_g", [D, 8])
    means = sb("means", [D, 4])
    msq = sb("msq", [D, 4])
    var3 = sb("var3", [D, 4])
    sd3 = sb("sd3", [D, 4])
    rs3 = sb("rs3", [D, 4])
    svec = sb("svec", [D, 4])
    mS = sb("mS", [D, 4])
    t3 = sb("t3", [D, 4])
    tv = sb("tv", [D, 1])
    eps_sb = sb("eps_sb", [D, 1])
    stage = sb("stage", [128, 2, D])          # transpose drain, 2 slots

    # ---------- psum ----------
    p_agg2 = [ps("p_agg0", [128, 128]), ps("p_agg1", [128, 128])]
    p_c = [ps("p_c0", [128, 512]), ps("p_c1", [128, 512])]
    p_h = ps("p_h", [128, 512])
    p_a = ps("p_a", [128, 512])
    p_t2 = [ps("p_t0", [128, 128]), ps("p_t1", [128, 128])]

    # ---------- semaphores ----------
    s_g2 = [sem(f"gather{i}") for i in range(c_.NQ)]  # +16/call, by queue
    s_oh = sem("oh")          # +1 per block (vector)
    s_peb = sem("peb")        # +1 per agg block (tensor)
    s_cp = sem("cp")          # +1 per agg copy (scalar)
    s_pe2 = sem("pe2")        # +5 per phase-C chunk (tensor)
    s_s2 = sem("s2")          # +2 per phase-C chunk (scalar relu)
    s_cp2 = sem("cp2")        # +1 per phase-C chunk copied (scalar)
    s_var = sem("var")        # +1 per layer (vector: vars ready)
    s_sqr = sem("sqr")        # +1 per layer (scalar: sqrt done)
    s_v2 = sem("v2")          # +1 per layer (vector: affines ready)
    s_acc = sem("acc")        # +1 per layer (vector: acc ready)
    s_hn = sem("hn")          # +1 per layer (h_new ready)
    s_pe3 = sem("pe3")        # +1 per transpose (tensor)
    s_s3 = sem("s3")          # +1 per stage copy (scalar)
    s_dma = sem("dma")        # +16 per sync DMA (init + stats)
    s_dt = [sem("dt0"), sem("dt1")]   # +16 per tile DMA, parity by tile
    s_cc = sem("cc")          # +1 per collective
    s_sq = sem("sq")          # +1 per layer (vector stats ready)
    s_vz = sem("vz")          # +1 init memset

    # ---------- schedule bookkeeping ----------
    # x staging layout: tile t lives in msgs[0] col t (t < M0) else
    # msgs[1] col t - M0
    NFULL = NPC // 128
    REM = NPC - NFULL * 128
    assert c_.NQ * MSG_T >= NBLK, "staging must fit in msgs buffers"

    def stg(t):
        return (t // MSG_T, t % MSG_T)

    # full-tile staging DMA ranges: one DMA per touched buffer
    STG_RANGES = []
    t0 = 0
    while t0 < NFULL:
        n = min(MSG_T - (t0 % MSG_T), NFULL - t0)
        STG_RANGES.append((t0, n))
        t0 += n

    # sync-engine DMA milestone values (must mirror the sync stream exactly)
    N_INIT_DMA = 9 + len(STG_RANGES) + (1 if REM else 0)
    dma_init = 16 * N_INIT_DMA

    def dma_after_statin(layer):
        # s_dma counts: init DMAs + 2 stats DMAs per layer
        return 16 * (N_INIT_DMA + 2 * layer + 1)

    def dma_after_statout(layer):
        return dma_after_statin(layer) + 16

    def tile_sem(layer, t):
        # tile DMA (layer, t) increments s_dt[gt % 2]; returns (sem index,
        # cumulative count after it)
        gt = layer * NBLK + t
        return gt % 2, 16 * (gt // 2 + 1)

    def bg(layer, b):
        return layer * NBLK + b

    def cg(layer, g):
        return layer * c_.NCHL + g

    def parts_done_through_chunk(cgi):
        # s_peb counts aggregation *parts* (2 per block: A then B)
        layer, g = divmod(cgi, c_.NCHL)
        b0, nb = c_.chunks[g]
        return 2 * (layer * NBLK + b0 + nb)

    def _split1024(n):
        subs, off = [], 0
        while off < n:
            c2 = min(1024, n - off)
            subs.append((off, c2))
            off += c2
        return subs

    SUBS = [( _split1024(nb * c_.CAPA), _split1024(nb * c_.CAPB))
            for b0, nb in c_.chunks]
    SC = [len(a) + len(b) for a, b in SUBS]

    def gather_target(cgi):
        tot = 0
        for q in range(cgi % c_.NQ, cgi + 1, c_.NQ):
            tot += SC[q % c_.NCHL]
        return 16 * tot

    def cc_ar(layer):
        return 2 * layer + 1

    def cc_ag(layer):
        return 2 * layer + 2

    def pe3_val(layer, t):
        # init transposes occupy [1, NBLK]; layer l tile t -> NBLK + l*NBLK+t+1
        return NBLK + layer * NBLK + t + 1

    def pe3_init(t):
        return t + 1

    # tiles of block b: (msgs columns, drel global tile index)
    def block_tiles(g, lb):
        b0, nb = c_.chunks[g]
        b = b0 + lb
        tiles = []
        for i in range(c_.TPA):
            tiles.append((lb * c_.TPA + i, b * c_.TPA + i))
        for i in range(c_.TPB):
            tiles.append((nb * c_.TPA + lb * c_.TPB + i,
                          c_.ASZT + b * c_.TPB + i))
        return tiles

    with nc.Block() as block:

        # ================= SYNC: plain DMAs =================
        @block.sync
        def _(eng):
            dmac = [0]

            def dma(dst, src_ap):
                eng.dma_start(out=dst, in_=src_ap).then_inc(s_dma, 16)
                dmac[0] += 16

            dma(gidx_sb[:, :], gidx_p[:, :])
            dma(drel_sb[:, :], drel_p[:, :])
            dma(iota_sb[:, :], iota_p[:, :])
            dma(idn_sb[:, :], idn_p[:, :])
            dma(w_sb[:, :], wt_p[:, :])
            dma(pw1_sb[:, :], pw1_p[:, :])
            dma(pw2_sb[:, :], pw2_p[:, :])
            dma(pb1_sb[:, :], pb1_p[:, :])
            dma(gb_sb[:, :], gb_p[:, :])
            # x_own -> staging (node-major tiles, spans the msgs buffers)
            eng.wait_ge(s_vz, 1)   # staging pad rows zeroed
            for t0, n in STG_RANGES:
                bi, bc = stg(t0)
                dma(msgs[bi][:, bc:bc + n, :],
                    x_own[t0 * 128:(t0 + n) * 128, :]
                    .rearrange("(t p) d -> p t d", p=128))
            if REM:
                bi, bc = stg(NFULL)
                dma(msgs[bi][0:REM, bc, :], x_own[NFULL * 128:NPC, :])
            assert dmac[0] == dma_init

            if TRUNC == 'B':
                eng.wait_ge(s_cp, NBLK)
                eng.dma_start(out=out_p[:, :],
                              in_=x_aggrT[:, 0:NPC]).then_inc(s_dt[0], 16)
                return
            if TRUNC == 'C':
                eng.wait_ge(s_cp2, c_.NKC)
                eng.dma_start(out=out_p[:, :],
                              in_=h_mlpT[:, 0:NPC]).then_inc(s_dt[0], 16)
                return

            for l in range(L):
                # stats out
                eng.wait_ge(s_sq, l + 1)
                dma(stat_in[:, 0:6], stats_sb[:, 0:6])
                assert dmac[0] == dma_after_statin(l)
                # stats back
                eng.wait_ge(s_cc, cc_ar(l))
                dma(stats_g[:, 0:6], stat_out[:, 0:6])
                if TRUNC == 'E':
                    eng.wait_ge(s_hn, 1)
                    eng.dma_start(out=out_p[:, :],
                                  in_=h_bufs[(l + 1) % 2][:, 0:NPC])\
                        .then_inc(s_dt[0], 16)
                    return
                # h_new tiles out
                if l > 0:
                    eng.wait_ge(s_cc, cc_ag(l - 1))  # h_shard free
                for t in range(NBLK):
                    eng.wait_ge(s_s3, pe3_val(l, t))
                    slot = stage[:, t % 2, :]
                    sidx, _ = tile_sem(l, t)
                    if l < L - 1:
                        tgt, src_ap = h_shard[t * 128:(t + 1) * 128, :], slot
                    elif t < NPC // 128:
                        tgt, src_ap = out_p[t * 128:(t + 1) * 128, :], slot
                    else:
                        rem = NPC - (NPC // 128) * 128
                        tgt = out_p[t * 128:t * 128 + rem, :]
                        src_ap = stage[0:rem, t % 2, :]
                    eng.dma_start(out=tgt, in_=src_ap).then_inc(s_dt[sidx], 16)
                if l == L - 1:
                    # flush: nothing further
                    pass

        # ================= GPSIMD: gathers + collectives =================
        @block.gpsimd
        def _(eng):
            for l in range(L):
                hsrc = x_full if l == 0 else h_full
                if l == 0:
                    eng.wait_ge(s_dma, dma_init)
                else:
                    eng.wait_ge(s_cc, cc_ag(l - 1))
                for g, (b0, nb) in enumerate(c_.chunks):
                    cgi = cg(l, g)
                    if cgi <= c_.NQ - 1:
                        # msgs buffers double as the x_own staging buffer
                        eng.wait_ge(s_pe3, NBLK)
                    if cgi >= c_.NQ:
                        eng.wait_ge(s_peb,
                                    parts_done_through_chunk(cgi - c_.NQ))
                    buf = msgs[cgi % c_.NQ]
                    subsA, subsB = SUBS[g]
                    a0 = b0 * c_.CAPA
                    for off, cnt in subsA:
                        t0 = a0 + off
                        eng.dma_gather(
                            buf[:, off // 128:(off + cnt) // 128, :],
                            hsrc[0:c_.SPLIT, :],
                            gidx_sb[:, t0 // 16:(t0 + cnt) // 16],
                            cnt, cnt, D, queue_num=cgi % c_.NQ,
                        ).then_inc(s_g2[cgi % c_.NQ], 16)
                    b0tok = c_.ASZ + b0 * c_.CAPB
                    for off, cnt in subsB:
                        t0 = b0tok + off
                        bt = nb * c_.TPA + off // 128
                        eng.dma_gather(
                            buf[:, bt:bt + cnt // 128, :],
                            hsrc[c_.SPLIT:GN, :],
                            gidx_sb[:, t0 // 16:(t0 + cnt) // 16],
                            cnt, cnt, D, queue_num=cgi % c_.NQ,
                        ).then_inc(s_g2[cgi % c_.NQ], 16)
                if TRUNC in ('B', 'C'):
                    return
                # stats AllReduce
                eng.wait_ge(s_dma, dma_after_statin(l))
                eng.collective_compute(
                    "AllReduce", ALU.add, replica_groups=rg,
                    ins=[stat_in[:, 0:6].opt()], outs=[stat_out[:, 0:6].opt()],
                ).then_inc(s_cc, 1)
                if TRUNC == 'E':
                    return
                # h AllGather
                if l < L - 1:
                    for tq in (NBLK - 1, NBLK - 2):
                        if tq >= 0:
                            si, cnt = tile_sem(l, tq)
                            eng.wait_ge(s_dt[si], cnt)
                    eng.collective_compute(
                        "AllGather", ALU.bypass, replica_groups=rg,
                        ins=[h_shard[:, :].opt()], outs=[h_full[:, :].opt()],
                    ).then_inc(s_cc, 1)

        # ================= VECTOR =================
        @block.vector
        def _(eng):
            # init: zero staging pad region for partial x tile
            eng.memset(eps_sb[:, :], EPS)
            if REM:
                bi, bc = stg(NFULL)
                eng.memset(msgs[bi][:, bc, :], 0.0)
            eng.drain().then_inc(s_vz, 1)
            eng.wait_ge(s_dma, dma_init)

            for l in range(L):
                # --- phase B: one-hot generation per block ---
                for g, (b0, nb) in enumerate(c_.chunks):
                    for lb in range(nb):
                        b = b0 + lb
                        bgi = bg(l, b)
                        # parts: (A tiles, drel base) then (B tiles, base)
                        parts = [(c_.TPA, b * c_.TPA),
                                 (c_.TPB, c_.ASZT + b * c_.TPB)]
                        for pi, (tcnt, d0) in enumerate(parts):
                            pgi = 2 * bgi + pi
                            if pgi >= 2:
                                eng.wait_ge(s_peb, pgi - 1)
                            o = oh_buf[:, pgi % 2, 0:tcnt * 128]
                            o = o.rearrange("p (t j) -> p t j", j=128)
                            d_in = drel_sb[:, d0:d0 + tcnt].unsqueeze(-1)\
                                .broadcast_to([128, tcnt, 128])
                            i_in = iota_sb[:, :].unsqueeze(1)\
                                .broadcast_to([128, tcnt, 128])
                            eng.tensor_tensor(
                                out=o, in0=d_in, in1=i_in,
                                op=ALU.is_equal).then_inc(s_oh, 1)

                if TRUNC in ('B', 'C'):
                    return
                # --- phase D (squares are computed by the scalar engine) ---
                base2 = l * c_.NKC
                eng.wait_ge(s_cp2, base2 + c_.NKC)
                for j in range(3):
                    eng.reduce_sum(out=stats_sb[:, j:j + 1],
                                   in_=sum_cols[j][:, :], axis=AX)
                    eng.reduce_sum(out=stats_sb[:, 3 + j:4 + j],
                                   in_=sq_cols[j][:, :], axis=AX)
                eng.drain().then_inc(s_sq, 1)
                # affine math
                eng.wait_ge(s_dma, dma_after_statout(l))
                invn = 1.0 / c_.N
                eng.tensor_scalar_mul(means[:, 0:3], stats_g[:, 0:3], invn)
                eng.tensor_scalar_mul(msq[:, 0:3], stats_g[:, 3:6], invn)
                eng.drain()
                eng.tensor_tensor(out=var3[:, 0:3], in0=means[:, 0:3],
                                  in1=means[:, 0:3], op=ALU.mult)
                eng.drain()
                eng.tensor_sub(var3[:, 0:3], msq[:, 0:3], var3[:, 0:3])
                eng.drain().then_inc(s_var, 1)
                eng.wait_ge(s_sqr, l + 1)
                eng.reciprocal(rs3[:, 0:3], sd3[:, 0:3])
                eng.drain()
                eng.tensor_tensor(out=svec[:, 0:3], in0=rs3[:, 0:3],
                                  in1=gb_sb[:, 6 * l:6 * l + 3], op=ALU.mult)
                eng.drain()
                eng.tensor_tensor(out=mS[:, 0:3], in0=means[:, 0:3],
                                  in1=svec[:, 0:3], op=ALU.mult)
                eng.drain()
                eng.tensor_sub(t3[:, 0:3], gb_sb[:, 6 * l + 3:6 * l + 6],
                               mS[:, 0:3])
                eng.drain()
                eng.reduce_sum(out=tv[:, :], in_=t3[:, 0:3], axis=AX)
                eng.drain().then_inc(s_v2, 1)
                # --- phase E ---
                eng.tensor_scalar_mul(x_aggrT[:, 0:NPC], h_mlpT[:, 0:NPC],
                                      svec[:, 0:1])
                eng.drain()
                eng.scalar_tensor_tensor(
                    out=x_aggrT[:, 0:NPC], in0=ad0T[:, 0:NPC],
                    scalar=svec[:, 1:2], in1=x_aggrT[:, 0:NPC],
                    op0=ALU.mult, op1=ALU.add)
                eng.drain()
                eng.scalar_tensor_tensor(
                    out=x_aggrT[:, 0:NPC], in0=ad1T[:, 0:NPC],
                    scalar=svec[:, 2:3], in1=x_aggrT[:, 0:NPC],
                    op0=ALU.mult, op1=ALU.add)
                eng.drain().then_inc(s_acc, 1)
                if l == L - 1:
                    h_new = h_bufs[(l + 1) % 2]
                    inst = eng.tensor_scalar_add(h_new[:, 0:NPAD],
                                                 x_aggrT[:, 0:NPAD],
                                                 tv[:, 0:1])
                    inst.then_inc(s_hn, 1)

        # ================= TENSOR =================
        @block.tensor
        def _(eng):
            # init: build h_ownT from x staging
            eng.wait_ge(s_dma, dma_init)
            for t in range(NBLK):
                if t >= 2:
                    eng.wait_ge(s_s3, pe3_init(t) - 2)
                bi, bc = stg(t)
                inst = eng.transpose(
                    p_t2[t % 2][0:D, 0:128],
                    msgs[bi][:, bc, :], idn_sb[0:128, 0:128])
                inst.then_inc(s_pe3, 1)

            for l in range(L):
                h_own = h_bufs[l % 2]
                # --- phase B: aggregation matmuls ---
                for g, (b0, nb) in enumerate(c_.chunks):
                    cgi = cg(l, g)
                    eng.wait_ge(s_g2[cgi % c_.NQ], gather_target(cgi))
                    buf = msgs[cgi % c_.NQ]
                    for lb in range(nb):
                        b = b0 + lb
                        bgi = bg(l, b)
                        if bgi >= 2:
                            eng.wait_ge(s_cp, bgi - 1)
                        pslot = p_agg2[bgi % 2][0:D, 0:128]
                        # msgs columns: A tiles, then B tiles (per chunk)
                        parts = [
                            (c_.TPA, lambda i, lb=lb: lb * c_.TPA + i),
                            (c_.TPB, lambda i, lb=lb, nb=nb:
                                nb * c_.TPA + lb * c_.TPB + i),
                        ]
                        for pi, (tcnt, mcol_of) in enumerate(parts):
                            pgi = 2 * bgi + pi
                            eng.wait_ge(s_oh, pgi + 1)
                            slot = oh_buf[:, pgi % 2, :]
                            for i in range(tcnt):
                                inst = eng.matmul(
                                    pslot, buf[:, mcol_of(i), :],
                                    slot[:, i * 128:(i + 1) * 128],
                                    start=(pi == 0 and i == 0),
                                    stop=(pi == 1 and i == tcnt - 1))
                            inst.then_inc(s_peb, 1)
                if TRUNC == 'B':
                    return
                # --- phase C ---
                eng.wait_ge(s_cp, (l + 1) * NBLK)
                if l == 0:
                    eng.wait_ge(s_s3, NBLK)        # init copies done
                else:
                    eng.wait_ge(s_hn, l)           # h_own ready
                base2, base_s2 = l * c_.NKC * 5, l * c_.NKC * 2
                wl = w_sb[:, l * D:(l + 1) * D]
                for k, (o0, w) in enumerate(c_.kchunks):
                    if k >= 2:
                        eng.wait_ge(s_cp2, base2 // 5 + k - 1)
                    pc = p_c[k % 2][0:D, 0:w]
                    eng.matmul(pc, wl, x_aggrT[:, o0:o0 + w],
                               start=True, stop=True).then_inc(s_pe2, 1)
                    # adapter 0 hidden (input h_own)
                    ph = p_h[0:BOT, 0:w]
                    pw1_0 = pw1_sb[:, l * BOT:(l + 1) * BOT]
                    eng.matmul(ph, pw1_0, h_own[:, o0:o0 + w],
                               start=True, stop=True).then_inc(s_pe2, 1)
                    eng.wait_ge(s_s2, base_s2 + 2 * k + 1)
                    pa = p_a[0:D, 0:w]
                    pw2_0 = pw2_sb[0:BOT, l * D:(l + 1) * D]
                    eng.matmul(pa, pw2_0, hid0[0:BOT, 0:w],
                               start=True, stop=True).then_inc(s_pe2, 1)
                    # adapter 1 hidden (input x_aggr)
                    pw1_1 = pw1_sb[:, (L + l) * BOT:(L + l + 1) * BOT]
                    eng.matmul(ph, pw1_1, x_aggrT[:, o0:o0 + w],
                               start=True, stop=True).then_inc(s_pe2, 1)
                    eng.wait_ge(s_s2, base_s2 + 2 * k + 2)
                    pw2_1 = pw2_sb[0:BOT, (L + l) * D:(L + l + 1) * D]
                    eng.matmul(pa, pw2_1, hid1[0:BOT, 0:w],
                               start=True, stop=True).then_inc(s_pe2, 1)
                if TRUNC in ('C', 'E'):
                    return
                # --- phase F: transposes ---
                h_new = h_bufs[(l + 1) % 2]
                eng.wait_ge(s_hn, l + 1)
                for t in range(NBLK):
                    eng.wait_ge(s_s3, pe3_val(l, t) - 2)
                    inst = eng.transpose(
                        p_t2[t % 2][0:128, 0:64],
                        h_new[:, t * 128:(t + 1) * 128],
                        idn_sb[0:64, 0:64])
                    inst.then_inc(s_pe3, 1)

        # ================= SCALAR =================
        @block.scalar
        def _(eng):
            # init: drain h_ownT transposes ([64 feat, 128 nodes] psum slots)
            for t in range(NBLK):
                eng.wait_ge(s_pe3, pe3_init(t))
                inst = eng.activation(
                    h_bufs[0][:, t * 128:(t + 1) * 128],
                    p_t2[t % 2][0:D, 0:128],
                    ACTF.Copy)
                inst.then_inc(s_s3, 1)

            for l in range(L):
                # --- phase B: psum -> x_aggrT ---
                for b in range(NBLK):
                    bgi = bg(l, b)
                    eng.wait_ge(s_peb, 2 * bgi + 2)
                    pslot = p_agg2[bgi % 2][0:D, 0:128]
                    inst = eng.activation(
                        x_aggrT[:, b * 128:(b + 1) * 128], pslot, ACTF.Copy)
                    inst.then_inc(s_cp, 1)
                if TRUNC == 'B':
                    return
                # --- phase C ---
                base2, base_s2 = l * c_.NKC * 5, l * c_.NKC * 2
                oh_flat = oh_buf[:, :, :].rearrange("p q f -> p (q f)")
                for k, (o0, w) in enumerate(c_.kchunks):
                    we = w if o0 + w <= NPC else max(NPC - o0, 0)
                    assert 3 * we <= 2 * TMX * 128
                    scr = [oh_flat[0:D, j * we:(j + 1) * we] for j in range(3)]
                    eng.wait_ge(s_pe2, base2 + 5 * k + 1)
                    eng.activation(h_mlpT[:, o0:o0 + we],
                                   p_c[k % 2][0:D, 0:we], ACTF.Copy,
                                   accum_out=sum_cols[0][:, k:k + 1])
                    eng.activation(scr[0], p_c[k % 2][0:D, 0:we], ACTF.Square,
                                   accum_out=sq_cols[0][:, k:k + 1])
                    eng.wait_ge(s_pe2, base2 + 5 * k + 2)
                    pb1_0 = pb1_sb[0:BOT, l:l + 1]
                    inst = eng.activation(hid0[0:BOT, 0:w], p_h[0:BOT, 0:w],
                                          ACTF.Relu, bias=pb1_0)
                    inst.then_inc(s_s2, 1)
                    eng.wait_ge(s_pe2, base2 + 5 * k + 3)
                    eng.activation(ad0T[:, o0:o0 + we], p_a[0:D, 0:we],
                                   ACTF.Copy,
                                   accum_out=sum_cols[1][:, k:k + 1])
                    eng.activation(scr[1], p_a[0:D, 0:we], ACTF.Square,
                                   accum_out=sq_cols[1][:, k:k + 1])
                    eng.drain()   # a0 square must finish before PE reuses p_a
                    eng.wait_ge(s_pe2, base2 + 5 * k + 4)
                    pb1_1 = pb1_sb[0:BOT, L + l:L + l + 1]
                    inst = eng.activation(hid1[0:BOT, 0:w], p_h[0:BOT, 0:w],
                                          ACTF.Relu, bias=pb1_1)
                    inst.then_inc(s_s2, 1)
                    eng.wait_ge(s_pe2, base2 + 5 * k + 5)
                    eng.activation(ad1T[:, o0:o0 + we], p_a[0:D, 0:we],
                                   ACTF.Copy,
                                   accum_out=sum_cols[2][:, k:k + 1])
                    eng.activation(scr[2], p_a[0:D, 0:we], ACTF.Square,
                                   accum_out=sq_cols[2][:, k:k + 1])
                    eng.drain().then_inc(s_cp2, 1)
                if TRUNC == 'C':
                    return
                # --- phase D: sqrt(var + eps) ---
                eng.wait_ge(s_var, l + 1)
                eng.activation(sd3[:, 0:3], var3[:, 0:3], ACTF.Sqrt,
                               bias=eps_sb[:, 0:1]).then_inc(s_sqr, 1)
                # --- phase E: relu ---
                if l < L - 1:
                    h_new = h_bufs[(l + 1) % 2]
                    eng.wait_ge(s_acc, l + 1)
                    eng.activation(h_new[:, 0:NPC], x_aggrT[:, 0:NPC],
                                   ACTF.Relu, bias=tv[:, 0:1])
                    if NPAD > NPC:
                        eng.activation(h_new[:, NPC:NPAD],
                                       x_aggrT[:, NPC:NPAD],
                                       ACTF.Copy, scale=0.0)
                    eng.drain().then_inc(s_hn, 1)
                if TRUNC == 'E':
                    return
                # --- phase F: psum -> stage ---
                for t in range(NBLK):
                    gt = l * NBLK + t
                    if gt >= 2:
                        lp, tp = divmod(gt - 2, NBLK)
                        si, cnt = tile_sem(lp, tp)
                        eng.wait_ge(s_dt[si], cnt)
                    eng.wait_ge(s_pe3, pe3_val(l, t))
                    inst = eng.activation(
                        stage[:, t % 2, :],
                        p_t2[t % 2][0:128, 0:64],
                        ACTF.Copy)
                    inst.then_inc(s_s3, 1)

    ctx.close()
    nc.finalize()
    return nc


def _host_pack(cfg, W, pw1, pw2, pb1, bn_g, bn_b, pbn_g, pbn_b, gating):
    L, D, BOT = cfg.L, cfg.D, cfg.BOT
    wt = np.ascontiguousarray(W.transpose(1, 0, 2).reshape(D, L * D))
    pw1t = np.ascontiguousarray(
        pw1.transpose(2, 0, 1, 3).reshape(D, 2 * L * BOT))
    pw2t = np.zeros((16, 2 * L * D), np.float32)
    pw2t[0:BOT] = pw2.transpose(2, 0, 1, 3).reshape(BOT, 2 * L * D)
    pb1t = np.zeros((16, 2 * L), np.float32)
    pb1t[0:BOT] = pb1.transpose(2, 0, 1).reshape(BOT, 2 * L)
    gb = np.zeros((D, 6 * L), np.float32)
    for l in range(L):
        g0 = gating[0, l, 0]
        g1 = gating[1, l, 0]
        gb[:, 6 * l + 0] = bn_g[l]
        gb[:, 6 * l + 1] = pbn_g[0, l] * g0
        gb[:, 6 * l + 2] = pbn_g[1, l] * g1
        gb[:, 6 * l + 3] = bn_b[l]
        gb[:, 6 * l + 4] = pbn_b[0, l] * g0
        gb[:, 6 * l + 5] = pbn_b[1, l] * g1
    iota = np.tile(np.arange(128, dtype=np.float32), (128, 1))
    idn = np.eye(128, dtype=np.float32)
    return dict(wt=wt, pw1t=pw1t, pw2t=pw2t, pb1t=pb1t, gbvec=gb,
                iota=np.ascontiguousarray(iota), idn=idn)


def make_in_maps(cfg, inputs):
    c_ = cfg
    x = np.asarray(inputs["x"], np.float32)
    edge_index = np.asarray(inputs["edge_index"])
    gidx_w, drel = _prep_tokens(c_, edge_index)
    packs = _host_pack(c_, *[np.asarray(inputs[k], np.float32) for k in
                             ("W", "pw1", "pw2", "pb1", "bn_g", "bn_b",
                              "pbn_g", "pbn_b", "gating")])
    xpad = np.zeros((c_.GN, c_.D), np.float32)
    xpad.reshape(c_.NC, c_.NPAD, c_.D)[:, 0:c_.NPC] = \
        x.reshape(c_.NC, c_.NPC, c_.D)
    in_maps = []
    for i in range(c_.NC):
        m = dict(packs)
        m["x_full"] = xpad
        m["x_own"] = np.ascontiguousarray(
            x[i * c_.NPC:(i + 1) * c_.NPC])
        m["gidx"] = np.ascontiguousarray(gidx_w[i])
        m["drel"] = np.ascontiguousarray(drel[i])
        in_maps.append(m)
    return in_maps


def _make_cfg(inputs, N=50000, E=800000, D=64, L=5, BOT=15, NC=8,
              SPLIT=32768, CH_BLOCKS=1, NQ=4):
    edge_index = np.asarray(inputs["edge_index"])
    capa, capb = _caps_from_edges(dict(N=N, NC=NC, SPLIT=SPLIT), edge_index)
    return Cfg(N, E, D, L, BOT, NC, SPLIT, CH_BLOCKS, capa, capb, NQ=NQ)


_GRAPH_CACHE = {}


def kernel(**inputs) -> np.ndarray:
    cfg = _make_cfg(inputs)
    key = (cfg.CAPA, cfg.CAPB)
    if key not in _GRAPH_CACHE:
        _GRAPH_CACHE[key] = build_graph(cfg)
    nc = _GRAPH_CACHE[key]
    in_maps = make_in_maps(cfg, inputs)
    res = run_bass_kernel_spmd(nc, in_maps, core_ids=list(range(cfg.NC)))
    outs = [res.results[i]["out"] for i in range(cfg.NC)]
    return np.concatenate(outs, axis=0)



# revision 27
# speedup vs baseline: 1.2820x; 1.0310x over previous
"""AdapterGNN on 8 TRN2 NeuronCores.

Strategy (dst-node sharding):
  - Nodes sharded: core c owns nodes [c*6250, (c+1)*6250). All edges whose dst
    belongs to core c are processed by core c (~100k edges/core).
  - Per layer: AllGather replicates h (node-major, padded to 6272 rows/core) to
    every core; each core dma_gather's its edges' source rows (token stream,
    grouped by 128-node dst block), then reduces tokens -> nodes with a
    one-hot matmul on TensorE (segment-sum, f32 PSUM accumulation, race-free).
  - All per-node compute (conv matmul, adapters, BN apply) runs feature-major
    ([64, nodes] in SBUF).  BatchNorm statistics are computed as per-core
    partial sums + a tiny AllReduce ([64,6] per layer).
  - Linear biases feeding straight into BatchNorm (b, pb2) cancel exactly and
    are skipped; gating is folded into the adapter BN affine on the host.
"""

import math
import os
import sys

import numpy as np

sys.path.insert(0, "/opt/trn_rl_repo")

from concourse import bass, mybir  # noqa: E402
from concourse.bacc import Bacc  # noqa: E402
from concourse.bass_utils import run_bass_kernel_spmd  # noqa: E402

F32 = mybir.dt.float32
I16 = mybir.dt.int16
AX = mybir.AxisListType.X
ALU = mybir.AluOpType
ACTF = mybir.ActivationFunctionType

EPS = 1e-5
TRUNC = None  # debug: 'B' | 'C' | 'E' | None


def _r128(v):
    return ((int(v) + 127) // 128) * 128


class Cfg:
    def __init__(self, N, E, D, L, BOT, NC, SPLIT, CH_BLOCKS, CAPA, CAPB,
                 NQ=4):
        self.N, self.E, self.D, self.L, self.BOT = N, E, D, L, BOT
        self.NC, self.SPLIT = NC, SPLIT
        self.NQ = NQ
        self.NPC = N // NC                      # real nodes per core
        self.NBLK = (self.NPC + 127) // 128     # dst blocks per core
        self.NPAD = self.NBLK * 128             # padded nodes per core
        self.GN = NC * self.NPAD                # padded global rows
        self.CAPA, self.CAPB = CAPA, CAPB       # tokens per block (A/B region)
        self.TPA, self.TPB = CAPA // 128, CAPB // 128
        self.ASZ = self.NBLK * CAPA             # B region token offset
        self.ASZT = self.NBLK * self.TPA        # B region tile offset
        self.NTOK = self.NBLK * (CAPA + CAPB)
        self.NTILES = self.NTOK // 128
        # gather chunks: groups of dst blocks
        self.chunks = []
        b = 0
        while b < self.NBLK:
            nb = min(CH_BLOCKS, self.NBLK - b)
            self.chunks.append((b, nb))
            b += nb
        self.NCHL = len(self.chunks)
        self.MAXCB = max(nb for _, nb in self.chunks)
        # phase-C column chunks over [0, NPAD)
        self.kchunks = []
        off = 0
        while off < self.NPAD:
            w = min(512, self.NPAD - off)
            self.kchunks.append((off, w))
            off += w
        self.NKC = len(self.kchunks)


def _prep_tokens(cfg, edge_index):
    """Per-core token streams: gather idx (wrapped int16) + dst_rel (f32)."""
    c_ = cfg
    src = edge_index[0].astype(np.int64)
    dst = edge_index[1].astype(np.int64)
    owner = dst // c_.NPC
    dloc = dst - owner * c_.NPC
    blk = dloc >> 7
    rel = (dloc & 127).astype(np.float32)
    gsrc = (src // c_.NPC) * c_.NPAD + (src % c_.NPC)
    isB = (gsrc >= c_.SPLIT).astype(np.int64)

    key = (owner * c_.NBLK + blk) * 2 + isB
    order = np.argsort(key, kind="stable")
    skey = key[order]
    counts = np.bincount(key, minlength=c_.NC * c_.NBLK * 2)
    starts = np.concatenate([[0], np.cumsum(counts)[:-1]])
    rank = np.arange(c_.E) - starts[skey]

    core = skey // (2 * c_.NBLK)
    remk = skey % (2 * c_.NBLK)
    b2 = remk // 2
    piece = remk % 2
    pos = np.where(piece == 0, b2 * c_.CAPA + rank,
                   c_.ASZ + b2 * c_.CAPB + rank)

    gs = gsrc[order]
    gs = np.where(piece == 1, gs - c_.SPLIT, gs)
    assert gs.max() < 32768 and gs.min() >= 0

    gidx_val = np.zeros((c_.NC, c_.NTOK), np.int16)
    rel_val = np.full((c_.NC, c_.NTOK), -1.0, np.float32)
    gidx_val[core, pos] = gs.astype(np.int16)
    rel_val[core, pos] = rel[order]

    # wrapped layout [16, NTOK/16], replicated to all 8 groups of 16
    # partitions (each Q7 descriptor-gen core reads its own group)
    wrap = gidx_val.reshape(c_.NC, c_.NTOK // 16, 16).transpose(0, 2, 1)
    gidx_w = np.tile(wrap, (1, 8, 1)).astype(np.int16)
    drel = rel_val.reshape(c_.NC, c_.NTILES, 128).transpose(0, 2, 1).copy()
    return gidx_w, drel


def _caps_from_edges(cfg_dims, edge_index):
    """Max per-(core, block) token counts for the A/B regions."""
    N, NC = cfg_dims["N"], cfg_dims["NC"]
    NPC = N // NC
    NBLK = (NPC + 127) // 128
    NPAD = NBLK * 128
    SPLIT = cfg_dims["SPLIT"]
    src = edge_index[0].astype(np.int64)
    dst = edge_index[1].astype(np.int64)
    owner = dst // NPC
    blk = (dst - owner * NPC) >> 7
    gsrc = (src // NPC) * NPAD + (src % NPC)
    isB = (gsrc >= SPLIT).astype(np.int64)
    key = (owner * NBLK + blk) * 2 + isB
    counts = np.bincount(key, minlength=NC * NBLK * 2).reshape(-1, 2)
    capa = _r128(max(counts[:, 0].max(), 1))
    capb = _r128(max(counts[:, 1].max(), 1))
    return capa, capb


def build_graph(cfg):
    c_ = cfg
    D, BOT, L = c_.D, c_.BOT, c_.L
    NPC, NPAD, NBLK, GN = c_.NPC, c_.NPAD, c_.NBLK, c_.GN
    TBLK = c_.TPA + c_.TPB      # tiles per block
    MSG_T = c_.MAXCB * TBLK     # tiles per msgs buffer

    nc = Bacc(target_bir_lowering=False, debug=False,
              num_swdge_queues=cfg.NQ)

    # ---------- dram parameters ----------
    x_full = nc.declare_dram_parameter("x_full", [GN, D], F32, isOutput=False)
    x_own = nc.declare_dram_parameter("x_own", [NPC, D], F32, isOutput=False)
    gidx_p = nc.declare_dram_parameter("gidx", [128, c_.NTOK // 16], I16,
                                       isOutput=False)
    drel_p = nc.declare_dram_parameter("drel", [128, c_.NTILES], F32,
                                       isOutput=False)
    iota_p = nc.declare_dram_parameter("iota", [128, 128], F32, isOutput=False)
    idn_p = nc.declare_dram_parameter("idn", [128, 128], F32, isOutput=False)
    wt_p = nc.declare_dram_parameter("wt", [D, L * D], F32, isOutput=False)
    pw1_p = nc.declare_dram_parameter("pw1t", [D, 2 * L * BOT], F32,
                                      isOutput=False)
    pw2_p = nc.declare_dram_parameter("pw2t", [16, 2 * L * D], F32,
                                      isOutput=False)
    pb1_p = nc.declare_dram_parameter("pb1t", [16, 2 * L], F32, isOutput=False)
    gb_p = nc.declare_dram_parameter("gbvec", [D, 6 * L], F32, isOutput=False)
    out_p = nc.declare_dram_parameter("out", [NPC, D], F32, isOutput=True)

    # ---------- internal dram ----------
    h_shard = nc.dram_tensor("h_shard", [NPAD, D], F32)
    h_full = nc.dram_tensor("h_full", [GN, D], F32, addr_space="Shared")
    stat_in = nc.dram_tensor("stat_in", [D, 6], F32)
    stat_out = nc.dram_tensor("stat_out", [D, 6], F32, addr_space="Shared")

    rg = [list(range(c_.NC))]

    import contextlib
    ctx = contextlib.ExitStack()

    def sb(name, shape, dt=F32):
        return ctx.enter_context(nc.sbuf_tensor(name, shape, dt))

    def ps(name, shape):
        return ctx.enter_context(nc.psum_tensor(name, shape, F32))

    def sem(name):
        return ctx.enter_context(nc.semaphore(name))

    # ---------- sbuf ----------
    h_bufs = [sb("h0", [D, NPAD]), sb("h1", [D, NPAD])]
    x_aggrT = sb("x_aggrT", [D, NPAD])          # also reused as phase-E accumulator
    h_mlpT = sb("h_mlpT", [D, NPAD])
    ad0T = sb("ad0T", [D, NPAD])
    ad1T = sb("ad1T", [D, NPAD])
    msgs = [sb(f"msgs{i}", [128, MSG_T, D]) for i in range(c_.NQ)]
    TMX = max(c_.TPA, c_.TPB)
    oh_buf = sb("oh_buf", [128, 2, TMX * 128])      # per-part one-hot, 2 slots
    drel_sb = sb("drel_sb", [128, c_.NTILES])
    gidx_sb = sb("gidx_sb", [128, c_.NTOK // 16], I16)
    iota_sb = sb("iota_sb", [128, 128])
    idn_sb = sb("idn_sb", [128, 128])
    w_sb = sb("w_sb", [D, L * D])
    pw1_sb = sb("pw1_sb", [D, 2 * L * BOT])
    pw2_sb = sb("pw2_sb", [16, 2 * L * D])
    pb1_sb = sb("pb1_sb", [16, 2 * L])
    gb_sb = sb("gb_sb", [D, 6 * L])
    hid0 = sb("hid0", [16, 512])
    hid1 = sb("hid1", [16, 512])
    # squares scratch overlays oh_buf (disjoint lifetime: phase C vs phase B)
    sum_cols = [sb(f"sum_cols{i}", [D, c_.NKC]) for i in range(3)]
    sq_cols = [sb(f"sq_cols{i}", [D, c_.NKC]) for i in range(3)]
    stats_sb = sb("stats_sb", [D, 8])
    stats_g = sb("stats_g", [D, 8])
    means = sb("means", [D, 4])
    msq = sb("msq", [D, 4])
    var3 = sb("var3", [D, 4])
    sd3 = sb("sd3", [D, 4])
    rs3 = sb("rs3", [D, 4])
    svec = sb("svec", [D, 4])
    mS = sb("mS", [D, 4])
    t3 = sb("t3", [D, 4])
    tv = sb("tv", [D, 1])
    eps_sb = sb("eps_sb", [D, 1])
    stage = sb("stage", [128, 2, D])          # transpose drain, 2 slots

    # ---------- psum ----------
    p_agg2 = [ps("p_agg0", [128, 128]), ps("p_agg1", [128, 128])]
    p_c = [ps("p_c0", [128, 512]), ps("p_c1", [128, 512])]
    p_h = ps("p_h", [128, 512])
    p_a = ps("p_a", [128, 512])
    p_t2 = [ps("p_t0", [128, 128]), ps("p_t1", [128, 128])]

    # ---------- semaphores ----------
    s_g2 = [sem(f"gather{i}") for i in range(c_.NQ)]  # +16/call, by queue
    s_oh = sem("oh")          # +1 per block (vector)
    s_peb = sem("peb")        # +1 per agg block (tensor)
    s_cp = sem("cp")          # +1 per agg copy (scalar)
    s_pe2 = sem("pe2")        # +5 per phase-C chunk (tensor)
    s_s2 = sem("s2")          # +2 per phase-C chunk (scalar relu)
    s_cp2 = sem("cp2")        # +1 per phase-C chunk copied (scalar)
    s_var = sem("var")        # +1 per layer (vector: vars ready)
    s_sqr = sem("sqr")        # +1 per layer (scalar: sqrt done)
    s_v2 = sem("v2")          # +1 per layer (vector: affines ready)
    s_acc = sem("acc")        # +1 per layer (vector: acc ready)
    s_hn = sem("hn")          # +1 per layer (h_new ready)
    s_pe3 = sem("pe3")        # +1 per transpose (tensor)
    s_s3 = sem("s3")          # +1 per stage copy (scalar)
    s_dma = sem("dma")        # +16 per sync DMA (init + stats)
    s_dt = [sem("dt0"), sem("dt1")]   # +16 per tile DMA, parity by tile
    s_cc = sem("cc")          # +1 per collective
    s_sq = sem("sq")          # +1 per layer (vector stats ready)
    s_vz = sem("vz")          # +1 init memset

    # ---------- schedule bookkeeping ----------
    # x staging layout: tile t lives in msgs[0] col t (t < M0) else
    # msgs[1] col t - M0
    NFULL = NPC // 128
    REM = NPC - NFULL * 128
    assert c_.NQ * MSG_T >= NBLK, "staging must fit in msgs buffers"

    def stg(t):
        return (t // MSG_T, t % MSG_T)

    # full-tile staging DMA ranges: one DMA per touched buffer
    STG_RANGES = []
    t0 = 0
    while t0 < NFULL:
        n = min(MSG_T - (t0 % MSG_T), NFULL - t0)
        STG_RANGES.append((t0, n))
        t0 += n

    # sync-engine DMA milestone values (must mirror the sync stream exactly)
    N_INIT_DMA = 9 + len(STG_RANGES) + (1 if REM else 0)
    dma_init = 16 * N_INIT_DMA

    def dma_after_statin(layer):
        # s_dma counts: init DMAs + 2 stats DMAs per layer
        return 16 * (N_INIT_DMA + 2 * layer + 1)

    def dma_after_statout(layer):
        return dma_after_statin(layer) + 16

    def tile_sem(layer, t):
        # tile DMA (layer, t) increments s_dt[gt % 2]; returns (sem index,
        # cumulative count after it)
        gt = layer * NBLK + t
        return gt % 2, 16 * (gt // 2 + 1)

    def bg(layer, b):
        return layer * NBLK + b

    def cg(layer, g):
        return layer * c_.NCHL + g

    def parts_done_through_chunk(cgi):
        # s_peb counts aggregation *parts* (2 per block: A then B)
        layer, g = divmod(cgi, c_.NCHL)
        b0, nb = c_.chunks[g]
        return 2 * (layer * NBLK + b0 + nb)

    def _split1024(n):
        subs, off = [], 0
        while off < n:
            c2 = min(1024, n - off)
            subs.append((off, c2))
            off += c2
        return subs

    SUBS = [( _split1024(nb * c_.CAPA), _split1024(nb * c_.CAPB))
            for b0, nb in c_.chunks]
    SC = [len(a) + len(b) for a, b in SUBS]

    def gather_target(cgi):
        tot = 0
        for q in range(cgi % c_.NQ, cgi + 1, c_.NQ):
            tot += SC[q % c_.NCHL]
        return 16 * tot

    def cc_ar(layer):
        return 2 * layer + 1

    def cc_ag(layer):
        return 2 * layer + 2

    def pe3_val(layer, t):
        # init transposes occupy [1, NBLK]; layer l tile t -> NBLK + l*NBLK+t+1
        return NBLK + layer * NBLK + t + 1

    def pe3_init(t):
        return t + 1

    # tiles of block b: (msgs columns, drel global tile index)
    def block_tiles(g, lb):
        b0, nb = c_.chunks[g]
        b = b0 + lb
        tiles = []
        for i in range(c_.TPA):
            tiles.append((lb * c_.TPA + i, b * c_.TPA + i))
        for i in range(c_.TPB):
            tiles.append((nb * c_.TPA + lb * c_.TPB + i,
                          c_.ASZT + b * c_.TPB + i))
        return tiles

    with nc.Block() as block:

        # ================= SYNC: plain DMAs =================
        @block.sync
        def _(eng):
            dmac = [0]

            def dma(dst, src_ap):
                eng.dma_start(out=dst, in_=src_ap).then_inc(s_dma, 16)
                dmac[0] += 16

            dma(gidx_sb[:, :], gidx_p[:, :])
            dma(drel_sb[:, :], drel_p[:, :])
            dma(iota_sb[:, :], iota_p[:, :])
            dma(idn_sb[:, :], idn_p[:, :])
            dma(w_sb[:, :], wt_p[:, :])
            dma(pw1_sb[:, :], pw1_p[:, :])
            dma(pw2_sb[:, :], pw2_p[:, :])
            dma(pb1_sb[:, :], pb1_p[:, :])
            dma(gb_sb[:, :], gb_p[:, :])
            # x_own -> staging (node-major tiles, spans the msgs buffers)
            eng.wait_ge(s_vz, 1)   # staging pad rows zeroed
            for t0, n in STG_RANGES:
                bi, bc = stg(t0)
                dma(msgs[bi][:, bc:bc + n, :],
                    x_own[t0 * 128:(t0 + n) * 128, :]
                    .rearrange("(t p) d -> p t d", p=128))
            if REM:
                bi, bc = stg(NFULL)
                dma(msgs[bi][0:REM, bc, :], x_own[NFULL * 128:NPC, :])
            assert dmac[0] == dma_init

            if TRUNC == 'B':
                eng.wait_ge(s_cp, NBLK)
                eng.dma_start(out=out_p[:, :],
                              in_=x_aggrT[:, 0:NPC]).then_inc(s_dt[0], 16)
                return
            if TRUNC == 'C':
                eng.wait_ge(s_cp2, c_.NKC)
                eng.dma_start(out=out_p[:, :],
                              in_=h_mlpT[:, 0:NPC]).then_inc(s_dt[0], 16)
                return

            for l in range(L):
                # stats out
                eng.wait_ge(s_sq, l + 1)
                dma(stat_in[:, 0:6], stats_sb[:, 0:6])
                assert dmac[0] == dma_after_statin(l)
                # stats back
                eng.wait_ge(s_cc, cc_ar(l))
                dma(stats_g[:, 0:6], stat_out[:, 0:6])
                if TRUNC == 'E':
                    eng.wait_ge(s_hn, 1)
                    eng.dma_start(out=out_p[:, :],
                                  in_=h_bufs[(l + 1) % 2][:, 0:NPC])\
                        .then_inc(s_dt[0], 16)
                    return
                # h_new tiles out
                if l > 0:
                    eng.wait_ge(s_cc, cc_ag(l - 1))  # h_shard free
                for t in range(NBLK):
                    eng.wait_ge(s_s3, pe3_val(l, t))
                    slot = stage[:, t % 2, :]
                    sidx, _ = tile_sem(l, t)
                    if l < L - 1:
                        tgt, src_ap = h_shard[t * 128:(t + 1) * 128, :], slot
                    elif t < NPC // 128:
                        tgt, src_ap = out_p[t * 128:(t + 1) * 128, :], slot
                    else:
                        rem = NPC - (NPC // 128) * 128
                        tgt = out_p[t * 128:t * 128 + rem, :]
                        src_ap = stage[0:rem, t % 2, :]
                    eng.dma_start(out=tgt, in_=src_ap).then_inc(s_dt[sidx], 16)
                if l == L - 1:
                    # flush: nothing further
                    pass

        # ================= GPSIMD: gathers + collectives =================
        @block.gpsimd
        def _(eng):
            for l in range(L):
                hsrc = x_full if l == 0 else h_full
                if l == 0:
                    eng.wait_ge(s_dma, dma_init)
                else:
                    eng.wait_ge(s_cc, cc_ag(l - 1))
                for g, (b0, nb) in enumerate(c_.chunks):
                    cgi = cg(l, g)
                    if cgi <= c_.NQ - 1:
                        # msgs buffers double as the x_own staging buffer
                        eng.wait_ge(s_pe3, NBLK)
                    if cgi >= c_.NQ:
                        eng.wait_ge(s_peb,
                                    parts_done_through_chunk(cgi - c_.NQ))
                    buf = msgs[cgi % c_.NQ]
                    subsA, subsB = SUBS[g]
                    a0 = b0 * c_.CAPA
                    for off, cnt in subsA:
                        t0 = a0 + off
                        eng.dma_gather(
                            buf[:, off // 128:(off + cnt) // 128, :],
                            hsrc[0:c_.SPLIT, :],
                            gidx_sb[:, t0 // 16:(t0 + cnt) // 16],
                            cnt, cnt, D, queue_num=cgi % c_.NQ,
                        ).then_inc(s_g2[cgi % c_.NQ], 16)
                    b0tok = c_.ASZ + b0 * c_.CAPB
                    for off, cnt in subsB:
                        t0 = b0tok + off
                        bt = nb * c_.TPA + off // 128
                        eng.dma_gather(
                            buf[:, bt:bt + cnt // 128, :],
                            hsrc[c_.SPLIT:GN, :],
                            gidx_sb[:, t0 // 16:(t0 + cnt) // 16],
                            cnt, cnt, D, queue_num=cgi % c_.NQ,
                        ).then_inc(s_g2[cgi % c_.NQ], 16)
                if TRUNC in ('B', 'C'):
                    return
                # stats AllReduce
                eng.wait_ge(s_dma, dma_after_statin(l))
                eng.collective_compute(
                    "AllReduce", ALU.add, replica_groups=rg,
                    ins=[stat_in[:, 0:6].opt()], outs=[stat_out[:, 0:6].opt()],
                ).then_inc(s_cc, 1)
                if TRUNC == 'E':
                    return
                # h AllGather
                if l < L - 1:
                    for tq in (NBLK - 1, NBLK - 2):
                        if tq >= 0:
                            si, cnt = tile_sem(l, tq)
                            eng.wait_ge(s_dt[si], cnt)
                    eng.collective_compute(
                        "AllGather", ALU.bypass, replica_groups=rg,
                        ins=[h_shard[:, :].opt()], outs=[h_full[:, :].opt()],
                    ).then_inc(s_cc, 1)

        # ================= VECTOR =================
        @block.vector
        def _(eng):
            # init: zero staging pad region for partial x tile
            eng.memset(eps_sb[:, :], EPS)
            if REM:
                bi, bc = stg(NFULL)
                eng.memset(msgs[bi][:, bc, :], 0.0)
            eng.drain().then_inc(s_vz, 1)
            eng.wait_ge(s_dma, dma_init)

            for l in range(L):
                # --- phase B: one-hot generation per block ---
                for g, (b0, nb) in enumerate(c_.chunks):
                    for lb in range(nb):
                        b = b0 + lb
                        bgi = bg(l, b)
                        # parts: (A tiles, drel base) then (B tiles, base)
                        parts = [(c_.TPA, b * c_.TPA),
                                 (c_.TPB, c_.ASZT + b * c_.TPB)]
                        for pi, (tcnt, d0) in enumerate(parts):
                            pgi = 2 * bgi + pi
                            if pgi >= 2:
                                eng.wait_ge(s_peb, pgi - 1)
                            o = oh_buf[:, pgi % 2, 0:tcnt * 128]
                            o = o.rearrange("p (t j) -> p t j", j=128)
                            d_in = drel_sb[:, d0:d0 + tcnt].unsqueeze(-1)\
                                .broadcast_to([128, tcnt, 128])
                            i_in = iota_sb[:, :].unsqueeze(1)\
                                .broadcast_to([128, tcnt, 128])
                            eng.tensor_tensor(
                                out=o, in0=d_in, in1=i_in,
                                op=ALU.is_equal).then_inc(s_oh, 1)

                if TRUNC in ('B', 'C'):
                    return
                # --- phase D (squares are computed by the scalar engine) ---
                base2 = l * c_.NKC
                eng.wait_ge(s_cp2, base2 + c_.NKC)
                for j in range(3):
                    eng.reduce_sum(out=stats_sb[:, j:j + 1],
                                   in_=sum_cols[j][:, :], axis=AX)
                    eng.reduce_sum(out=stats_sb[:, 3 + j:4 + j],
                                   in_=sq_cols[j][:, :], axis=AX)
                eng.drain().then_inc(s_sq, 1)
                # affine math
                eng.wait_ge(s_dma, dma_after_statout(l))
                invn = 1.0 / c_.N
                eng.tensor_scalar_mul(means[:, 0:3], stats_g[:, 0:3], invn)
                eng.tensor_scalar_mul(msq[:, 0:3], stats_g[:, 3:6], invn)
                eng.drain()
                eng.tensor_tensor(out=var3[:, 0:3], in0=means[:, 0:3],
                                  in1=means[:, 0:3], op=ALU.mult)
                eng.drain()
                eng.tensor_sub(var3[:, 0:3], msq[:, 0:3], var3[:, 0:3])
                eng.drain().then_inc(s_var, 1)
                eng.wait_ge(s_sqr, l + 1)
                eng.reciprocal(rs3[:, 0:3], sd3[:, 0:3])
                eng.drain()
                eng.tensor_tensor(out=svec[:, 0:3], in0=rs3[:, 0:3],
                                  in1=gb_sb[:, 6 * l:6 * l + 3], op=ALU.mult)
                eng.drain()
                eng.tensor_tensor(out=mS[:, 0:3], in0=means[:, 0:3],
                                  in1=svec[:, 0:3], op=ALU.mult)
                eng.drain()
                eng.tensor_sub(t3[:, 0:3], gb_sb[:, 6 * l + 3:6 * l + 6],
                               mS[:, 0:3])
                eng.drain()
                eng.reduce_sum(out=tv[:, :], in_=t3[:, 0:3], axis=AX)
                eng.drain().then_inc(s_v2, 1)
                # --- phase E ---
                eng.tensor_scalar_mul(x_aggrT[:, 0:NPC], h_mlpT[:, 0:NPC],
                                      svec[:, 0:1])
                eng.drain()
                eng.scalar_tensor_tensor(
                    out=x_aggrT[:, 0:NPC], in0=ad0T[:, 0:NPC],
                    scalar=svec[:, 1:2], in1=x_aggrT[:, 0:NPC],
                    op0=ALU.mult, op1=ALU.add)
                eng.drain()
                eng.scalar_tensor_tensor(
                    out=x_aggrT[:, 0:NPC], in0=ad1T[:, 0:NPC],
                    scalar=svec[:, 2:3], in1=x_aggrT[:, 0:NPC],
                    op0=ALU.mult, op1=ALU.add)
                eng.drain().then_inc(s_acc, 1)
                if l == L - 1:
                    h_new = h_bufs[(l + 1) % 2]
                    inst = eng.tensor_scalar_add(h_new[:, 0:NPAD],
                                                 x_aggrT[:, 0:NPAD],
                                                 tv[:, 0:1])
                    inst.then_inc(s_hn, 1)

        # ================= TENSOR =================
        @block.tensor
        def _(eng):
            # init: build h_ownT from x staging
            eng.wait_ge(s_dma, dma_init)
            for t in range(NBLK):
                if t >= 2:
                    eng.wait_ge(s_s3, pe3_init(t) - 2)
                bi, bc = stg(t)
                inst = eng.transpose(
                    p_t2[t % 2][0:D, 0:128],
                    msgs[bi][:, bc, :], idn_sb[0:128, 0:128])
                inst.then_inc(s_pe3, 1)

            for l in range(L):
                h_own = h_bufs[l % 2]
                # --- phase B: aggregation matmuls ---
                for g, (b0, nb) in enumerate(c_.chunks):
                    cgi = cg(l, g)
                    eng.wait_ge(s_g2[cgi % c_.NQ], gather_target(cgi))
                    buf = msgs[cgi % c_.NQ]
                    for lb in range(nb):
                        b = b0 + lb
                        bgi = bg(l, b)
                        if bgi >= 2:
                            eng.wait_ge(s_cp, bgi - 1)
                        pslot = p_agg2[bgi % 2][0:D, 0:128]
                        # msgs columns: A tiles, then B tiles (per chunk)
                        parts = [
                            (c_.TPA, lambda i, lb=lb: lb * c_.TPA + i),
                            (c_.TPB, lambda i, lb=lb, nb=nb:
                                nb * c_.TPA + lb * c_.TPB + i),
                        ]
                        for pi, (tcnt, mcol_of) in enumerate(parts):
                            pgi = 2 * bgi + pi
                            eng.wait_ge(s_oh, pgi + 1)
                            slot = oh_buf[:, pgi % 2, :]
                            for i in range(tcnt):
                                inst = eng.matmul(
                                    pslot, buf[:, mcol_of(i), :],
                                    slot[:, i * 128:(i + 1) * 128],
                                    start=(pi == 0 and i == 0),
                                    stop=(pi == 1 and i == tcnt - 1))
                            inst.then_inc(s_peb, 1)
                if TRUNC == 'B':
                    return
                # --- phase C ---
                eng.wait_ge(s_cp, (l + 1) * NBLK)
                if l == 0:
                    eng.wait_ge(s_s3, NBLK)        # init copies done
                else:
                    eng.wait_ge(s_hn, l)           # h_own ready
                base2, base_s2 = l * c_.NKC * 5, l * c_.NKC * 2
                wl = w_sb[:, l * D:(l + 1) * D]
                for k, (o0, w) in enumerate(c_.kchunks):
                    if k >= 2:
                        eng.wait_ge(s_cp2, base2 // 5 + k - 1)
                    pc = p_c[k % 2][0:D, 0:w]
                    eng.matmul(pc, wl, x_aggrT[:, o0:o0 + w],
                               start=True, stop=True).then_inc(s_pe2, 1)
                    # adapter 0 hidden (input h_own)
                    ph = p_h[0:BOT, 0:w]
                    pw1_0 = pw1_sb[:, l * BOT:(l + 1) * BOT]
                    eng.matmul(ph, pw1_0, h_own[:, o0:o0 + w],
                               start=True, stop=True).then_inc(s_pe2, 1)
                    eng.wait_ge(s_s2, base_s2 + 2 * k + 1)
                    pa = p_a[0:D, 0:w]
                    pw2_0 = pw2_sb[0:BOT, l * D:(l + 1) * D]
                    eng.matmul(pa, pw2_0, hid0[0:BOT, 0:w],
                               start=True, stop=True).then_inc(s_pe2, 1)
                    # adapter 1 hidden (input x_aggr)
                    pw1_1 = pw1_sb[:, (L + l) * BOT:(L + l + 1) * BOT]
                    eng.matmul(ph, pw1_1, x_aggrT[:, o0:o0 + w],
                               start=True, stop=True).then_inc(s_pe2, 1)
                    eng.wait_ge(s_s2, base_s2 + 2 * k + 2)
                    pw2_1 = pw2_sb[0:BOT, (L + l) * D:(L + l + 1) * D]
                    eng.matmul(pa, pw2_1, hid1[0:BOT, 0:w],
                               start=True, stop=True).then_inc(s_pe2, 1)
                if TRUNC in ('C', 'E'):
                    return
                # --- phase F: transposes ---
                h_new = h_bufs[(l + 1) % 2]
                eng.wait_ge(s_hn, l + 1)
                for t in range(NBLK):
                    eng.wait_ge(s_s3, pe3_val(l, t) - 2)
                    inst = eng.transpose(
                        p_t2[t % 2][0:128, 0:64],
                        h_new[:, t * 128:(t + 1) * 128],
                        idn_sb[0:64, 0:64])
                    inst.then_inc(s_pe3, 1)

        # ================= SCALAR =================
        @block.scalar
        def _(eng):
            # init: drain h_ownT transposes ([64 feat, 128 nodes] psum slots)
            for t in range(NBLK):
                eng.wait_ge(s_pe3, pe3_init(t))
                inst = eng.activation(
                    h_bufs[0][:, t * 128:(t + 1) * 128],
                    p_t2[t % 2][0:D, 0:128],
                    ACTF.Copy)
                inst.then_inc(s_s3, 1)

            for l in range(L):
                # --- phase B: psum -> x_aggrT ---
                for b in range(NBLK):
                    bgi = bg(l, b)
                    eng.wait_ge(s_peb, 2 * bgi + 2)
                    pslot = p_agg2[bgi % 2][0:D, 0:128]
                    inst = eng.activation(
                        x_aggrT[:, b * 128:(b + 1) * 128], pslot, ACTF.Copy)
                    inst.then_inc(s_cp, 1)
                if TRUNC == 'B':
                    return
                # --- phase C ---
                base2, base_s2 = l * c_.NKC * 5, l * c_.NKC * 2
                oh_flat = oh_buf[:, :, :].rearrange("p q f -> p (q f)")
                for k, (o0, w) in enumerate(c_.kchunks):
                    we = w if o0 + w <= NPC else max(NPC - o0, 0)
                    assert 3 * we <= 2 * TMX * 128
                    scr = [oh_flat[0:D, j * we:(j + 1) * we] for j in range(3)]
                    eng.wait_ge(s_pe2, base2 + 5 * k + 1)
                    eng.activation(h_mlpT[:, o0:o0 + we],
                                   p_c[k % 2][0:D, 0:we], ACTF.Copy,
                                   accum_out=sum_cols[0][:, k:k + 1])
                    eng.activation(scr[0], p_c[k % 2][0:D, 0:we], ACTF.Square,
                                   accum_out=sq_cols[0][:, k:k + 1])
                    eng.wait_ge(s_pe2, base2 + 5 * k + 2)
                    pb1_0 = pb1_sb[0:BOT, l:l + 1]
                    inst = eng.activation(hid0[0:BOT, 0:w], p_h[0:BOT, 0:w],
                                          ACTF.Relu, bias=pb1_0)
                    inst.then_inc(s_s2, 1)
                    eng.wait_ge(s_pe2, base2 + 5 * k + 3)
                    eng.activation(ad0T[:, o0:o0 + we], p_a[0:D, 0:we],
                                   ACTF.Copy,
                                   accum_out=sum_cols[1][:, k:k + 1])
                    eng.activation(scr[1], p_a[0:D, 0:we], ACTF.Square,
                                   accum_out=sq_cols[1][:, k:k + 1])
                    eng.drain()   # a0 square must finish before PE reuses p_a
                    eng.wait_ge(s_pe2, base2 + 5 * k + 4)
                    pb1_1 = pb1_sb[0:BOT, L + l:L + l + 1]
                    inst = eng.activation(hid1[0:BOT, 0:w], p_h[0:BOT, 0:w],
                                          ACTF.Relu, bias=pb1_1)
                    inst.then_inc(s_s2, 1)
                    eng.wait_ge(s_pe2, base2 + 5 * k + 5)
                    eng.activation(ad1T[:, o0:o0 + we], p_a[0:D, 0:we],
                                   ACTF.Copy,
                                   accum_out=sum_cols[2][:, k:k + 1])
                    eng.activation(scr[2], p_a[0:D, 0:we], ACTF.Square,
                                   accum_out=sq_cols[2][:, k:k + 1])
                    eng.drain().then_inc(s_cp2, 1)
                if TRUNC == 'C':
                    return
                # --- phase D: sqrt(var + eps) ---
                eng.wait_ge(s_var, l + 1)
                eng.activation(sd3[:, 0:3], var3[:, 0:3], ACTF.Sqrt,
                               bias=eps_sb[:, 0:1]).then_inc(s_sqr, 1)
                # --- phase E: relu ---
                if l < L - 1:
                    h_new = h_bufs[(l + 1) % 2]
                    eng.wait_ge(s_acc, l + 1)
                    eng.activation(h_new[:, 0:NPC], x_aggrT[:, 0:NPC],
                                   ACTF.Relu, bias=tv[:, 0:1])
                    if NPAD > NPC:
                        eng.activation(h_new[:, NPC:NPAD],
                                       x_aggrT[:, NPC:NPAD],
                                       ACTF.Copy, scale=0.0)
                    eng.drain().then_inc(s_hn, 1)
                if TRUNC == 'E':
                    return
                # --- phase F: psum -> stage ---
                for t in range(NBLK):
                    gt = l * NBLK + t
                    if gt >= 2:
                        lp, tp = divmod(gt - 2, NBLK)
                        si, cnt = tile_sem(lp, tp)
                        eng.wait_ge(s_dt[si], cnt)
                    eng.wait_ge(s_pe3, pe3_val(l, t))
                    inst = eng.activation(
                        stage[:, t % 2, :],
                        p_t2[t % 2][0:128, 0:64],
                        ACTF.Copy)
                    inst.then_inc(s_s3, 1)

    ctx.close()
    nc.finalize()
    return nc


def _host_pack(cfg, W, pw1, pw2, pb1, bn_g, bn_b, pbn_g, pbn_b, gating):
    L, D, BOT = cfg.L, cfg.D, cfg.BOT
    wt = np.ascontiguousarray(W.transpose(1, 0, 2).reshape(D, L * D))
    pw1t = np.ascontiguousarray(
        pw1.transpose(2, 0, 1, 3).reshape(D, 2 * L * BOT))
    pw2t = np.zeros((16, 2 * L * D), np.float32)
    pw2t[0:BOT] = pw2.transpose(2, 0, 1, 3).reshape(BOT, 2 * L * D)
    pb1t = np.zeros((16, 2 * L), np.float32)
    pb1t[0:BOT] = pb1.transpose(2, 0, 1).reshape(BOT, 2 * L)
    gb = np.zeros((D, 6 * L), np.float32)
    for l in range(L):
        g0 = gating[0, l, 0]
        g1 = gating[1, l, 0]
        gb[:, 6 * l + 0] = bn_g[l]
        gb[:, 6 * l + 1] = pbn_g[0, l] * g0
        gb[:, 6 * l + 2] = pbn_g[1, l] * g1
        gb[:, 6 * l + 3] = bn_b[l]
        gb[:, 6 * l + 4] = pbn_b[0, l] * g0
        gb[:, 6 * l + 5] = pbn_b[1, l] * g1
    iota = np.tile(np.arange(128, dtype=np.float32), (128, 1))
    idn = np.eye(128, dtype=np.float32)
    return dict(wt=wt, pw1t=pw1t, pw2t=pw2t, pb1t=pb1t, gbvec=gb,
                iota=np.ascontiguousarray(iota), idn=idn)


def make_in_maps(cfg, inputs):
    c_ = cfg
    x = np.asarray(inputs["x"], np.float32)
    edge_index = np.asarray(inputs["edge_index"])
    gidx_w, drel = _prep_tokens(c_, edge_index)
    packs = _host_pack(c_, *[np.asarray(inputs[k], np.float32) for k in
                             ("W", "pw1", "pw2", "pb1", "bn_g", "bn_b",
                              "pbn_g", "pbn_b", "gating")])
    xpad = np.zeros((c_.GN, c_.D), np.float32)
    xpad.reshape(c_.NC, c_.NPAD, c_.D)[:, 0:c_.NPC] = \
        x.reshape(c_.NC, c_.NPC, c_.D)
    in_maps = []
    for i in range(c_.NC):
        m = dict(packs)
        m["x_full"] = xpad
        m["x_own"] = np.ascontiguousarray(
            x[i * c_.NPC:(i + 1) * c_.NPC])
        m["gidx"] = np.ascontiguousarray(gidx_w[i])
        m["drel"] = np.ascontiguousarray(drel[i])
        in_maps.append(m)
    return in_maps


def _make_cfg(inputs, N=50000, E=800000, D=64, L=5, BOT=15, NC=8,
              SPLIT=32768, CH_BLOCKS=1, NQ=4):
    edge_index = np.asarray(inputs["edge_index"])
    capa, capb = _caps_from_edges(dict(N=N, NC=NC, SPLIT=SPLIT), edge_index)
    return Cfg(N, E, D, L, BOT, NC, SPLIT, CH_BLOCKS, capa, capb, NQ=NQ)


_GRAPH_CACHE = {}


def kernel(**inputs) -> np.ndarray:
    cfg = _make_cfg(inputs)
    key = (cfg.CAPA, cfg.CAPB)
    if key not in _GRAPH_CACHE:
        _GRAPH_CACHE[key] = build_graph(cfg)
    nc = _GRAPH_CACHE[key]
    in_maps = make_in_maps(cfg, inputs)
    res = run_bass_kernel_spmd(nc, in_maps, core_ids=list(range(cfg.NC)))
    outs = [res.results[i]["out"] for i in range(cfg.NC)]
    return np.concatenate(outs, axis=0)



# revision 68
# speedup vs baseline: 2.3313x; 1.8184x over previous
"""AdapterGNN on 8 TRN2 NeuronCores.

Strategy (dst-node sharding, halved source tables):
  - Nodes sharded: core c owns nodes [c*6250, (c+1)*6250). All edges whose dst
    belongs to core c are processed by core c (~100k edges/core).
  - h_full (node-major, padded 6272 rows/core) is laid out as two tables:
    H1 = all cores' local positions [0, 3200) and H2 = positions [3200, 6272).
    Each layer the halves are published with two separate AllGathers so the
    next layer's H1-sourced gathers can start as soon as AllGather-1 lands.
  - Per dst block (128 nodes), tokens are split into an A region (src in H1)
    and a B region (src in H2); per-(block, region) gather calls stream over 4
    SWDGE queues (block mod 4), and a one-hot matmul on TensorE reduces token
    tiles into dst columns (segment-sum, f32 PSUM).  The A round lands via
    scalar copies into x_aggrT; the B round accumulates via vector adds.
  - conv/adapter compute (phase C) runs per 512-column chunk as soon as the
    chunk's aggregation columns are final, overlapped with the B round.
  - BatchNorm statistics are per-core partial sums + a [64,6] AllReduce;
    Linear biases feeding straight into BatchNorm (b, pb2) cancel and are
    skipped; gating is folded into the adapter BN affine on the host.
"""

import math
import os
import sys

import numpy as np

sys.path.insert(0, "/opt/trn_rl_repo")

from concourse import bass, mybir  # noqa: E402
from concourse.bacc import Bacc  # noqa: E402
from concourse.bass_utils import run_bass_kernel_spmd  # noqa: E402

F32 = mybir.dt.float32
BF16 = mybir.dt.bfloat16
I16 = mybir.dt.int16
AX = mybir.AxisListType.X
ALU = mybir.AluOpType
ACTF = mybir.ActivationFunctionType

EPS = 1e-5
TRUNC = None  # debug hang bisection: 'AGG' | 'C' | 'E' | 'F' | None


def _r16(v):
    return max(16, ((int(v) + 15) // 16) * 16)


class Cfg:
    def __init__(self, N, E, D, L, BOT, NC, capA, capB, NQ=4):
        self.N, self.E, self.D, self.L, self.BOT = N, E, D, L, BOT
        self.NC, self.NQ = NC, NQ
        self.NPC = N // NC                      # real nodes per core
        self.NBLK = (self.NPC + 127) // 128     # dst blocks per core
        self.NPAD = self.NBLK * 128             # padded nodes per core
        self.H1T = (self.NBLK + 1) // 2         # tiles in half 1
        self.H2T = self.NBLK - self.H1T
        self.H1 = self.H1T * 128                # local positions [0, H1)
        self.H2 = self.H2T * 128
        self.GA = NC * self.H1                  # A-table rows
        self.GB = NC * self.H2                  # B-table rows
        self.GN = self.GA + self.GB
        assert self.GA < 32768 and self.GB < 32768
        self.capA, self.capB = capA, capB       # per-block r16 token caps
        self.tA = [(c + 127) // 128 for c in capA]
        self.tB = [(c + 127) // 128 for c in capB]
        self.offA = np.concatenate([[0], np.cumsum(capA)]).astype(int)
        self.offB = np.concatenate([[0], np.cumsum(capB)]).astype(int)
        self.ASZ = int(self.offA[-1])           # B region token offset
        self.NTOK = self.ASZ + int(self.offB[-1])
        self.toffA = np.concatenate([[0], np.cumsum(self.tA)]).astype(int)
        self.toffB = np.concatenate([[0], np.cumsum(self.tB)]).astype(int)
        self.TTA = int(self.toffA[-1])
        self.NTILES = self.TTA + int(self.toffB[-1])
        self.TMAX = max(max(self.tA), max(self.tB))

        # gather pieces: ucode handles at most 1024 idxs per dma_gather
        def _splits(cap):
            out, off = [], 0
            while off < cap:
                n = min(1024, cap - off)
                out.append((off, n))
                off += n
            return out

        self.spA = [_splits(c) for c in capA]
        self.spB = [_splits(c) for c in capB]
        # per-queue cumulative piece counts (A-blocks == q mod NQ, then B)
        self.cumA = [0] * self.NBLK
        self.cumB = [0] * self.NBLK
        self.PA_q = [0] * NQ
        self.PB_q = [0] * NQ
        for b in range(self.NBLK):
            q = b % NQ
            self.PA_q[q] += len(self.spA[b])
            self.cumA[b] = self.PA_q[q]
        for b in range(self.NBLK):
            q = b % NQ
            self.PB_q[q] += len(self.spB[b])
            self.cumB[b] = self.PB_q[q]
        # phase-C column chunks over [0, NPAD)
        self.kchunks = []
        off = 0
        while off < self.NPAD:
            w = min(512, self.NPAD - off)
            self.kchunks.append((off, w))
            off += w
        self.NKC = len(self.kchunks)


def _src_tables(cfg, src):
    """Map global src node -> (isB, table row index)."""
    o = src // cfg.NPC
    p = src - o * cfg.NPC
    isB = (p >= cfg.H1).astype(np.int64)
    row = np.where(isB == 0, o * cfg.H1 + p, o * cfg.H2 + (p - cfg.H1))
    return isB, row


def _caps_from_edges(cfg_dims, edge_index):
    """Per-(block, region) r16 token caps (max over cores)."""
    N, NC = cfg_dims["N"], cfg_dims["NC"]
    NPC = N // NC
    NBLK = (NPC + 127) // 128
    H1 = ((NBLK + 1) // 2) * 128
    src = edge_index[0].astype(np.int64)
    dst = edge_index[1].astype(np.int64)
    owner = dst // NPC
    blk = (dst - owner * NPC) >> 7
    isB = ((src % NPC) >= H1).astype(np.int64)
    key = (owner * NBLK + blk) * 2 + isB
    counts = np.bincount(key, minlength=NC * NBLK * 2).reshape(NC, NBLK, 2)
    mx = counts.max(axis=0)
    capA = [_r16(v) for v in mx[:, 0]]
    capB = [_r16(v) for v in mx[:, 1]]
    return capA, capB


def _prep_tokens(cfg, edge_index):
    """Per-core token streams: gather idx (wrapped int16) + dst_rel (f32)."""
    c_ = cfg
    src = edge_index[0].astype(np.int64)
    dst = edge_index[1].astype(np.int64)
    owner = dst // c_.NPC
    dloc = dst - owner * c_.NPC
    blk = dloc >> 7
    rel = (dloc & 127).astype(np.float32)
    isB, row = _src_tables(c_, src)

    key = (owner * c_.NBLK + blk) * 2 + isB
    order = np.argsort(key, kind="stable")
    skey = key[order]
    counts = np.bincount(key, minlength=c_.NC * c_.NBLK * 2)
    starts = np.concatenate([[0], np.cumsum(counts)[:-1]])
    rank = np.arange(c_.E) - starts[skey]

    core = skey // (2 * c_.NBLK)
    remk = skey % (2 * c_.NBLK)
    b2 = remk // 2
    piece = remk % 2
    pos = np.where(piece == 0, c_.offA[b2] + rank,
                   c_.ASZ + c_.offB[b2] + rank)
    # tile-granular position for drel (tiles are 128-padded per call)
    tpos = np.where(piece == 0,
                    (c_.toffA[b2] + rank // 128) * 128 + rank % 128,
                    (c_.TTA + c_.toffB[b2] + rank // 128) * 128 + rank % 128)

    gs = row[order]
    assert gs.max() < 32768 and gs.min() >= 0

    gidx_val = np.zeros((c_.NC, c_.NTOK), np.int16)
    rel_flat = np.full((c_.NC, c_.NTILES * 128), -1.0, np.float32)
    gidx_val[core, pos] = gs.astype(np.int16)
    rel_flat[core, tpos] = rel[order]

    # wrapped layout [16, NTOK/16], replicated to all 8 groups of 16
    # partitions (each Q7 descriptor-gen core reads its own group)
    wrap = gidx_val.reshape(c_.NC, c_.NTOK // 16, 16).transpose(0, 2, 1)
    gidx_w = np.tile(wrap, (1, 8, 1)).astype(np.int16)
    drel = rel_flat.reshape(c_.NC, c_.NTILES, 128).transpose(0, 2, 1).copy()
    return gidx_w, drel


def build_graph(cfg):
    c_ = cfg
    D, BOT, L, NQ = c_.D, c_.BOT, c_.L, c_.NQ
    NPC, NPAD, NBLK, GN, GA = c_.NPC, c_.NPAD, c_.NBLK, c_.GN, c_.GA

    nc = Bacc(target_bir_lowering=False, debug=False, num_swdge_queues=NQ,
              dynamic_dma_scratch_size=32768)

    # ---------- dram parameters ----------
    x_fullA = nc.declare_dram_parameter("x_fullA", [GA, 128], BF16,
                                        isOutput=False)
    x_fullB = nc.declare_dram_parameter("x_fullB", [GN - GA, 128], BF16,
                                        isOutput=False)
    x_own = nc.declare_dram_parameter("x_own", [NPC, D], F32,
                                      isOutput=False)
    gidx_p = nc.declare_dram_parameter("gidx", [128, c_.NTOK // 16], I16,
                                       isOutput=False)
    drel_p = nc.declare_dram_parameter("drel", [128, c_.NTILES], BF16,
                                       isOutput=False)
    iota_p = nc.declare_dram_parameter("iota", [128, 128], BF16,
                                       isOutput=False)
    idn_p = nc.declare_dram_parameter("idn", [128, 128], F32, isOutput=False)
    idnh_p = nc.declare_dram_parameter("idnh", [128, 128], BF16,
                                       isOutput=False)
    wt_p = nc.declare_dram_parameter("wt", [D, L * D], F32, isOutput=False)
    pw1_p = nc.declare_dram_parameter("pw1t", [D, 2 * L * BOT], F32,
                                      isOutput=False)
    pw2_p = nc.declare_dram_parameter("pw2t", [16, 2 * L * D], F32,
                                      isOutput=False)
    pb1_p = nc.declare_dram_parameter("pb1t", [16, 2 * L], F32, isOutput=False)
    gb_p = nc.declare_dram_parameter("gbvec", [D, 6 * L], F32, isOutput=False)
    out_p = nc.declare_dram_parameter("out", [NPC, D], F32, isOutput=True)

    # ---------- internal dram ----------
    h_sh1 = nc.dram_tensor("h_sh1", [c_.H1, 128], BF16)
    h_sh2 = nc.dram_tensor("h_sh2", [c_.H2, 128], BF16)
    h_fullA = nc.dram_tensor("h_fullA", [GA, 128], BF16, addr_space="Shared")
    h_fullB = nc.dram_tensor("h_fullB", [GN - GA, 128], BF16,
                             addr_space="Shared")
    stat_in = nc.dram_tensor("stat_in", [D, 6], F32)
    stat_out = nc.dram_tensor("stat_out", [D, 6], F32, addr_space="Shared")

    rg = [list(range(c_.NC))]

    import contextlib
    ctx = contextlib.ExitStack()

    def sb(name, shape, dt=F32):
        return ctx.enter_context(nc.sbuf_tensor(name, shape, dt))

    def ps(name, shape):
        return ctx.enter_context(nc.psum_tensor(name, shape, F32))

    def sem(name):
        return ctx.enter_context(nc.semaphore(name))

    # ---------- sbuf ----------
    MCOL = max(c_.TMAX, (NBLK + NQ - 1) // NQ)    # msgs columns per queue
    h_bufs = [sb("h0", [D, NPAD]), sb("h1", [D, NPAD])]
    x_aggrT = sb("x_aggrT", [D, NPAD])          # also phase-E accumulator
    h_mlpT = sb("h_mlpT", [D, NPAD])
    ad0T = sb("ad0T", [D, NPAD])
    ad1T = sb("ad1T", [D, NPAD])
    msgs = [sb(f"msgs{i}", [128, MCOL, 128], BF16) for i in range(NQ)]
    oh_buf = sb("oh_buf", [128, 2, c_.TMAX * 128], BF16)  # one-hot, 2 slots
    drel_sb = sb("drel_sb", [128, c_.NTILES], BF16)
    gidx_sb = sb("gidx_sb", [128, c_.NTOK // 16], I16)
    iota_sb = sb("iota_sb", [128, 128], BF16)
    idn_sb = sb("idn_sb", [128, 128])
    idnh_sb = sb("idnh_sb", [128, 128], BF16)
    w_sb = sb("w_sb", [D, L * D])
    pw1_sb = sb("pw1_sb", [D, 2 * L * BOT])
    pw2_sb = sb("pw2_sb", [16, 2 * L * D])
    pb1_sb = sb("pb1_sb", [16, 2 * L])
    gb_sb = sb("gb_sb", [D, 6 * L])
    hid0 = sb("hid0", [16, 512])
    hid1 = sb("hid1", [16, 512])
    sq_scr = sb("sq_scr", [D, 512])             # square-activation dump
    tmpB = sb("tmpB", [D, 2, 128])              # B-part psum drain, 2 slots
    sum_cols = [sb(f"sum_cols{i}", [D, c_.NKC]) for i in range(3)]
    sq_cols = [sb(f"sq_cols{i}", [D, c_.NKC]) for i in range(3)]
    stats_sb = sb("stats_sb", [D, 8])
    stats_g = sb("stats_g", [D, 8])
    means = sb("means", [D, 4])
    msq = sb("msq", [D, 4])
    var3 = sb("var3", [D, 4])
    sd3 = sb("sd3", [D, 4])
    rs3 = sb("rs3", [D, 4])
    svec = sb("svec", [D, 4])
    mS = sb("mS", [D, 4])
    t3 = sb("t3", [D, 4])
    tv = sb("tv", [D, 1])
    eps_sb = sb("eps_sb", [D, 1])
    stage = sb("stage", [128, 2, D])          # f32 drain (last layer)
    stageh = sb("stageh", [128, 2, 128], BF16)  # bf16 drain (publish)

    # ---------- psum ----------
    p_agg2 = [ps("p_agg0", [128, 128]), ps("p_agg1", [128, 128])]
    p_c = [ps("p_c0", [128, 512]), ps("p_c1", [128, 512])]
    p_h = ps("p_h", [128, 512])
    p_a = ps("p_a", [128, 512])
    p_t2 = [ps("p_t0", [128, 128]), ps("p_t1", [128, 128])]

    # ---------- semaphores ----------
    s_g = [sem(f"g{i}") for i in range(NQ)]  # +16 per gather call (by queue)
    s_prep = [sem(f"pr{i}") for i in range(NQ)]  # +1 per desc-gen prep
    s_gx = sem("gx")          # +16 when gidx_sb is loaded
    s_oh = sem("oh")          # +1 per one-hot job (vector)
    s_peb = sem("peb")        # +1 per agg part (tensor): 2*NBLK per layer
    s_cp = sem("cp")          # +1 per A copy (scalar)
    s_cpb = sem("cpb")        # +1 per B psum->tmpB copy (scalar)
    s_badd = sem("badd")      # +1 per B add (vector)
    s_pe2 = sem("pe2")        # +5 per phase-C chunk (tensor)
    s_s2 = sem("s2")          # +2 per phase-C chunk (scalar relu)
    s_cp2 = sem("cp2")        # +1 per phase-C chunk copied (scalar)
    s_var = sem("var")        # +1 per layer (vector: vars ready)
    s_sqr = sem("sqr")        # +1 per layer (scalar: sqrt done)
    s_v2 = sem("v2")          # +1 per layer (vector: affines ready)
    s_acc = sem("acc")        # +2 per layer (vector: E halves done)
    s_hn = sem("hn")          # +2 per layer (h_new halves ready)
    s_pe3 = sem("pe3")        # +1 per transpose (tensor)
    s_s3 = sem("s3")          # +1 per stage copy (scalar)
    s_dma = sem("dma")        # +16 per sync DMA (init + stats)
    s_dt = [sem("dt0"), sem("dt1")]   # +16 per tile DMA, parity by tile
    s_cc = sem("cc")          # +1 per collective
    s_sq = sem("sq")          # +1 per layer (vector stats ready)
    s_vz = sem("vz")          # +1 init memset

    # ---------- schedule bookkeeping ----------
    NFULL = NPC // 128
    REM = NPC - NFULL * 128
    assert NQ * MCOL >= NBLK, "staging must fit in msgs buffers"

    def stg(t):
        return (t // MCOL, t % MCOL)

    STG_RANGES = []
    t0 = 0
    while t0 < NFULL:
        n = min(MCOL - (t0 % MCOL), NFULL - t0)
        STG_RANGES.append((t0, n))
        t0 += n

    # gidx DMA increments s_gx instead of s_dma (one update per DMA);
    # 9 params (incl. idnh) + staging
    N_INIT_DMA = 9 + len(STG_RANGES) + (1 if REM else 0)
    dma_init = 16 * N_INIT_DMA

    def dma_after_statin(layer):
        return 16 * (N_INIT_DMA + 2 * layer + 1)

    def dma_after_statout(layer):
        return dma_after_statin(layer) + 16

    def tile_sem(layer, t):
        gt = layer * NBLK + t
        return gt % 2, 16 * (gt // 2 + 1)

    # collective ids: per layer AR, then (if l < L-1) AG1, AG2
    cc_n = 0
    cc_ar_, cc_ag1_, cc_ag2_ = {}, {}, {}
    for l in range(L):
        cc_n += 1
        cc_ar_[l] = cc_n
        if l < L - 1:
            cc_n += 1
            cc_ag1_[l] = cc_n
            cc_n += 1
            cc_ag2_[l] = cc_n

    # per-queue cumulative gather-piece count after (layer, region, block)
    def gcount(l, region, b):
        q = b % NQ
        base = l * (c_.PA_q[q] + c_.PB_q[q])
        if region == 0:
            return 16 * (base + c_.cumA[b])
        return 16 * (base + c_.PA_q[q] + c_.cumB[b])

    # part index helpers (s_peb order: per layer A blocks, then B blocks)
    def part_a(l, b):
        return l * 2 * NBLK + b

    def part_b(l, b):
        return l * 2 * NBLK + NBLK + b

    # largest A/B-block on queue q
    bmax_q = [max(range(q, NBLK, NQ)) for q in range(NQ)]

    def pe3_val(layer, t):
        return NBLK + layer * NBLK + t + 1

    def pe3_init(t):
        return t + 1

    H1T, H2T = c_.H1T, c_.H2T

    with nc.Block() as block:

        # ================= SYNC: plain DMAs =================
        @block.sync
        def _(eng):
            dmac = [0]

            def dma(dst, src_ap):
                eng.dma_start(out=dst, in_=src_ap).then_inc(s_dma, 16)
                dmac[0] += 16

            eng.dma_start(out=gidx_sb[:, :], in_=gidx_p[:, :])\
                .then_inc(s_gx, 16)
            dma(drel_sb[:, :], drel_p[:, :])
            dma(iota_sb[:, :], iota_p[:, :])
            dma(idn_sb[:, :], idn_p[:, :])
            dma(idnh_sb[:, :], idnh_p[:, :])
            dma(w_sb[:, :], wt_p[:, :])
            dma(pw1_sb[:, :], pw1_p[:, :])
            dma(pw2_sb[:, :], pw2_p[:, :])
            dma(pb1_sb[:, :], pb1_p[:, :])
            dma(gb_sb[:, :], gb_p[:, :])
            # x_own -> staging (node-major tiles, spans the msgs buffers)
            eng.wait_ge(s_vz, 1)   # staging pad rows zeroed
            for t0, n in STG_RANGES:
                bi, bc = stg(t0)
                dma(msgs[bi][:, bc:bc + n, :].bitcast(F32),
                    x_own[t0 * 128:(t0 + n) * 128, :]
                    .rearrange("(t p) d -> p t d", p=128))
            if REM:
                bi, bc = stg(NFULL)
                dma(msgs[bi][0:REM, bc, :].bitcast(F32),
                    x_own[NFULL * 128:NPC, :])
            assert dmac[0] == dma_init

            if TRUNC in ('AGG', 'C'):
                if TRUNC == 'AGG':
                    eng.wait_ge(s_badd, NBLK)
                    src = x_aggrT
                else:
                    eng.wait_ge(s_cp2, c_.NKC)
                    src = h_mlpT
                eng.dma_start(out=out_p[:, :],
                              in_=src[:, 0:NPC]).then_inc(s_dt[0], 16)
                return

            for l in range(L):
                # stats out
                eng.wait_ge(s_sq, l + 1)
                dma(stat_in[:, 0:6], stats_sb[:, 0:6])
                assert dmac[0] == dma_after_statin(l)
                # stats back
                eng.wait_ge(s_cc, cc_ar_[l])
                dma(stats_g[:, 0:6], stat_out[:, 0:6])
                if TRUNC == 'E':
                    eng.wait_ge(s_hn, 2)
                    eng.dma_start(out=out_p[:, :],
                                  in_=h_bufs[1][:, 0:NPC])\
                        .then_inc(s_dt[0], 16)
                    return
                # h_new tiles out
                for t in range(NBLK):
                    if l > 0:
                        if t == 0:
                            eng.wait_ge(s_cc, cc_ag1_[l - 1])  # h_sh1 free
                        elif t == H1T:
                            eng.wait_ge(s_cc, cc_ag2_[l - 1])  # h_sh2 free
                    eng.wait_ge(s_s3, pe3_val(l, t))
                    slot = stage[:, t % 2, :]
                    sidx, _ = tile_sem(l, t)
                    if l < L - 1:
                        if t < H1T:
                            tgt = h_sh1[t * 128:(t + 1) * 128, :]
                        else:
                            t2 = t - H1T
                            tgt = h_sh2[t2 * 128:(t2 + 1) * 128, :]
                        src_ap = stageh[:, t % 2, :]
                    elif t < NPC // 128:
                        tgt, src_ap = out_p[t * 128:(t + 1) * 128, :], slot
                    else:
                        rem = NPC - (NPC // 128) * 128
                        tgt = out_p[t * 128:t * 128 + rem, :]
                        src_ap = stage[0:rem, t % 2, :]
                    eng.dma_start(out=tgt, in_=src_ap).then_inc(s_dt[sidx], 16)
                if TRUNC == 'G' and l == 0:
                    eng.wait_ge(s_badd, 2 * NBLK)
                    eng.dma_start(out=out_p[:, :],
                                  in_=x_aggrT[:, 0:NPC]).then_inc(s_dt[0], 16)
                    return
                if TRUNC == 'F':
                    eng.wait_ge(s_cc, cc_ag2_[0])
                    eng.dma_start(out=out_p[:, :],
                                  in_=h_bufs[1][:, 0:NPC])\
                        .then_inc(s_dt[0], 16)
                    return

        # ================= GPSIMD: gathers + collectives =================
        # Desc-gen runs ahead via prepare_only; triggers (which carry the
        # data/buffer waits) fire the DMAs.  The SWDGE ring buffers ~9
        # pieces/queue so the Q7 cores stay busy through the layer tail.
        @block.gpsimd
        def _(eng):
            # piece list: (layer, region, block, off, cnt, first)
            pieces = []
            lstart = []
            for l in range(L):
                lstart.append(len(pieces))
                for b in range(NBLK):
                    for i, (off, cnt) in enumerate(c_.spA[b]):
                        pieces.append((l, 0, b, off, cnt, i == 0))
                for b in range(NBLK):
                    for i, (off, cnt) in enumerate(c_.spB[b]):
                        pieces.append((l, 1, b, off, cnt, i == 0))
            total = len(pieces)
            PPL = total // L                     # pieces per layer
            # per-queue prep counts through piece i
            prep_cnt = [0] * NQ
            prep_thru = [0] * total
            for i, (l, r, b, off, cnt, first) in enumerate(pieces):
                prep_cnt[b % NQ] += 1
                prep_thru[i] = prep_cnt[b % NQ]

            pe_ptr = [0]
            ring_occ = [0] * NQ   # descs prepped but not yet triggered

            def emit_prep(i):
                l, r, b, off, cnt, first = pieces[i]
                q = b % NQ
                if i == 0:
                    eng.wait_ge(s_gx, 16)
                if r == 0:
                    src = x_fullA[:, :] if l == 0 else h_fullA[:, :]
                    t0 = int(c_.offA[b]) + off
                else:
                    src = x_fullB[:, :] if l == 0 else h_fullB[:, :]
                    t0 = c_.ASZ + int(c_.offB[b]) + off
                eng.dma_gather(
                    msgs[q][:, off // 128:(off + cnt + 127) // 128, :], src,
                    gidx_sb[:, t0 // 16:(t0 + cnt) // 16],
                    cnt, cnt, 128, queue_num=q,
                    prepare_only=True, sem=s_g[q],
                ).then_inc(s_prep[q], 1)
                ring_occ[q] += cnt // 16 + 1
                assert ring_occ[q] <= 1400, (q, ring_occ[q])

            def fill(n):
                k = min(n, total - pe_ptr[0])
                for _ in range(k):
                    emit_prep(pe_ptr[0])
                    pe_ptr[0] += 1

            def emit_trigger(i):
                l, r, b, off, cnt, first = pieces[i]
                q = b % NQ
                if first:
                    if r == 0:
                        if l > 0 and b == 0:
                            eng.wait_ge(s_cc, cc_ag1_[l - 1])
                        if b >= NQ:
                            eng.wait_ge(s_peb, part_a(l, b - NQ) + 1)
                        elif l == 0:
                            eng.wait_ge(s_vz, 2)   # staging consumed + zeroed
                        else:
                            eng.wait_ge(s_peb,
                                        part_b(l - 1, bmax_q[q]) + 1)
                    else:
                        if l > 0 and b == 0:
                            eng.wait_ge(s_cc, cc_ag2_[l - 1])
                        if b >= NQ:
                            eng.wait_ge(s_peb, part_b(l, b - NQ) + 1)
                        else:
                            eng.wait_ge(s_peb, part_a(l, bmax_q[q]) + 1)
                eng.wait_ge(s_prep[q], prep_thru[i])
                eng.trigger_dma(count=1, queue_num=q)
                ring_occ[q] -= cnt // 16 + 1

            def emit_events(l):
                # stats AllReduce for layer l; AllGathers if more layers
                eng.wait_ge(s_dma, dma_after_statin(l))
                eng.collective_compute(
                    "AllReduce", ALU.add, replica_groups=rg,
                    ins=[stat_in[:, 0:6].opt()],
                    outs=[stat_out[:, 0:6].opt()],
                ).then_inc(s_cc, 1)
                fill(16)
                if l < L - 1:
                    for tq in (H1T - 1, H1T - 2):
                        si, cnt = tile_sem(l, tq)
                        eng.wait_ge(s_dt[si], cnt)
                    eng.collective_compute(
                        "AllGather", ALU.bypass, replica_groups=rg,
                        ins=[h_sh1[:, :].opt()],
                        outs=[h_fullA[:, :].opt()],
                    ).then_inc(s_cc, 1)
                    fill(16)
                    for tq in (NBLK - 1, NBLK - 2):
                        si, cnt = tile_sem(l, tq)
                        eng.wait_ge(s_dt[si], cnt)
                    eng.collective_compute(
                        "AllGather", ALU.bypass, replica_groups=rg,
                        ins=[h_sh2[:, :].opt()],
                        outs=[h_fullB[:, :].opt()],
                    ).then_inc(s_cc, 1)
                    fill(16)

            assert TRUNC is None, "TRUNC unsupported with prep/trigger"
            for j in range(total):
                l, i_in_l = divmod(j, PPL)
                if i_in_l == 0 and l > 0:
                    emit_events(l - 1)
                W = min(total, lstart[l] + 20 + i_in_l)
                while pe_ptr[0] < W:
                    emit_prep(pe_ptr[0])
                    pe_ptr[0] += 1
                emit_trigger(j)
            emit_events(L - 1)

        # ================= VECTOR =================
        @block.vector
        def _(eng):
            # init: zero staging pad region for partial x tile
            eng.memset(eps_sb[:, :], EPS)
            eng.memset(stageh[:, :, :], 0.0)
            if REM:
                bi, bc = stg(NFULL)
                eng.memset(msgs[bi][:, bc, :], 0.0)
            eng.drain().then_inc(s_vz, 1)
            eng.wait_ge(s_dma, dma_init)
            # zero token buffers after staging is consumed: partial gather
            # tiles leave stale rows; they must be finite (0 * NaN = NaN)
            eng.wait_ge(s_pe3, NBLK)
            for q in range(NQ):
                eng.memset(msgs[q][:, :, :], 0.0)
            eng.drain().then_inc(s_vz, 1)

            for l in range(L):
                # --- A one-hots ---
                for b in range(NBLK):
                    j = part_a(l, b)
                    if j >= 2:
                        eng.wait_ge(s_peb, j - 1)
                    tcnt = c_.tA[b]
                    d0 = int(c_.toffA[b])
                    o = oh_buf[:, j % 2, 0:tcnt * 128]
                    o = o.rearrange("p (t j) -> p t j", j=128)
                    d_in = drel_sb[:, d0:d0 + tcnt].unsqueeze(-1)\
                        .broadcast_to([128, tcnt, 128])
                    i_in = iota_sb[:, :].unsqueeze(1)\
                        .broadcast_to([128, tcnt, 128])
                    eng.tensor_tensor(
                        out=o, in0=d_in, in1=i_in,
                        op=ALU.is_equal).then_inc(s_oh, 1)
                # --- B one-hots + B adds (lag 2) ---
                def b_add(b):
                    eng.wait_ge(s_cpb, l * NBLK + b + 1)
                    eng.tensor_tensor(
                        out=x_aggrT[:, b * 128:(b + 1) * 128],
                        in0=tmpB[:, b % 2, :],
                        in1=x_aggrT[:, b * 128:(b + 1) * 128],
                        op=ALU.add).then_inc(s_badd, 1)

                for b in range(NBLK):
                    j = part_b(l, b)
                    if j >= 2:
                        eng.wait_ge(s_peb, j - 1)
                    tcnt = c_.tB[b]
                    d0 = c_.TTA + int(c_.toffB[b])
                    o = oh_buf[:, j % 2, 0:tcnt * 128]
                    o = o.rearrange("p (t j) -> p t j", j=128)
                    d_in = drel_sb[:, d0:d0 + tcnt].unsqueeze(-1)\
                        .broadcast_to([128, tcnt, 128])
                    i_in = iota_sb[:, :].unsqueeze(1)\
                        .broadcast_to([128, tcnt, 128])
                    eng.tensor_tensor(
                        out=o, in0=d_in, in1=i_in,
                        op=ALU.is_equal).then_inc(s_oh, 1)
                    if b >= 2:
                        b_add(b - 2)
                b_add(NBLK - 2)
                b_add(NBLK - 1)
                if TRUNC in ('AGG', 'C') or (TRUNC == 'G' and l == 1):
                    return

                # --- stats reduce ---
                eng.wait_ge(s_cp2, (l + 1) * c_.NKC)
                for j in range(3):
                    eng.reduce_sum(out=stats_sb[:, j:j + 1],
                                   in_=sum_cols[j][:, :], axis=AX)
                    eng.reduce_sum(out=stats_sb[:, 3 + j:4 + j],
                                   in_=sq_cols[j][:, :], axis=AX)
                eng.drain().then_inc(s_sq, 1)
                # --- affine math ---
                eng.wait_ge(s_dma, dma_after_statout(l))
                invn = 1.0 / c_.N
                eng.tensor_scalar_mul(means[:, 0:3], stats_g[:, 0:3], invn)
                eng.tensor_scalar_mul(msq[:, 0:3], stats_g[:, 3:6], invn)
                eng.drain()
                eng.tensor_tensor(out=var3[:, 0:3], in0=means[:, 0:3],
                                  in1=means[:, 0:3], op=ALU.mult)
                eng.drain()
                eng.tensor_sub(var3[:, 0:3], msq[:, 0:3], var3[:, 0:3])
                eng.drain().then_inc(s_var, 1)
                eng.wait_ge(s_sqr, l + 1)
                eng.reciprocal(rs3[:, 0:3], sd3[:, 0:3])
                eng.drain()
                eng.tensor_tensor(out=svec[:, 0:3], in0=rs3[:, 0:3],
                                  in1=gb_sb[:, 6 * l:6 * l + 3], op=ALU.mult)
                eng.drain()
                eng.tensor_tensor(out=mS[:, 0:3], in0=means[:, 0:3],
                                  in1=svec[:, 0:3], op=ALU.mult)
                eng.drain()
                eng.tensor_sub(t3[:, 0:3], gb_sb[:, 6 * l + 3:6 * l + 6],
                               mS[:, 0:3])
                eng.drain()
                eng.reduce_sum(out=tv[:, :], in_=t3[:, 0:3], axis=AX)
                eng.drain().then_inc(s_v2, 1)
                # --- phase E (halves; only real node columns) ---
                for h0, hw in ((0, c_.H1), (c_.H1, NPC - c_.H1)):
                    sl = slice(h0, h0 + hw)
                    eng.tensor_scalar_mul(x_aggrT[:, sl], h_mlpT[:, sl],
                                          svec[:, 0:1])
                    eng.drain()
                    eng.scalar_tensor_tensor(
                        out=x_aggrT[:, sl], in0=ad0T[:, sl],
                        scalar=svec[:, 1:2], in1=x_aggrT[:, sl],
                        op0=ALU.mult, op1=ALU.add)
                    eng.drain()
                    eng.scalar_tensor_tensor(
                        out=x_aggrT[:, sl], in0=ad1T[:, sl],
                        scalar=svec[:, 2:3], in1=x_aggrT[:, sl],
                        op0=ALU.mult, op1=ALU.add)
                    if l == L - 1:
                        eng.drain()
                        eng.tensor_scalar_add(
                            h_bufs[(l + 1) % 2][:, sl],
                            x_aggrT[:, sl], tv[:, 0:1])
                    eng.drain().then_inc(s_acc, 1)
                if TRUNC in ('E', 'F'):
                    return

        # ================= TENSOR =================
        @block.tensor
        def _(eng):
            # init: build h_ownT from x staging
            eng.wait_ge(s_dma, dma_init)
            for t in range(NBLK):
                if t >= 2:
                    eng.wait_ge(s_s3, pe3_init(t) - 2)
                bi, bc = stg(t)
                inst = eng.transpose(
                    p_t2[t % 2][0:D, 0:128],
                    msgs[bi][:, bc, :].bitcast(F32),
                    idn_sb[0:128, 0:128])
                inst.then_inc(s_pe3, 1)

            for l in range(L):
                h_own = h_bufs[l % 2]
                base2 = l * c_.NKC * 5
                base_s2 = l * c_.NKC * 2
                wl = w_sb[:, l * D:(l + 1) * D]

                def c_chunk(k):
                    o0, w = c_.kchunks[k]
                    # aggregation columns for this chunk final (B adds)
                    eng.wait_ge(s_badd,
                                l * NBLK + min(4 * (k + 1), NBLK))
                    if k >= 2:
                        eng.wait_ge(s_cp2, l * c_.NKC + k - 1)
                    pc = p_c[k % 2][0:D, 0:w]
                    eng.matmul(pc, wl, x_aggrT[:, o0:o0 + w],
                               start=True, stop=True).then_inc(s_pe2, 1)
                    # adapter 0 hidden (input h_own)
                    ph = p_h[0:BOT, 0:w]
                    pw1_0 = pw1_sb[:, l * BOT:(l + 1) * BOT]
                    eng.matmul(ph, pw1_0, h_own[:, o0:o0 + w],
                               start=True, stop=True).then_inc(s_pe2, 1)
                    eng.wait_ge(s_s2, base_s2 + 2 * k + 1)
                    pa_ = p_a[0:D, 0:w]
                    pw2_0 = pw2_sb[0:BOT, l * D:(l + 1) * D]
                    eng.matmul(pa_, pw2_0, hid0[0:BOT, 0:w],
                               start=True, stop=True).then_inc(s_pe2, 1)
                    # adapter 1 hidden (input x_aggr)
                    pw1_1 = pw1_sb[:, (L + l) * BOT:(L + l + 1) * BOT]
                    eng.matmul(ph, pw1_1, x_aggrT[:, o0:o0 + w],
                               start=True, stop=True).then_inc(s_pe2, 1)
                    eng.wait_ge(s_s2, base_s2 + 2 * k + 2)
                    pw2_1 = pw2_sb[0:BOT, (L + l) * D:(L + l + 1) * D]
                    eng.matmul(pa_, pw2_1, hid1[0:BOT, 0:w],
                               start=True, stop=True).then_inc(s_pe2, 1)

                if l == 0:
                    eng.wait_ge(s_s3, NBLK)        # init copies done
                # --- A parts ---
                for b in range(NBLK):
                    pa = part_a(l, b)
                    q = b % NQ
                    eng.wait_ge(s_oh, pa + 1)
                    eng.wait_ge(s_g[q], gcount(l, 0, b))
                    # psum slot reuse (consumer of part pa-2 done)
                    if b >= 2:
                        eng.wait_ge(s_cp, l * NBLK + b - 1)
                    elif l > 0:
                        # consumer of part pa-2 = B-add (l-1, NBLK-2+b)
                        eng.wait_ge(s_badd, l * NBLK + b - 1)
                    pslot = p_agg2[pa % 2][0:D, 0:128]
                    oh = oh_buf[:, pa % 2, :]
                    for i in range(c_.tA[b]):
                        inst = eng.matmul(
                            pslot, msgs[q][:, i, 0:D],
                            oh[:, i * 128:(i + 1) * 128],
                            start=(i == 0), stop=(i == c_.tA[b] - 1))
                    inst.then_inc(s_peb, 1)
                # --- B parts + phase C interleave ---
                nextk = 0
                for b in range(NBLK):
                    pa = part_b(l, b)
                    q = b % NQ
                    eng.wait_ge(s_oh, pa + 1)
                    eng.wait_ge(s_g[q], gcount(l, 1, b))
                    if b >= 2:
                        eng.wait_ge(s_badd, l * NBLK + b - 1)
                    else:
                        eng.wait_ge(s_cp, l * NBLK + NBLK - 2 + b + 1)
                    pslot = p_agg2[pa % 2][0:D, 0:128]
                    oh = oh_buf[:, pa % 2, :]
                    for i in range(c_.tB[b]):
                        inst = eng.matmul(
                            pslot, msgs[q][:, i, 0:D],
                            oh[:, i * 128:(i + 1) * 128],
                            start=(i == 0), stop=(i == c_.tB[b] - 1))
                    inst.then_inc(s_peb, 1)
                    if TRUNC == 'AGG' or (TRUNC == 'G' and l == 1):
                        continue
                    if nextk < c_.NKC and b == min(4 * (nextk + 1) + 1,
                                                   NBLK - 1):
                        c_chunk(nextk)
                        nextk += 1
                if TRUNC == 'AGG' or (TRUNC == 'G' and l == 1):
                    return
                while nextk < c_.NKC:
                    c_chunk(nextk)
                    nextk += 1
                if TRUNC in ('C', 'E'):
                    return
                # --- phase F: transposes (halves) ---
                h_new = h_bufs[(l + 1) % 2]
                for t in range(NBLK):
                    eng.wait_ge(s_hn, 2 * l + (1 if t < H1T else 2))
                    eng.wait_ge(s_s3, pe3_val(l, t) - 2)
                    inst = eng.transpose(
                        p_t2[t % 2][0:128, 0:64],
                        h_new[:, t * 128:(t + 1) * 128],
                        idn_sb[0:64, 0:64])
                    inst.then_inc(s_pe3, 1)
                if TRUNC == 'F':
                    return

        # ================= SCALAR =================
        @block.scalar
        def _(eng):
            # init: drain h_ownT transposes ([64 feat, 128 nodes] psum slots)
            for t in range(NBLK):
                eng.wait_ge(s_pe3, pe3_init(t))
                inst = eng.activation(
                    h_bufs[0][:, t * 128:(t + 1) * 128],
                    p_t2[t % 2][0:D, 0:128],
                    ACTF.Copy)
                inst.then_inc(s_s3, 1)

            for l in range(L):
                # --- A copies: psum -> x_aggrT ---
                for b in range(NBLK):
                    pa = part_a(l, b)
                    eng.wait_ge(s_peb, pa + 1)
                    pslot = p_agg2[pa % 2][0:D, 0:128]
                    inst = eng.activation(
                        x_aggrT[:, b * 128:(b + 1) * 128], pslot, ACTF.Copy)
                    inst.then_inc(s_cp, 1)
                # --- B copies (psum -> tmpB) + phase C scalar interleave ---
                base2, base_s2 = l * c_.NKC * 5, l * c_.NKC * 2

                def c_scalar(k):
                    o0, w = c_.kchunks[k]
                    we = w if o0 + w <= NPC else max(NPC - o0, 0)
                    eng.wait_ge(s_pe2, base2 + 5 * k + 1)
                    eng.activation(h_mlpT[:, o0:o0 + we],
                                   p_c[k % 2][0:D, 0:we], ACTF.Copy,
                                   accum_out=sum_cols[0][:, k:k + 1])
                    eng.activation(sq_scr[:, 0:we], p_c[k % 2][0:D, 0:we],
                                   ACTF.Square,
                                   accum_out=sq_cols[0][:, k:k + 1])
                    eng.wait_ge(s_pe2, base2 + 5 * k + 2)
                    pb1_0 = pb1_sb[0:BOT, l:l + 1]
                    inst = eng.activation(hid0[0:BOT, 0:w], p_h[0:BOT, 0:w],
                                          ACTF.Relu, bias=pb1_0)
                    inst.then_inc(s_s2, 1)
                    eng.wait_ge(s_pe2, base2 + 5 * k + 3)
                    eng.activation(ad0T[:, o0:o0 + we], p_a[0:D, 0:we],
                                   ACTF.Copy,
                                   accum_out=sum_cols[1][:, k:k + 1])
                    eng.activation(sq_scr[:, 0:we], p_a[0:D, 0:we],
                                   ACTF.Square,
                                   accum_out=sq_cols[1][:, k:k + 1])
                    eng.drain()   # a0 square must finish before PE reuses p_a
                    eng.wait_ge(s_pe2, base2 + 5 * k + 4)
                    pb1_1 = pb1_sb[0:BOT, L + l:L + l + 1]
                    inst = eng.activation(hid1[0:BOT, 0:w], p_h[0:BOT, 0:w],
                                          ACTF.Relu, bias=pb1_1)
                    inst.then_inc(s_s2, 1)
                    eng.wait_ge(s_pe2, base2 + 5 * k + 5)
                    eng.activation(ad1T[:, o0:o0 + we], p_a[0:D, 0:we],
                                   ACTF.Copy,
                                   accum_out=sum_cols[2][:, k:k + 1])
                    eng.activation(sq_scr[:, 0:we], p_a[0:D, 0:we],
                                   ACTF.Square,
                                   accum_out=sq_cols[2][:, k:k + 1])
                    eng.drain().then_inc(s_cp2, 1)

                nextk = 0
                for b in range(NBLK):
                    pa = part_b(l, b)
                    eng.wait_ge(s_peb, pa + 1)
                    if b >= 2:
                        eng.wait_ge(s_badd, l * NBLK + b - 1)
                    pslot = p_agg2[pa % 2][0:D, 0:128]
                    inst = eng.activation(
                        tmpB[:, b % 2, :], pslot, ACTF.Copy)
                    inst.then_inc(s_cpb, 1)
                    if TRUNC == 'AGG' or (TRUNC == 'G' and l == 1):
                        continue
                    if nextk < c_.NKC and b == min(4 * (nextk + 1) + 1,
                                                   NBLK - 1):
                        c_scalar(nextk)
                        nextk += 1
                if TRUNC == 'AGG' or (TRUNC == 'G' and l == 1):
                    return
                while nextk < c_.NKC:
                    c_scalar(nextk)
                    nextk += 1
                if TRUNC == 'C':
                    return
                # --- phase D: sqrt(var + eps) ---
                eng.wait_ge(s_var, l + 1)
                eng.activation(sd3[:, 0:3], var3[:, 0:3], ACTF.Sqrt,
                               bias=eps_sb[:, 0:1]).then_inc(s_sqr, 1)
                # --- phase E: relu (halves) ---
                if l < L - 1:
                    h_new = h_bufs[(l + 1) % 2]
                    eng.wait_ge(s_acc, 2 * l + 1)
                    eng.activation(h_new[:, 0:c_.H1], x_aggrT[:, 0:c_.H1],
                                   ACTF.Relu, bias=tv[:, 0:1])
                    eng.drain().then_inc(s_hn, 1)
                    eng.wait_ge(s_acc, 2 * l + 2)
                    eng.activation(h_new[:, c_.H1:NPC], x_aggrT[:, c_.H1:NPC],
                                   ACTF.Relu, bias=tv[:, 0:1])
                    if NPAD > NPC:
                        eng.activation(h_new[:, NPC:NPAD],
                                       x_aggrT[:, NPC:NPAD],
                                       ACTF.Copy, scale=0.0)
                    eng.drain().then_inc(s_hn, 1)
                else:
                    # vector wrote h_new directly (tensor_scalar_add)
                    eng.wait_ge(s_acc, 2 * l + 1)
                    eng.drain().then_inc(s_hn, 1)
                    eng.wait_ge(s_acc, 2 * l + 2)
                    eng.drain().then_inc(s_hn, 1)
                if TRUNC == 'E':
                    return
                # --- phase F: psum -> stage ---
                for t in range(NBLK):
                    gt = l * NBLK + t
                    if gt >= 2:
                        lp, tp = divmod(gt - 2, NBLK)
                        si, cnt = tile_sem(lp, tp)
                        eng.wait_ge(s_dt[si], cnt)
                    eng.wait_ge(s_pe3, pe3_val(l, t))
                    if l < L - 1:
                        inst = eng.activation(
                            stageh[:, t % 2, 0:D],
                            p_t2[t % 2][0:128, 0:64],
                            ACTF.Copy)
                    else:
                        inst = eng.activation(
                            stage[:, t % 2, :],
                            p_t2[t % 2][0:128, 0:64],
                            ACTF.Copy)
                    inst.then_inc(s_s3, 1)
                if TRUNC == 'F':
                    return

    ctx.close()
    nc.finalize()
    return nc


def _host_pack(cfg, W, pw1, pw2, pb1, bn_g, bn_b, pbn_g, pbn_b, gating):
    L, D, BOT = cfg.L, cfg.D, cfg.BOT
    wt = np.ascontiguousarray(W.transpose(1, 0, 2).reshape(D, L * D))
    pw1t = np.ascontiguousarray(
        pw1.transpose(2, 0, 1, 3).reshape(D, 2 * L * BOT))
    pw2t = np.zeros((16, 2 * L * D), np.float32)
    pw2t[0:BOT] = pw2.transpose(2, 0, 1, 3).reshape(BOT, 2 * L * D)
    pb1t = np.zeros((16, 2 * L), np.float32)
    pb1t[0:BOT] = pb1.transpose(2, 0, 1).reshape(BOT, 2 * L)
    gb = np.zeros((D, 6 * L), np.float32)
    for l in range(L):
        g0 = gating[0, l, 0]
        g1 = gating[1, l, 0]
        gb[:, 6 * l + 0] = bn_g[l]
        gb[:, 6 * l + 1] = pbn_g[0, l] * g0
        gb[:, 6 * l + 2] = pbn_g[1, l] * g1
        gb[:, 6 * l + 3] = bn_b[l]
        gb[:, 6 * l + 4] = pbn_b[0, l] * g0
        gb[:, 6 * l + 5] = pbn_b[1, l] * g1
    import ml_dtypes
    bf16 = ml_dtypes.bfloat16
    iota = np.tile(np.arange(128, dtype=np.float32), (128, 1)).astype(bf16)
    idn = np.eye(128, dtype=np.float32)
    idnh = np.eye(128, dtype=np.float32).astype(bf16)
    return dict(wt=wt, pw1t=pw1t, pw2t=pw2t, pb1t=pb1t, gbvec=gb,
                iota=np.ascontiguousarray(iota), idn=idn, idnh=idnh)


def make_in_maps(cfg, inputs):
    c_ = cfg
    x = np.asarray(inputs["x"], np.float32)
    edge_index = np.asarray(inputs["edge_index"])
    gidx_w, drel = _prep_tokens(c_, edge_index)
    packs = _host_pack(c_, *[np.asarray(inputs[k], np.float32) for k in
                             ("W", "pw1", "pw2", "pb1", "bn_g", "bn_b",
                              "pbn_g", "pbn_b", "gating")])
    import ml_dtypes
    bf16 = ml_dtypes.bfloat16
    xpadA = np.zeros((c_.GA, 128), bf16)
    xpadB = np.zeros((c_.GB, 128), bf16)
    for o in range(c_.NC):
        xpadA[o * c_.H1:(o + 1) * c_.H1, 0:c_.D] = \
            x[o * c_.NPC:o * c_.NPC + c_.H1].astype(bf16)
        nreal = c_.NPC - c_.H1
        xpadB[o * c_.H2:o * c_.H2 + nreal, 0:c_.D] = \
            x[o * c_.NPC + c_.H1:(o + 1) * c_.NPC].astype(bf16)
    in_maps = []
    for i in range(c_.NC):
        m = dict(packs)
        m["x_fullA"] = xpadA
        m["x_fullB"] = xpadB
        m["x_own"] = np.ascontiguousarray(
            x[i * c_.NPC:(i + 1) * c_.NPC])
        m["gidx"] = np.ascontiguousarray(gidx_w[i])
        m["drel"] = np.ascontiguousarray(drel[i].astype(bf16))
        in_maps.append(m)
    return in_maps


def _make_cfg(inputs, N=50000, E=800000, D=64, L=5, BOT=15, NC=8, NQ=4):
    edge_index = np.asarray(inputs["edge_index"])
    capA, capB = _caps_from_edges(dict(N=N, NC=NC), edge_index)
    return Cfg(N, E, D, L, BOT, NC, capA, capB, NQ=NQ)


_GRAPH_CACHE = {}


def kernel(**inputs) -> np.ndarray:
    cfg = _make_cfg(inputs)
    key = (tuple(cfg.capA), tuple(cfg.capB))
    if key not in _GRAPH_CACHE:
        _GRAPH_CACHE[key] = build_graph(cfg)
    nc = _GRAPH_CACHE[key]
    in_maps = make_in_maps(cfg, inputs)
    res = run_bass_kernel_spmd(nc, in_maps, core_ids=list(range(cfg.NC)))
    outs = [res.results[i]["out"] for i in range(cfg.NC)]
    return np.concatenate(outs, axis=0)


# revision 70
# speedup vs baseline: 2.3700x; 1.0166x over previous
"""AdapterGNN on 8 TRN2 NeuronCores.

Strategy (dst-node sharding, halved source tables):
  - Nodes sharded: core c owns nodes [c*6250, (c+1)*6250). All edges whose dst
    belongs to core c are processed by core c (~100k edges/core).
  - h_full (node-major, padded 6272 rows/core) is laid out as two tables:
    H1 = all cores' local positions [0, 3200) and H2 = positions [3200, 6272).
    Each layer the halves are published with two separate AllGathers so the
    next layer's H1-sourced gathers can start as soon as AllGather-1 lands.
  - Per dst block (128 nodes), tokens are split into an A region (src in H1)
    and a B region (src in H2); per-(block, region) gather calls stream over 4
    SWDGE queues (block mod 4), and a one-hot matmul on TensorE reduces token
    tiles into dst columns (segment-sum, f32 PSUM).  The A round lands via
    scalar copies into x_aggrT; the B round accumulates via vector adds.
  - conv/adapter compute (phase C) runs per 512-column chunk as soon as the
    chunk's aggregation columns are final, overlapped with the B round.
  - BatchNorm statistics are per-core partial sums + a [64,6] AllReduce;
    Linear biases feeding straight into BatchNorm (b, pb2) cancel and are
    skipped; gating is folded into the adapter BN affine on the host.
"""

import math
import os
import sys

import numpy as np

sys.path.insert(0, "/opt/trn_rl_repo")

from concourse import bass, mybir  # noqa: E402
from concourse.bacc import Bacc  # noqa: E402
from concourse.bass_utils import run_bass_kernel_spmd  # noqa: E402

F32 = mybir.dt.float32
BF16 = mybir.dt.bfloat16
I16 = mybir.dt.int16
AX = mybir.AxisListType.X
ALU = mybir.AluOpType
ACTF = mybir.ActivationFunctionType

EPS = 1e-5
TRUNC = None  # debug hang bisection: 'AGG' | 'C' | 'E' | 'F' | None


def _r16(v):
    return max(16, ((int(v) + 15) // 16) * 16)


class Cfg:
    def __init__(self, N, E, D, L, BOT, NC, capA, capB, NQ=4):
        self.N, self.E, self.D, self.L, self.BOT = N, E, D, L, BOT
        self.NC, self.NQ = NC, NQ
        self.NPC = N // NC                      # real nodes per core
        self.NBLK = (self.NPC + 127) // 128     # dst blocks per core
        self.NPAD = self.NBLK * 128             # padded nodes per core
        # smallest half-1 the int16 gather index allows (H2T <= 31 so the
        # B-table stays under 32768 rows): shortens the per-layer critical
        # chain affine -> E1 -> relu1 -> F1 -> AllGather-1
        self.H1T = self.NBLK - 31
        self.H2T = self.NBLK - self.H1T
        self.H1 = self.H1T * 128                # local positions [0, H1)
        self.H2 = self.H2T * 128
        self.GA = NC * self.H1                  # A-table rows
        self.GB = NC * self.H2                  # B-table rows
        self.GN = self.GA + self.GB
        assert self.GA < 32768 and self.GB < 32768
        self.capA, self.capB = capA, capB       # per-block r16 token caps
        self.tA = [(c + 127) // 128 for c in capA]
        self.tB = [(c + 127) // 128 for c in capB]
        self.offA = np.concatenate([[0], np.cumsum(capA)]).astype(int)
        self.offB = np.concatenate([[0], np.cumsum(capB)]).astype(int)
        self.ASZ = int(self.offA[-1])           # B region token offset
        self.NTOK = self.ASZ + int(self.offB[-1])
        self.toffA = np.concatenate([[0], np.cumsum(self.tA)]).astype(int)
        self.toffB = np.concatenate([[0], np.cumsum(self.tB)]).astype(int)
        self.TTA = int(self.toffA[-1])
        self.NTILES = self.TTA + int(self.toffB[-1])
        self.TMAX = max(max(self.tA), max(self.tB))

        # gather pieces: ucode handles at most 1024 idxs per dma_gather
        def _splits(cap):
            out, off = [], 0
            while off < cap:
                n = min(1024, cap - off)
                out.append((off, n))
                off += n
            return out

        self.spA = [_splits(c) for c in capA]
        self.spB = [_splits(c) for c in capB]
        # per-queue cumulative piece counts (A-blocks == q mod NQ, then B)
        self.cumA = [0] * self.NBLK
        self.cumB = [0] * self.NBLK
        self.PA_q = [0] * NQ
        self.PB_q = [0] * NQ
        for b in range(self.NBLK):
            q = b % NQ
            self.PA_q[q] += len(self.spA[b])
            self.cumA[b] = self.PA_q[q]
        for b in range(self.NBLK):
            q = b % NQ
            self.PB_q[q] += len(self.spB[b])
            self.cumB[b] = self.PB_q[q]
        # phase-C column chunks over [0, NPAD)
        self.kchunks = []
        off = 0
        while off < self.NPAD:
            w = min(512, self.NPAD - off)
            self.kchunks.append((off, w))
            off += w
        self.NKC = len(self.kchunks)


def _src_tables(cfg, src):
    """Map global src node -> (isB, table row index)."""
    o = src // cfg.NPC
    p = src - o * cfg.NPC
    isB = (p >= cfg.H1).astype(np.int64)
    row = np.where(isB == 0, o * cfg.H1 + p, o * cfg.H2 + (p - cfg.H1))
    return isB, row


def _caps_from_edges(cfg_dims, edge_index):
    """Per-(block, region) r16 token caps (max over cores)."""
    N, NC = cfg_dims["N"], cfg_dims["NC"]
    NPC = N // NC
    NBLK = (NPC + 127) // 128
    H1 = (NBLK - 31) * 128          # must match Cfg.H1T
    src = edge_index[0].astype(np.int64)
    dst = edge_index[1].astype(np.int64)
    owner = dst // NPC
    blk = (dst - owner * NPC) >> 7
    isB = ((src % NPC) >= H1).astype(np.int64)
    key = (owner * NBLK + blk) * 2 + isB
    counts = np.bincount(key, minlength=NC * NBLK * 2).reshape(NC, NBLK, 2)
    mx = counts.max(axis=0)
    capA = [_r16(v) for v in mx[:, 0]]
    capB = [_r16(v) for v in mx[:, 1]]
    return capA, capB


def _prep_tokens(cfg, edge_index):
    """Per-core token streams: gather idx (wrapped int16) + dst_rel (f32)."""
    c_ = cfg
    src = edge_index[0].astype(np.int64)
    dst = edge_index[1].astype(np.int64)
    owner = dst // c_.NPC
    dloc = dst - owner * c_.NPC
    blk = dloc >> 7
    rel = (dloc & 127).astype(np.float32)
    isB, row = _src_tables(c_, src)

    key = (owner * c_.NBLK + blk) * 2 + isB
    order = np.argsort(key, kind="stable")
    skey = key[order]
    counts = np.bincount(key, minlength=c_.NC * c_.NBLK * 2)
    starts = np.concatenate([[0], np.cumsum(counts)[:-1]])
    rank = np.arange(c_.E) - starts[skey]

    core = skey // (2 * c_.NBLK)
    remk = skey % (2 * c_.NBLK)
    b2 = remk // 2
    piece = remk % 2
    pos = np.where(piece == 0, c_.offA[b2] + rank,
                   c_.ASZ + c_.offB[b2] + rank)
    # tile-granular position for drel (tiles are 128-padded per call)
    tpos = np.where(piece == 0,
                    (c_.toffA[b2] + rank // 128) * 128 + rank % 128,
                    (c_.TTA + c_.toffB[b2] + rank // 128) * 128 + rank % 128)

    gs = row[order]
    assert gs.max() < 32768 and gs.min() >= 0

    gidx_val = np.zeros((c_.NC, c_.NTOK), np.int16)
    rel_flat = np.full((c_.NC, c_.NTILES * 128), -1.0, np.float32)
    gidx_val[core, pos] = gs.astype(np.int16)
    rel_flat[core, tpos] = rel[order]

    # wrapped layout [16, NTOK/16], replicated to all 8 groups of 16
    # partitions (each Q7 descriptor-gen core reads its own group)
    wrap = gidx_val.reshape(c_.NC, c_.NTOK // 16, 16).transpose(0, 2, 1)
    gidx_w = np.tile(wrap, (1, 8, 1)).astype(np.int16)
    drel = rel_flat.reshape(c_.NC, c_.NTILES, 128).transpose(0, 2, 1).copy()
    return gidx_w, drel


def build_graph(cfg):
    c_ = cfg
    D, BOT, L, NQ = c_.D, c_.BOT, c_.L, c_.NQ
    NPC, NPAD, NBLK, GN, GA = c_.NPC, c_.NPAD, c_.NBLK, c_.GN, c_.GA

    nc = Bacc(target_bir_lowering=False, debug=False, num_swdge_queues=NQ,
              dynamic_dma_scratch_size=32768)

    # ---------- dram parameters ----------
    x_fullA = nc.declare_dram_parameter("x_fullA", [GA, 128], BF16,
                                        isOutput=False)
    x_fullB = nc.declare_dram_parameter("x_fullB", [GN - GA, 128], BF16,
                                        isOutput=False)
    x_own = nc.declare_dram_parameter("x_own", [NPC, D], F32,
                                      isOutput=False)
    gidx_p = nc.declare_dram_parameter("gidx", [128, c_.NTOK // 16], I16,
                                       isOutput=False)
    drel_p = nc.declare_dram_parameter("drel", [128, c_.NTILES], BF16,
                                       isOutput=False)
    iota_p = nc.declare_dram_parameter("iota", [128, 128], BF16,
                                       isOutput=False)
    idn_p = nc.declare_dram_parameter("idn", [128, 128], F32, isOutput=False)
    idnh_p = nc.declare_dram_parameter("idnh", [128, 128], BF16,
                                       isOutput=False)
    wt_p = nc.declare_dram_parameter("wt", [D, L * D], F32, isOutput=False)
    pw1_p = nc.declare_dram_parameter("pw1t", [D, 2 * L * BOT], F32,
                                      isOutput=False)
    pw2_p = nc.declare_dram_parameter("pw2t", [16, 2 * L * D], F32,
                                      isOutput=False)
    pb1_p = nc.declare_dram_parameter("pb1t", [16, 2 * L], F32, isOutput=False)
    gb_p = nc.declare_dram_parameter("gbvec", [D, 6 * L], F32, isOutput=False)
    out_p = nc.declare_dram_parameter("out", [NPC, D], F32, isOutput=True)

    # ---------- internal dram ----------
    h_sh1 = nc.dram_tensor("h_sh1", [c_.H1, 128], BF16)
    h_sh2 = nc.dram_tensor("h_sh2", [c_.H2, 128], BF16)
    h_fullA = nc.dram_tensor("h_fullA", [GA, 128], BF16, addr_space="Shared")
    h_fullB = nc.dram_tensor("h_fullB", [GN - GA, 128], BF16,
                             addr_space="Shared")
    stat_in = nc.dram_tensor("stat_in", [D, 6], F32)
    stat_out = nc.dram_tensor("stat_out", [D, 6], F32, addr_space="Shared")

    rg = [list(range(c_.NC))]

    import contextlib
    ctx = contextlib.ExitStack()

    def sb(name, shape, dt=F32):
        return ctx.enter_context(nc.sbuf_tensor(name, shape, dt))

    def ps(name, shape):
        return ctx.enter_context(nc.psum_tensor(name, shape, F32))

    def sem(name):
        return ctx.enter_context(nc.semaphore(name))

    # ---------- sbuf ----------
    MCOL = max(c_.TMAX, (NBLK + NQ - 1) // NQ)    # msgs columns per queue
    h_bufs = [sb("h0", [D, NPAD]), sb("h1", [D, NPAD])]
    x_aggrT = sb("x_aggrT", [D, NPAD])          # also phase-E accumulator
    h_mlpT = sb("h_mlpT", [D, NPAD])
    ad0T = sb("ad0T", [D, NPAD])
    ad1T = sb("ad1T", [D, NPAD])
    msgs = [sb(f"msgs{i}", [128, MCOL, 128], BF16) for i in range(NQ)]
    oh_buf = sb("oh_buf", [128, 2, c_.TMAX * 128], BF16)  # one-hot, 2 slots
    drel_sb = sb("drel_sb", [128, c_.NTILES], BF16)
    gidx_sb = sb("gidx_sb", [128, c_.NTOK // 16], I16)
    iota_sb = sb("iota_sb", [128, 128], BF16)
    idn_sb = sb("idn_sb", [128, 128])
    idnh_sb = sb("idnh_sb", [128, 128], BF16)
    w_sb = sb("w_sb", [D, L * D])
    pw1_sb = sb("pw1_sb", [D, 2 * L * BOT])
    pw2_sb = sb("pw2_sb", [16, 2 * L * D])
    pb1_sb = sb("pb1_sb", [16, 2 * L])
    gb_sb = sb("gb_sb", [D, 6 * L])
    hid0 = sb("hid0", [16, 512])
    hid1 = sb("hid1", [16, 512])
    sq_scr = sb("sq_scr", [D, 512])             # square-activation dump
    tmpB = sb("tmpB", [D, 2, 128])              # B-part psum drain, 2 slots
    sum_cols = [sb(f"sum_cols{i}", [D, c_.NKC]) for i in range(3)]
    sq_cols = [sb(f"sq_cols{i}", [D, c_.NKC]) for i in range(3)]
    stats_sb = sb("stats_sb", [D, 8])
    stats_g = sb("stats_g", [D, 8])
    means = sb("means", [D, 4])
    msq = sb("msq", [D, 4])
    var3 = sb("var3", [D, 4])
    sd3 = sb("sd3", [D, 4])
    rs3 = sb("rs3", [D, 4])
    svec = sb("svec", [D, 4])
    mS = sb("mS", [D, 4])
    t3 = sb("t3", [D, 4])
    tv = sb("tv", [D, 1])
    eps_sb = sb("eps_sb", [D, 1])
    stage = sb("stage", [128, 2, D])          # f32 drain (last layer)
    stageh = sb("stageh", [128, 2, 128], BF16)  # bf16 drain (publish)

    # ---------- psum ----------
    p_agg2 = [ps("p_agg0", [128, 128]), ps("p_agg1", [128, 128])]
    p_c = [ps("p_c0", [128, 512]), ps("p_c1", [128, 512])]
    p_h = ps("p_h", [128, 512])
    p_a = ps("p_a", [128, 512])
    p_t2 = [ps("p_t0", [128, 128]), ps("p_t1", [128, 128])]

    # ---------- semaphores ----------
    s_g = [sem(f"g{i}") for i in range(NQ)]  # +16 per gather call (by queue)
    s_prep = [sem(f"pr{i}") for i in range(NQ)]  # +1 per desc-gen prep
    s_gx = sem("gx")          # +16 when gidx_sb is loaded
    s_oh = sem("oh")          # +1 per one-hot job (vector)
    s_peb = sem("peb")        # +1 per agg part (tensor): 2*NBLK per layer
    s_cp = sem("cp")          # +1 per A copy (scalar)
    s_cpb = sem("cpb")        # +1 per B psum->tmpB copy (scalar)
    s_badd = sem("badd")      # +1 per B add (vector)
    s_pe2 = sem("pe2")        # +5 per phase-C chunk (tensor)
    s_s2 = sem("s2")          # +2 per phase-C chunk (scalar relu)
    s_cp2 = sem("cp2")        # +1 per phase-C chunk copied (scalar)
    s_var = sem("var")        # +1 per layer (vector: vars ready)
    s_sqr = sem("sqr")        # +1 per layer (scalar: sqrt done)
    s_v2 = sem("v2")          # +1 per layer (vector: affines ready)
    s_acc = sem("acc")        # +2 per layer (vector: E halves done)
    s_hn = sem("hn")          # +2 per layer (h_new halves ready)
    s_pe3 = sem("pe3")        # +1 per transpose (tensor)
    s_s3 = sem("s3")          # +1 per stage copy (scalar)
    s_dma = sem("dma")        # +16 per sync DMA (init + stats)
    s_dt = [sem("dt0"), sem("dt1")]   # +16 per tile DMA, parity by tile
    s_cc = sem("cc")          # +1 per collective
    s_sq = sem("sq")          # +1 per layer (vector stats ready)
    s_vz = sem("vz")          # +1 init memset

    # ---------- schedule bookkeeping ----------
    NFULL = NPC // 128
    REM = NPC - NFULL * 128
    assert NQ * MCOL >= NBLK, "staging must fit in msgs buffers"

    def stg(t):
        return (t // MCOL, t % MCOL)

    STG_RANGES = []
    t0 = 0
    while t0 < NFULL:
        n = min(MCOL - (t0 % MCOL), NFULL - t0)
        STG_RANGES.append((t0, n))
        t0 += n

    # gidx DMA increments s_gx instead of s_dma (one update per DMA);
    # 9 params (incl. idnh) + staging
    N_INIT_DMA = 9 + len(STG_RANGES) + (1 if REM else 0)
    dma_init = 16 * N_INIT_DMA

    def dma_after_statin(layer):
        return 16 * (N_INIT_DMA + 2 * layer + 1)

    def dma_after_statout(layer):
        return dma_after_statin(layer) + 16

    def tile_sem(layer, t):
        gt = layer * NBLK + t
        return gt % 2, 16 * (gt // 2 + 1)

    # collective ids: per layer AR, then (if l < L-1) AG1, AG2
    cc_n = 0
    cc_ar_, cc_ag1_, cc_ag2_ = {}, {}, {}
    for l in range(L):
        cc_n += 1
        cc_ar_[l] = cc_n
        if l < L - 1:
            cc_n += 1
            cc_ag1_[l] = cc_n
            cc_n += 1
            cc_ag2_[l] = cc_n

    # per-queue cumulative gather-piece count after (layer, region, block)
    def gcount(l, region, b):
        q = b % NQ
        base = l * (c_.PA_q[q] + c_.PB_q[q])
        if region == 0:
            return 16 * (base + c_.cumA[b])
        return 16 * (base + c_.PA_q[q] + c_.cumB[b])

    # part index helpers (s_peb order: per layer A blocks, then B blocks)
    def part_a(l, b):
        return l * 2 * NBLK + b

    def part_b(l, b):
        return l * 2 * NBLK + NBLK + b

    # largest A/B-block on queue q
    bmax_q = [max(range(q, NBLK, NQ)) for q in range(NQ)]

    def pe3_val(layer, t):
        return NBLK + layer * NBLK + t + 1

    def pe3_init(t):
        return t + 1

    H1T, H2T = c_.H1T, c_.H2T

    with nc.Block() as block:

        # ================= SYNC: plain DMAs =================
        @block.sync
        def _(eng):
            dmac = [0]

            def dma(dst, src_ap):
                eng.dma_start(out=dst, in_=src_ap).then_inc(s_dma, 16)
                dmac[0] += 16

            eng.dma_start(out=gidx_sb[:, :], in_=gidx_p[:, :])\
                .then_inc(s_gx, 16)
            dma(drel_sb[:, :], drel_p[:, :])
            dma(iota_sb[:, :], iota_p[:, :])
            dma(idn_sb[:, :], idn_p[:, :])
            dma(idnh_sb[:, :], idnh_p[:, :])
            dma(w_sb[:, :], wt_p[:, :])
            dma(pw1_sb[:, :], pw1_p[:, :])
            dma(pw2_sb[:, :], pw2_p[:, :])
            dma(pb1_sb[:, :], pb1_p[:, :])
            dma(gb_sb[:, :], gb_p[:, :])
            # x_own -> staging (node-major tiles, spans the msgs buffers)
            eng.wait_ge(s_vz, 1)   # staging pad rows zeroed
            for t0, n in STG_RANGES:
                bi, bc = stg(t0)
                dma(msgs[bi][:, bc:bc + n, :].bitcast(F32),
                    x_own[t0 * 128:(t0 + n) * 128, :]
                    .rearrange("(t p) d -> p t d", p=128))
            if REM:
                bi, bc = stg(NFULL)
                dma(msgs[bi][0:REM, bc, :].bitcast(F32),
                    x_own[NFULL * 128:NPC, :])
            assert dmac[0] == dma_init

            if TRUNC in ('AGG', 'C'):
                if TRUNC == 'AGG':
                    eng.wait_ge(s_badd, NBLK)
                    src = x_aggrT
                else:
                    eng.wait_ge(s_cp2, c_.NKC)
                    src = h_mlpT
                eng.dma_start(out=out_p[:, :],
                              in_=src[:, 0:NPC]).then_inc(s_dt[0], 16)
                return

            for l in range(L):
                # stats out
                eng.wait_ge(s_sq, l + 1)
                dma(stat_in[:, 0:6], stats_sb[:, 0:6])
                assert dmac[0] == dma_after_statin(l)
                # stats back
                eng.wait_ge(s_cc, cc_ar_[l])
                dma(stats_g[:, 0:6], stat_out[:, 0:6])
                if TRUNC == 'E':
                    eng.wait_ge(s_hn, 2)
                    eng.dma_start(out=out_p[:, :],
                                  in_=h_bufs[1][:, 0:NPC])\
                        .then_inc(s_dt[0], 16)
                    return
                # h_new tiles out
                for t in range(NBLK):
                    if l > 0:
                        if t == 0:
                            eng.wait_ge(s_cc, cc_ag1_[l - 1])  # h_sh1 free
                        elif t == H1T:
                            eng.wait_ge(s_cc, cc_ag2_[l - 1])  # h_sh2 free
                    eng.wait_ge(s_s3, pe3_val(l, t))
                    slot = stage[:, t % 2, :]
                    sidx, _ = tile_sem(l, t)
                    if l < L - 1:
                        if t < H1T:
                            tgt = h_sh1[t * 128:(t + 1) * 128, :]
                        else:
                            t2 = t - H1T
                            tgt = h_sh2[t2 * 128:(t2 + 1) * 128, :]
                        src_ap = stageh[:, t % 2, :]
                    elif t < NPC // 128:
                        tgt, src_ap = out_p[t * 128:(t + 1) * 128, :], slot
                    else:
                        rem = NPC - (NPC // 128) * 128
                        tgt = out_p[t * 128:t * 128 + rem, :]
                        src_ap = stage[0:rem, t % 2, :]
                    eng.dma_start(out=tgt, in_=src_ap).then_inc(s_dt[sidx], 16)
                if TRUNC == 'G' and l == 0:
                    eng.wait_ge(s_badd, 2 * NBLK)
                    eng.dma_start(out=out_p[:, :],
                                  in_=x_aggrT[:, 0:NPC]).then_inc(s_dt[0], 16)
                    return
                if TRUNC == 'F':
                    eng.wait_ge(s_cc, cc_ag2_[0])
                    eng.dma_start(out=out_p[:, :],
                                  in_=h_bufs[1][:, 0:NPC])\
                        .then_inc(s_dt[0], 16)
                    return

        # ================= GPSIMD: gathers + collectives =================
        # Desc-gen runs ahead via prepare_only; triggers (which carry the
        # data/buffer waits) fire the DMAs.  The SWDGE ring buffers ~9
        # pieces/queue so the Q7 cores stay busy through the layer tail.
        @block.gpsimd
        def _(eng):
            # piece list: (layer, region, block, off, cnt, first)
            pieces = []
            lstart = []
            for l in range(L):
                lstart.append(len(pieces))
                for b in range(NBLK):
                    for i, (off, cnt) in enumerate(c_.spA[b]):
                        pieces.append((l, 0, b, off, cnt, i == 0))
                for b in range(NBLK):
                    for i, (off, cnt) in enumerate(c_.spB[b]):
                        pieces.append((l, 1, b, off, cnt, i == 0))
            total = len(pieces)
            PPL = total // L                     # pieces per layer
            # per-queue prep counts through piece i
            prep_cnt = [0] * NQ
            prep_thru = [0] * total
            for i, (l, r, b, off, cnt, first) in enumerate(pieces):
                prep_cnt[b % NQ] += 1
                prep_thru[i] = prep_cnt[b % NQ]

            pe_ptr = [0]
            ring_occ = [0] * NQ   # descs prepped but not yet triggered

            def emit_prep(i):
                l, r, b, off, cnt, first = pieces[i]
                q = b % NQ
                if i == 0:
                    eng.wait_ge(s_gx, 16)
                if r == 0:
                    src = x_fullA[:, :] if l == 0 else h_fullA[:, :]
                    t0 = int(c_.offA[b]) + off
                else:
                    src = x_fullB[:, :] if l == 0 else h_fullB[:, :]
                    t0 = c_.ASZ + int(c_.offB[b]) + off
                eng.dma_gather(
                    msgs[q][:, off // 128:(off + cnt + 127) // 128, :], src,
                    gidx_sb[:, t0 // 16:(t0 + cnt) // 16],
                    cnt, cnt, 128, queue_num=q,
                    prepare_only=True, sem=s_g[q],
                ).then_inc(s_prep[q], 1)
                ring_occ[q] += cnt // 16 + 1
                assert ring_occ[q] <= 1400, (q, ring_occ[q])

            def fill(n):
                k = min(n, total - pe_ptr[0])
                for _ in range(k):
                    emit_prep(pe_ptr[0])
                    pe_ptr[0] += 1

            def emit_trigger(i):
                l, r, b, off, cnt, first = pieces[i]
                q = b % NQ
                if first:
                    if r == 0:
                        if l > 0 and b == 0:
                            eng.wait_ge(s_cc, cc_ag1_[l - 1])
                        if b >= NQ:
                            eng.wait_ge(s_peb, part_a(l, b - NQ) + 1)
                        elif l == 0:
                            eng.wait_ge(s_vz, 2)   # staging consumed + zeroed
                        else:
                            eng.wait_ge(s_peb,
                                        part_b(l - 1, bmax_q[q]) + 1)
                    else:
                        if l > 0 and b == 0:
                            eng.wait_ge(s_cc, cc_ag2_[l - 1])
                        if b >= NQ:
                            eng.wait_ge(s_peb, part_b(l, b - NQ) + 1)
                        else:
                            eng.wait_ge(s_peb, part_a(l, bmax_q[q]) + 1)
                eng.wait_ge(s_prep[q], prep_thru[i])
                eng.trigger_dma(count=1, queue_num=q)
                ring_occ[q] -= cnt // 16 + 1

            def emit_events(l):
                # stats AllReduce for layer l; AllGathers if more layers
                eng.wait_ge(s_dma, dma_after_statin(l))
                eng.collective_compute(
                    "AllReduce", ALU.add, replica_groups=rg,
                    ins=[stat_in[:, 0:6].opt()],
                    outs=[stat_out[:, 0:6].opt()],
                ).then_inc(s_cc, 1)
                fill(16)
                if l < L - 1:
                    for tq in (H1T - 1, H1T - 2):
                        si, cnt = tile_sem(l, tq)
                        eng.wait_ge(s_dt[si], cnt)
                    eng.collective_compute(
                        "AllGather", ALU.bypass, replica_groups=rg,
                        ins=[h_sh1[:, :].opt()],
                        outs=[h_fullA[:, :].opt()],
                    ).then_inc(s_cc, 1)
                    fill(16)
                    for tq in (NBLK - 1, NBLK - 2):
                        si, cnt = tile_sem(l, tq)
                        eng.wait_ge(s_dt[si], cnt)
                    eng.collective_compute(
                        "AllGather", ALU.bypass, replica_groups=rg,
                        ins=[h_sh2[:, :].opt()],
                        outs=[h_fullB[:, :].opt()],
                    ).then_inc(s_cc, 1)
                    fill(16)

            assert TRUNC is None, "TRUNC unsupported with prep/trigger"
            for j in range(total):
                l, i_in_l = divmod(j, PPL)
                if i_in_l == 0 and l > 0:
                    emit_events(l - 1)
                W = min(total, lstart[l] + 20 + i_in_l)
                while pe_ptr[0] < W:
                    emit_prep(pe_ptr[0])
                    pe_ptr[0] += 1
                emit_trigger(j)
            emit_events(L - 1)

        # ================= VECTOR =================
        @block.vector
        def _(eng):
            # init: zero staging pad region for partial x tile
            eng.memset(eps_sb[:, :], EPS)
            eng.memset(stageh[:, :, :], 0.0)
            if REM:
                bi, bc = stg(NFULL)
                eng.memset(msgs[bi][:, bc, :], 0.0)
            eng.drain().then_inc(s_vz, 1)
            eng.wait_ge(s_dma, dma_init)
            # zero token buffers after staging is consumed: partial gather
            # tiles leave stale rows; they must be finite (0 * NaN = NaN)
            eng.wait_ge(s_pe3, NBLK)
            for q in range(NQ):
                eng.memset(msgs[q][:, :, :], 0.0)
            eng.drain().then_inc(s_vz, 1)

            for l in range(L):
                # --- A one-hots ---
                for b in range(NBLK):
                    j = part_a(l, b)
                    if j >= 2:
                        eng.wait_ge(s_peb, j - 1)
                    tcnt = c_.tA[b]
                    d0 = int(c_.toffA[b])
                    o = oh_buf[:, j % 2, 0:tcnt * 128]
                    o = o.rearrange("p (t j) -> p t j", j=128)
                    d_in = drel_sb[:, d0:d0 + tcnt].unsqueeze(-1)\
                        .broadcast_to([128, tcnt, 128])
                    i_in = iota_sb[:, :].unsqueeze(1)\
                        .broadcast_to([128, tcnt, 128])
                    eng.tensor_tensor(
                        out=o, in0=d_in, in1=i_in,
                        op=ALU.is_equal).then_inc(s_oh, 1)
                # --- B one-hots + B adds (lag 2) ---
                def b_add(b):
                    eng.wait_ge(s_cpb, l * NBLK + b + 1)
                    eng.tensor_tensor(
                        out=x_aggrT[:, b * 128:(b + 1) * 128],
                        in0=tmpB[:, b % 2, :],
                        in1=x_aggrT[:, b * 128:(b + 1) * 128],
                        op=ALU.add).then_inc(s_badd, 1)

                for b in range(NBLK):
                    j = part_b(l, b)
                    if j >= 2:
                        eng.wait_ge(s_peb, j - 1)
                    tcnt = c_.tB[b]
                    d0 = c_.TTA + int(c_.toffB[b])
                    o = oh_buf[:, j % 2, 0:tcnt * 128]
                    o = o.rearrange("p (t j) -> p t j", j=128)
                    d_in = drel_sb[:, d0:d0 + tcnt].unsqueeze(-1)\
                        .broadcast_to([128, tcnt, 128])
                    i_in = iota_sb[:, :].unsqueeze(1)\
                        .broadcast_to([128, tcnt, 128])
                    eng.tensor_tensor(
                        out=o, in0=d_in, in1=i_in,
                        op=ALU.is_equal).then_inc(s_oh, 1)
                    if b >= 2:
                        b_add(b - 2)
                b_add(NBLK - 2)
                b_add(NBLK - 1)
                if TRUNC in ('AGG', 'C') or (TRUNC == 'G' and l == 1):
                    return

                # --- stats reduce ---
                eng.wait_ge(s_cp2, (l + 1) * c_.NKC)
                for j in range(3):
                    eng.reduce_sum(out=stats_sb[:, j:j + 1],
                                   in_=sum_cols[j][:, :], axis=AX)
                    eng.reduce_sum(out=stats_sb[:, 3 + j:4 + j],
                                   in_=sq_cols[j][:, :], axis=AX)
                eng.drain().then_inc(s_sq, 1)
                # --- affine math ---
                eng.wait_ge(s_dma, dma_after_statout(l))
                invn = 1.0 / c_.N
                eng.tensor_scalar_mul(means[:, 0:3], stats_g[:, 0:3], invn)
                eng.tensor_scalar_mul(msq[:, 0:3], stats_g[:, 3:6], invn)
                eng.drain()
                eng.tensor_tensor(out=var3[:, 0:3], in0=means[:, 0:3],
                                  in1=means[:, 0:3], op=ALU.mult)
                eng.drain()
                eng.tensor_sub(var3[:, 0:3], msq[:, 0:3], var3[:, 0:3])
                eng.drain().then_inc(s_var, 1)
                eng.wait_ge(s_sqr, l + 1)
                eng.reciprocal(rs3[:, 0:3], sd3[:, 0:3])
                eng.drain()
                eng.tensor_tensor(out=svec[:, 0:3], in0=rs3[:, 0:3],
                                  in1=gb_sb[:, 6 * l:6 * l + 3], op=ALU.mult)
                eng.drain()
                eng.tensor_tensor(out=mS[:, 0:3], in0=means[:, 0:3],
                                  in1=svec[:, 0:3], op=ALU.mult)
                eng.drain()
                eng.tensor_sub(t3[:, 0:3], gb_sb[:, 6 * l + 3:6 * l + 6],
                               mS[:, 0:3])
                eng.drain()
                eng.reduce_sum(out=tv[:, :], in_=t3[:, 0:3], axis=AX)
                eng.drain().then_inc(s_v2, 1)
                # --- phase E (halves; only real node columns) ---
                for h0, hw in ((0, c_.H1), (c_.H1, NPC - c_.H1)):
                    sl = slice(h0, h0 + hw)
                    eng.tensor_scalar_mul(x_aggrT[:, sl], h_mlpT[:, sl],
                                          svec[:, 0:1])
                    eng.drain()
                    eng.scalar_tensor_tensor(
                        out=x_aggrT[:, sl], in0=ad0T[:, sl],
                        scalar=svec[:, 1:2], in1=x_aggrT[:, sl],
                        op0=ALU.mult, op1=ALU.add)
                    eng.drain()
                    eng.scalar_tensor_tensor(
                        out=x_aggrT[:, sl], in0=ad1T[:, sl],
                        scalar=svec[:, 2:3], in1=x_aggrT[:, sl],
                        op0=ALU.mult, op1=ALU.add)
                    if l == L - 1:
                        eng.drain()
                        eng.tensor_scalar_add(
                            h_bufs[(l + 1) % 2][:, sl],
                            x_aggrT[:, sl], tv[:, 0:1])
                    eng.drain().then_inc(s_acc, 1)
                if TRUNC in ('E', 'F'):
                    return

        # ================= TENSOR =================
        @block.tensor
        def _(eng):
            # init: build h_ownT from x staging
            eng.wait_ge(s_dma, dma_init)
            for t in range(NBLK):
                if t >= 2:
                    eng.wait_ge(s_s3, pe3_init(t) - 2)
                bi, bc = stg(t)
                inst = eng.transpose(
                    p_t2[t % 2][0:D, 0:128],
                    msgs[bi][:, bc, :].bitcast(F32),
                    idn_sb[0:128, 0:128])
                inst.then_inc(s_pe3, 1)

            for l in range(L):
                h_own = h_bufs[l % 2]
                base2 = l * c_.NKC * 5
                base_s2 = l * c_.NKC * 2
                wl = w_sb[:, l * D:(l + 1) * D]

                def c_chunk(k):
                    o0, w = c_.kchunks[k]
                    # aggregation columns for this chunk final (B adds)
                    eng.wait_ge(s_badd,
                                l * NBLK + min(4 * (k + 1), NBLK))
                    if k >= 2:
                        eng.wait_ge(s_cp2, l * c_.NKC + k - 1)
                    pc = p_c[k % 2][0:D, 0:w]
                    eng.matmul(pc, wl, x_aggrT[:, o0:o0 + w],
                               start=True, stop=True).then_inc(s_pe2, 1)
                    # adapter 0 hidden (input h_own)
                    ph = p_h[0:BOT, 0:w]
                    pw1_0 = pw1_sb[:, l * BOT:(l + 1) * BOT]
                    eng.matmul(ph, pw1_0, h_own[:, o0:o0 + w],
                               start=True, stop=True).then_inc(s_pe2, 1)
                    eng.wait_ge(s_s2, base_s2 + 2 * k + 1)
                    pa_ = p_a[0:D, 0:w]
                    pw2_0 = pw2_sb[0:BOT, l * D:(l + 1) * D]
                    eng.matmul(pa_, pw2_0, hid0[0:BOT, 0:w],
                               start=True, stop=True).then_inc(s_pe2, 1)
                    # adapter 1 hidden (input x_aggr)
                    pw1_1 = pw1_sb[:, (L + l) * BOT:(L + l + 1) * BOT]
                    eng.matmul(ph, pw1_1, x_aggrT[:, o0:o0 + w],
                               start=True, stop=True).then_inc(s_pe2, 1)
                    eng.wait_ge(s_s2, base_s2 + 2 * k + 2)
                    pw2_1 = pw2_sb[0:BOT, (L + l) * D:(L + l + 1) * D]
                    eng.matmul(pa_, pw2_1, hid1[0:BOT, 0:w],
                               start=True, stop=True).then_inc(s_pe2, 1)

                if l == 0:
                    eng.wait_ge(s_s3, NBLK)        # init copies done
                # --- A parts ---
                for b in range(NBLK):
                    pa = part_a(l, b)
                    q = b % NQ
                    eng.wait_ge(s_oh, pa + 1)
                    eng.wait_ge(s_g[q], gcount(l, 0, b))
                    # psum slot reuse (consumer of part pa-2 done)
                    if b >= 2:
                        eng.wait_ge(s_cp, l * NBLK + b - 1)
                    elif l > 0:
                        # consumer of part pa-2 = B-add (l-1, NBLK-2+b)
                        eng.wait_ge(s_badd, l * NBLK + b - 1)
                    pslot = p_agg2[pa % 2][0:D, 0:128]
                    oh = oh_buf[:, pa % 2, :]
                    for i in range(c_.tA[b]):
                        inst = eng.matmul(
                            pslot, msgs[q][:, i, 0:D],
                            oh[:, i * 128:(i + 1) * 128],
                            start=(i == 0), stop=(i == c_.tA[b] - 1))
                    inst.then_inc(s_peb, 1)
                # --- B parts + phase C interleave ---
                nextk = 0
                for b in range(NBLK):
                    pa = part_b(l, b)
                    q = b % NQ
                    eng.wait_ge(s_oh, pa + 1)
                    eng.wait_ge(s_g[q], gcount(l, 1, b))
                    if b >= 2:
                        eng.wait_ge(s_badd, l * NBLK + b - 1)
                    else:
                        eng.wait_ge(s_cp, l * NBLK + NBLK - 2 + b + 1)
                    pslot = p_agg2[pa % 2][0:D, 0:128]
                    oh = oh_buf[:, pa % 2, :]
                    for i in range(c_.tB[b]):
                        inst = eng.matmul(
                            pslot, msgs[q][:, i, 0:D],
                            oh[:, i * 128:(i + 1) * 128],
                            start=(i == 0), stop=(i == c_.tB[b] - 1))
                    inst.then_inc(s_peb, 1)
                    if TRUNC == 'AGG' or (TRUNC == 'G' and l == 1):
                        continue
                    if nextk < c_.NKC and b == min(4 * (nextk + 1) + 1,
                                                   NBLK - 1):
                        c_chunk(nextk)
                        nextk += 1
                if TRUNC == 'AGG' or (TRUNC == 'G' and l == 1):
                    return
                while nextk < c_.NKC:
                    c_chunk(nextk)
                    nextk += 1
                if TRUNC in ('C', 'E'):
                    return
                # --- phase F: transposes (halves) ---
                h_new = h_bufs[(l + 1) % 2]
                for t in range(NBLK):
                    eng.wait_ge(s_hn, 2 * l + (1 if t < H1T else 2))
                    eng.wait_ge(s_s3, pe3_val(l, t) - 2)
                    inst = eng.transpose(
                        p_t2[t % 2][0:128, 0:64],
                        h_new[:, t * 128:(t + 1) * 128],
                        idn_sb[0:64, 0:64])
                    inst.then_inc(s_pe3, 1)
                if TRUNC == 'F':
                    return

        # ================= SCALAR =================
        @block.scalar
        def _(eng):
            # init: drain h_ownT transposes ([64 feat, 128 nodes] psum slots)
            for t in range(NBLK):
                eng.wait_ge(s_pe3, pe3_init(t))
                inst = eng.activation(
                    h_bufs[0][:, t * 128:(t + 1) * 128],
                    p_t2[t % 2][0:D, 0:128],
                    ACTF.Copy)
                inst.then_inc(s_s3, 1)

            for l in range(L):
                # --- A copies: psum -> x_aggrT ---
                for b in range(NBLK):
                    pa = part_a(l, b)
                    eng.wait_ge(s_peb, pa + 1)
                    pslot = p_agg2[pa % 2][0:D, 0:128]
                    inst = eng.activation(
                        x_aggrT[:, b * 128:(b + 1) * 128], pslot, ACTF.Copy)
                    inst.then_inc(s_cp, 1)
                # --- B copies (psum -> tmpB) + phase C scalar interleave ---
                base2, base_s2 = l * c_.NKC * 5, l * c_.NKC * 2

                def c_scalar(k):
                    o0, w = c_.kchunks[k]
                    we = w if o0 + w <= NPC else max(NPC - o0, 0)
                    eng.wait_ge(s_pe2, base2 + 5 * k + 1)
                    eng.activation(h_mlpT[:, o0:o0 + we],
                                   p_c[k % 2][0:D, 0:we], ACTF.Copy,
                                   accum_out=sum_cols[0][:, k:k + 1])
                    eng.activation(sq_scr[:, 0:we], p_c[k % 2][0:D, 0:we],
                                   ACTF.Square,
                                   accum_out=sq_cols[0][:, k:k + 1])
                    eng.wait_ge(s_pe2, base2 + 5 * k + 2)
                    pb1_0 = pb1_sb[0:BOT, l:l + 1]
                    inst = eng.activation(hid0[0:BOT, 0:w], p_h[0:BOT, 0:w],
                                          ACTF.Relu, bias=pb1_0)
                    inst.then_inc(s_s2, 1)
                    eng.wait_ge(s_pe2, base2 + 5 * k + 3)
                    eng.activation(ad0T[:, o0:o0 + we], p_a[0:D, 0:we],
                                   ACTF.Copy,
                                   accum_out=sum_cols[1][:, k:k + 1])
                    eng.activation(sq_scr[:, 0:we], p_a[0:D, 0:we],
                                   ACTF.Square,
                                   accum_out=sq_cols[1][:, k:k + 1])
                    eng.drain()   # a0 square must finish before PE reuses p_a
                    eng.wait_ge(s_pe2, base2 + 5 * k + 4)
                    pb1_1 = pb1_sb[0:BOT, L + l:L + l + 1]
                    inst = eng.activation(hid1[0:BOT, 0:w], p_h[0:BOT, 0:w],
                                          ACTF.Relu, bias=pb1_1)
                    inst.then_inc(s_s2, 1)
                    eng.wait_ge(s_pe2, base2 + 5 * k + 5)
                    eng.activation(ad1T[:, o0:o0 + we], p_a[0:D, 0:we],
                                   ACTF.Copy,
                                   accum_out=sum_cols[2][:, k:k + 1])
                    eng.activation(sq_scr[:, 0:we], p_a[0:D, 0:we],
                                   ACTF.Square,
                                   accum_out=sq_cols[2][:, k:k + 1])
                    eng.drain().then_inc(s_cp2, 1)

                nextk = 0
                for b in range(NBLK):
                    pa = part_b(l, b)
                    eng.wait_ge(s_peb, pa + 1)
                    if b >= 2:
                        eng.wait_ge(s_badd, l * NBLK + b - 1)
                    pslot = p_agg2[pa % 2][0:D, 0:128]
                    inst = eng.activation(
                        tmpB[:, b % 2, :], pslot, ACTF.Copy)
                    inst.then_inc(s_cpb, 1)
                    if TRUNC == 'AGG' or (TRUNC == 'G' and l == 1):
                        continue
                    if nextk < c_.NKC and b == min(4 * (nextk + 1) + 1,
                                                   NBLK - 1):
                        c_scalar(nextk)
                        nextk += 1
                if TRUNC == 'AGG' or (TRUNC == 'G' and l == 1):
                    return
                while nextk < c_.NKC:
                    c_scalar(nextk)
                    nextk += 1
                if TRUNC == 'C':
                    return
                # --- phase D: sqrt(var + eps) ---
                eng.wait_ge(s_var, l + 1)
                eng.activation(sd3[:, 0:3], var3[:, 0:3], ACTF.Sqrt,
                               bias=eps_sb[:, 0:1]).then_inc(s_sqr, 1)
                # --- phase E: relu (halves) ---
                if l < L - 1:
                    h_new = h_bufs[(l + 1) % 2]
                    eng.wait_ge(s_acc, 2 * l + 1)
                    eng.activation(h_new[:, 0:c_.H1], x_aggrT[:, 0:c_.H1],
                                   ACTF.Relu, bias=tv[:, 0:1])
                    eng.drain().then_inc(s_hn, 1)
                    eng.wait_ge(s_acc, 2 * l + 2)
                    eng.activation(h_new[:, c_.H1:NPC], x_aggrT[:, c_.H1:NPC],
                                   ACTF.Relu, bias=tv[:, 0:1])
                    if NPAD > NPC:
                        eng.activation(h_new[:, NPC:NPAD],
                                       x_aggrT[:, NPC:NPAD],
                                       ACTF.Copy, scale=0.0)
                    eng.drain().then_inc(s_hn, 1)
                else:
                    # vector wrote h_new directly (tensor_scalar_add)
                    eng.wait_ge(s_acc, 2 * l + 1)
                    eng.drain().then_inc(s_hn, 1)
                    eng.wait_ge(s_acc, 2 * l + 2)
                    eng.drain().then_inc(s_hn, 1)
                if TRUNC == 'E':
                    return
                # --- phase F: psum -> stage ---
                for t in range(NBLK):
                    gt = l * NBLK + t
                    if gt >= 2:
                        lp, tp = divmod(gt - 2, NBLK)
                        si, cnt = tile_sem(lp, tp)
                        eng.wait_ge(s_dt[si], cnt)
                    eng.wait_ge(s_pe3, pe3_val(l, t))
                    if l < L - 1:
                        inst = eng.activation(
                            stageh[:, t % 2, 0:D],
                            p_t2[t % 2][0:128, 0:64],
                            ACTF.Copy)
                    else:
                        inst = eng.activation(
                            stage[:, t % 2, :],
                            p_t2[t % 2][0:128, 0:64],
                            ACTF.Copy)
                    inst.then_inc(s_s3, 1)
                if TRUNC == 'F':
                    return

    ctx.close()
    nc.finalize()
    return nc


def _host_pack(cfg, W, pw1, pw2, pb1, bn_g, bn_b, pbn_g, pbn_b, gating):
    L, D, BOT = cfg.L, cfg.D, cfg.BOT
    wt = np.ascontiguousarray(W.transpose(1, 0, 2).reshape(D, L * D))
    pw1t = np.ascontiguousarray(
        pw1.transpose(2, 0, 1, 3).reshape(D, 2 * L * BOT))
    pw2t = np.zeros((16, 2 * L * D), np.float32)
    pw2t[0:BOT] = pw2.transpose(2, 0, 1, 3).reshape(BOT, 2 * L * D)
    pb1t = np.zeros((16, 2 * L), np.float32)
    pb1t[0:BOT] = pb1.transpose(2, 0, 1).reshape(BOT, 2 * L)
    gb = np.zeros((D, 6 * L), np.float32)
    for l in range(L):
        g0 = gating[0, l, 0]
        g1 = gating[1, l, 0]
        gb[:, 6 * l + 0] = bn_g[l]
        gb[:, 6 * l + 1] = pbn_g[0, l] * g0
        gb[:, 6 * l + 2] = pbn_g[1, l] * g1
        gb[:, 6 * l + 3] = bn_b[l]
        gb[:, 6 * l + 4] = pbn_b[0, l] * g0
        gb[:, 6 * l + 5] = pbn_b[1, l] * g1
    import ml_dtypes
    bf16 = ml_dtypes.bfloat16
    iota = np.tile(np.arange(128, dtype=np.float32), (128, 1)).astype(bf16)
    idn = np.eye(128, dtype=np.float32)
    idnh = np.eye(128, dtype=np.float32).astype(bf16)
    return dict(wt=wt, pw1t=pw1t, pw2t=pw2t, pb1t=pb1t, gbvec=gb,
                iota=np.ascontiguousarray(iota), idn=idn, idnh=idnh)


def make_in_maps(cfg, inputs):
    c_ = cfg
    x = np.asarray(inputs["x"], np.float32)
    edge_index = np.asarray(inputs["edge_index"])
    gidx_w, drel = _prep_tokens(c_, edge_index)
    packs = _host_pack(c_, *[np.asarray(inputs[k], np.float32) for k in
                             ("W", "pw1", "pw2", "pb1", "bn_g", "bn_b",
                              "pbn_g", "pbn_b", "gating")])
    import ml_dtypes
    bf16 = ml_dtypes.bfloat16
    xpadA = np.zeros((c_.GA, 128), bf16)
    xpadB = np.zeros((c_.GB, 128), bf16)
    for o in range(c_.NC):
        xpadA[o * c_.H1:(o + 1) * c_.H1, 0:c_.D] = \
            x[o * c_.NPC:o * c_.NPC + c_.H1].astype(bf16)
        nreal = c_.NPC - c_.H1
        xpadB[o * c_.H2:o * c_.H2 + nreal, 0:c_.D] = \
            x[o * c_.NPC + c_.H1:(o + 1) * c_.NPC].astype(bf16)
    in_maps = []
    for i in range(c_.NC):
        m = dict(packs)
        m["x_fullA"] = xpadA
        m["x_fullB"] = xpadB
        m["x_own"] = np.ascontiguousarray(
            x[i * c_.NPC:(i + 1) * c_.NPC])
        m["gidx"] = np.ascontiguousarray(gidx_w[i])
        m["drel"] = np.ascontiguousarray(drel[i].astype(bf16))
        in_maps.append(m)
    return in_maps


def _make_cfg(inputs, N=50000, E=800000, D=64, L=5, BOT=15, NC=8, NQ=4):
    edge_index = np.asarray(inputs["edge_index"])
    capA, capB = _caps_from_edges(dict(N=N, NC=NC), edge_index)
    return Cfg(N, E, D, L, BOT, NC, capA, capB, NQ=NQ)


_GRAPH_CACHE = {}


def kernel(**inputs) -> np.ndarray:
    cfg = _make_cfg(inputs)
    key = (tuple(cfg.capA), tuple(cfg.capB))
    if key not in _GRAPH_CACHE:
        _GRAPH_CACHE[key] = build_graph(cfg)
    nc = _GRAPH_CACHE[key]
    in_maps = make_in_maps(cfg, inputs)
    res = run_bass_kernel_spmd(nc, in_maps, core_ids=list(range(cfg.NC)))
    outs = [res.results[i]["out"] for i in range(cfg.NC)]
    return np.concatenate(outs, axis=0)
